# revision 1
# baseline (speedup 1.0000x reference)
"""BiWKV6 encoder kernel for 8 Trainium2 NeuronCores.

Sharding: (batch, direction) -> 8 units, one per core; core c handles
batch c % 4, direction c // 4. Backward cores run the identical SPMD
program on time-reversed inputs; the only cross-core communication is a
pairwise AllGather of each layer's block output, written time-reversed
into the partner's domain. Within a core activations are channel-major
[C, T]; the WKV scan uses the chunked linear-attention formulation
(chunk 128) with log-space cumulative decay from the DVE prefix scan.
"""
import numpy as np

import concourse.bass as bass
import concourse.tile as tile
from concourse import bacc, mybir
from concourse.bass_utils import run_bass_kernel_spmd

B, T, C = 4, 1024, 512
H, HN = 8, 64
L = 128
TTW = 512
NTT = T // TTW
NCPT = TTW // L
CT = C // 128
TM, TD, FFN, NL = 32, 64, 1792, 2
NFF = FFN // 128
EPS_LN, EPS_GN = 1e-5, 64e-5
NV = 19

F32 = mybir.dt.float32
F32R = mybir.dt.float32r
AF = mybir.ActivationFunctionType
OP = mybir.AluOpType

WB_R, WB_K, WB_V, WB_G, WB_TM1, WB_TD1, WB_O = 0, 512, 1024, 1536, 2048, 2208, 2272
WB_COLS = 2784
(V_LN1W, V_LN1B, V_LN2W, V_LN2B, V_MAAX, V_MAAW, V_MAAK, V_MAAV, V_MAAR,
 V_MAAG, V_TDCY, V_CMK, V_CMR, V_GBM, V_CW0, V_CW1, V_CW2, V_LN0W,
 V_LN0B) = range(NV)
S_LN0, S_NEGS, S_ALPHA, S_BETA = 0, 1, 2, 3

_CACHE = {}


def _revap(ap):
    n = ap.ap[-1][1]
    return bass.AP(tensor=ap.tensor, offset=ap.offset + (n - 1) * ap.ap[-1][0],
                   ap=[ap.ap[0], [-ap.ap[-1][0], n]])


def _build(dbg=False, solo=False):
    nc = bacc.Bacc("TRN2", target_bir_lowering=False, debug=False, num_devices=8)

    x0 = nc.declare_dram_parameter("x0", [C, T], F32, isOutput=False)
    mask05 = nc.declare_dram_parameter("mask05", [128, T], F32, isOutput=False)
    sel_in = nc.declare_dram_parameter("sel", [128, 8], F32, isOutput=False)
    consts = nc.declare_dram_parameter("consts", [128, 384], F32, isOutput=False)
    wbig, tmw2, tdw2, cmkp, cmvp, cmrg, vecs_in, lnx_in, hmu_in = \
        [], [], [], [], [], [], [], [], []
    for l in range(NL):
        wbig.append(nc.declare_dram_parameter(f"wbig{l}", [C, WB_COLS], F32, isOutput=False))
        tmw2.append(nc.declare_dram_parameter(f"tmw2{l}", [5 * TM, C], F32, isOutput=False))
        tdw2.append(nc.declare_dram_parameter(f"tdw2{l}", [TD, C], F32, isOutput=False))
        cmkp.append(nc.declare_dram_parameter(f"cmk{l}", [NFF, C, 128], F32, isOutput=False))
        cmvp.append(nc.declare_dram_parameter(f"cmv{l}", [FFN, C], F32, isOutput=False))
        cmrg.append(nc.declare_dram_parameter(f"cmrg{l}", [C, 1024], F32, isOutput=False))
        vecs_in.append(nc.declare_dram_parameter(f"vecs{l}", [C, NV], F32, isOutput=False))
        lnx_in.append(nc.declare_dram_parameter(f"lnx{l}", [128, 1024], F32, isOutput=False))
        hmu_in.append(nc.declare_dram_parameter(f"hmu{l}", [C, 8], F32, isOutput=False))
    xout = nc.declare_dram_parameter("xout", [C, T], F32, isOutput=True)
    dbg_o = {}
    if dbg:
        for nm in ["xt0", "r0", "lai0", "xbtm0", "p0", "w0"]:
            dbg_o[nm] = nc.declare_dram_parameter(nm, [C, T], F32, isOutput=True)
        for nm in ["y0tm", "g0tm"]:
            dbg_o[nm] = nc.declare_dram_parameter(nm, [T, C], F32, isOutput=True)

    groups = [[0, 4], [1, 5], [2, 6], [3, 7]]

    with tile.TileContext(nc) as tc:
        with (
            tc.tile_pool(name="pp", bufs=1) as pp,
            tc.tile_pool(name="wp", bufs=2) as wp,
            tc.tile_pool(name="kp", bufs=1) as kp,
            tc.tile_pool(name="k2", bufs=2) as k2,
            tc.tile_pool(name="psA", bufs=1, space="PSUM") as psA,
            tc.tile_pool(name="psB", bufs=2, space="PSUM") as psB,
            tc.tile_pool(name="psC", bufs=1, space="PSUM") as psC,
            tc.tile_pool(name="dp", bufs=2, space="DRAM") as dp,
        ):
            # ------------- persistent loads -------------
            xres = [pp.tile([128, T], F32R, tag=f"xres{i}", name=f"xres{i}") for i in range(CT)]
            xb = [pp.tile([128, T], F32R, tag=f"xb{i}", name=f"xb{i}") for i in range(CT)]
            for i in range(CT):
                nc.sync.dma_start(out=xres[i],
                                  in_=x0[i * 128:(i + 1) * 128, :].bitcast(F32R))
            maskt = pp.tile([128, T], F32, tag="mask", name="mask")
            nc.sync.dma_start(out=maskt, in_=mask05[:, :])
            selt = pp.tile([128, 8], F32, tag="sel", name="sel")
            nc.sync.dma_start(out=selt, in_=sel_in[:, :])
            cst = pp.tile([128, 384], F32, tag="consts", name="consts")
            nc.sync.dma_start(out=cst, in_=consts[:, :])
            eps_ln_t = pp.tile([128, 1], F32, tag="epsln", name="epsln")
            nc.vector.memset(eps_ln_t, EPS_LN)
            eps_gn_t = pp.tile([128, 1], F32, tag="epsgn", name="epsgn")
            nc.vector.memset(eps_gn_t, EPS_GN)
            ident = cst[:, 0:128]
            triu = cst[:, 128:256]
            onesr_t = pp.tile([128, 128], F32R, tag="onesr", name="onesr")
            nc.sync.dma_start(out=onesr_t, in_=consts[:, 256:384].bitcast(F32R))
            ones_r = onesr_t

            def vcol(vt, i, j):
                return vt[i][:, j:j + 1]

            def load_w(dram_ap, shape, tag, bufs=2):
                t = wp.tile(shape, F32R, tag=tag, name="wld", bufs=bufs)
                nc.sync.dma_start(out=t, in_=dram_ap.bitcast(F32R))
                return t

            def load_wblk(dram_2d, shape):
                # one DMA for a [C, w] weight block -> SBUF [128, CT, w]
                t = wp.tile(shape, F32R, tag="wblk", name="wblk", bufs=2)
                nc.sync.dma_start(
                    out=t, in_=dram_2d.rearrange("(k p) n -> p k n", p=128).bitcast(F32R))
                return t

            def ln_stats(src_sl):
                ssum = psC.tile([1, TTW], F32, tag="stA", name="stA")
                ssq = psC.tile([1, TTW], F32, tag="stB", name="stB")
                for i in range(CT):
                    sq = k2.tile([128, TTW], F32R, tag="lnt1", name="lnt1")
                    nc.vector.tensor_mul(out=sq, in0=src_sl[i], in1=src_sl[i])
                    nc.tensor.matmul(out=ssum, lhsT=ones_r[:, 0:1], rhs=src_sl[i],
                                     start=(i == 0), stop=(i == CT - 1))
                    nc.tensor.matmul(out=ssq, lhsT=ones_r[:, 0:1], rhs=sq,
                                     start=(i == 0), stop=(i == CT - 1))
                rows = k2.tile([128, TTW], F32, tag="lnrows", name="lnrows", bufs=1)
                srow, s2, varu, lnv = (rows[j:j + 1, :] for j in (0, 32, 64, 96))
                nc.scalar.activation(out=srow, in_=ssum, func=AF.Copy)
                nc.vector.tensor_mul(out=s2, in0=srow, in1=srow)
                nc.vector.scalar_tensor_tensor(out=varu, in0=s2, scalar=-1.0 / C,
                                               in1=ssq, op0=OP.mult, op1=OP.add)
                nc.scalar.activation(out=lnv, in_=varu, func=AF.Ln, scale=1.0 / C,
                                     bias=eps_ln_t[0:1, :])
                rs = k2.tile([1, TTW], F32R, tag="lnrs", name="lnrs", bufs=1)
                nc.scalar.activation(out=rs, in_=lnv, func=AF.Exp, scale=-0.5)
                murs = k2.tile([1, TTW], F32R, tag="lnmu", name="lnmu", bufs=1)
                nc.vector.scalar_tensor_tensor(out=murs, in0=srow, scalar=1.0 / C,
                                               in1=rs, op0=OP.mult, op1=OP.mult)
                bc0 = psB.tile([128, TTW], F32, tag="pw", name="pw")
                nc.tensor.matmul(out=bc0, lhsT=ones_r[0:1, 0:128], rhs=rs,
                                 start=True, stop=True)
                bc1 = psB.tile([128, TTW], F32, tag="pw", name="pw")
                nc.tensor.matmul(out=bc1, lhsT=ones_r[0:1, 0:128], rhs=murs,
                                 start=True, stop=True)
                return bc0, bc1

            def ln_apply(src_sl, bc0, bc1, vt, wi, bi, out_tiles):
                for i in range(CT):
                    t1 = k2.tile([128, TTW], F32, tag="lnt1", name="lnt1")
                    nc.vector.tensor_mul(out=t1, in0=src_sl[i], in1=bc0)
                    t2 = k2.tile([128, TTW], F32, tag="lnt2", name="lnt2")
                    nc.vector.tensor_sub(out=t2, in0=t1, in1=bc1)
                    nc.vector.tensor_scalar(out=out_tiles[i], in0=t2,
                                            scalar1=vcol(vt, i, wi),
                                            scalar2=vcol(vt, i, bi),
                                            op0=OP.mult, op1=OP.add)

            def tanh_route(psum_ap, out_tile, scale=2.0):
                p, q = psum_ap.shape[0], psum_ap.shape[1]
                e = k2.tile([p, q], F32, tag="er0", name="er0", bufs=2)
                nc.scalar.activation(out=e, in_=psum_ap, func=AF.Exp, scale=scale)
                den = k2.tile([p, q], F32, tag="er1", name="er1", bufs=2)
                nc.vector.tensor_scalar_add(out=den, in0=e, scalar1=1.0)
                nc.vector.reciprocal(out=den, in_=den)
                nc.vector.tensor_scalar_add(out=e, in0=e, scalar1=-1.0)
                nc.vector.tensor_mul(out=out_tile, in0=e, in1=den)

            # ================= layers =================
            for l in range(NL):
                vecs = []
                for i in range(CT):
                    vt = pp.tile([128, NV], F32, tag=f"vecs{i}", name=f"vecs{i}")
                    nc.sync.dma_start(out=vt, in_=vecs_in[l][i * 128:(i + 1) * 128, :])
                    vecs.append(vt)
                lnxt = pp.tile([128, 1024], F32, tag="lnx", name="lnx")
                nc.sync.dma_start(out=lnxt, in_=lnx_in[l][:, :])
                hmu = []
                for i in range(CT):
                    ht = pp.tile([128, 8], F32, tag=f"hmu{i}", name=f"hmu{i}")
                    nc.sync.dma_start(out=ht, in_=hmu_in[l][i * 128:(i + 1) * 128, :])
                    hmu.append(ht)

                # ---- xb init ----
                if l == 0:
                    for tt in range(NTT):
                        sl = slice(tt * TTW, (tt + 1) * TTW)
                        src = [xres[i][:, sl] for i in range(CT)]
                        bc0, bc1 = ln_stats(src)
                        xbs = [xb[i][:, sl] for i in range(CT)]
                        ln_apply(src, bc0, bc1, vecs, V_LN0W, V_LN0B, xbs)
                        for i in range(CT):
                            d = k2.tile([128, TTW], F32, tag="lnt2", name="lnt2")
                            nc.vector.tensor_sub(out=d, in0=xb[i][:, sl],
                                                 in1=xres[i][:, sl])
                            nc.vector.scalar_tensor_tensor(
                                out=xb[i][:, sl], in0=d,
                                scalar=selt[:, S_LN0:S_LN0 + 1],
                                in1=xres[i][:, sl], op0=OP.mult, op1=OP.add)
                else:
                    for i in range(CT):
                        nc.gpsimd.tensor_copy(out=xb[i], in_=xres[i])

                S_cur = [pp.tile([128, HN], F32, tag=f"S{i}", name=f"S{i}") for i in range(CT)]
                for i in range(CT):
                    nc.vector.memset(S_cur[i], 0.0)
                carry = [pp.tile([128, 1], F32, tag=f"ca{i}", name=f"ca{i}") for i in range(CT)]
                carry2 = [pp.tile([128, 1], F32, tag=f"cb{i}", name=f"cb{i}") for i in range(CT)]
                for i in range(CT):
                    nc.gpsimd.memset(carry[i], 0.0)
                    nc.gpsimd.memset(carry2[i], 0.0)

                # ================= time mix =================
                for tt in range(NTT):
                    sl = slice(tt * TTW, (tt + 1) * TTW)
                    xbs = [xb[i][:, sl] for i in range(CT)]
                    bc0, bc1 = ln_stats(xbs)
                    xt = [kp.tile([128, TTW], F32, tag=f"xt{i}", name=f"xt{i}") for i in range(CT)]
                    ln_apply(xbs, bc0, bc1, vecs, V_LN1W, V_LN1B, xt)
                    xx = [kp.tile([128, TTW], F32, tag=f"xx{i}", name=f"xx{i}") for i in range(CT)]
                    for i in range(CT):
                        nc.vector.tensor_sub(out=xx[i][:, 1:TTW],
                                             in0=xt[i][:, 0:TTW - 1],
                                             in1=xt[i][:, 1:TTW])
                        nc.vector.scalar_tensor_tensor(
                            out=xx[i][:, 0:1], in0=carry[i], scalar=1.0,
                            in1=xt[i][:, 0:1], op0=OP.mult, op1=OP.subtract)
                        nc.gpsimd.tensor_copy(out=carry[i], in_=xt[i][:, TTW - 1:TTW])
                    if dbg and l == 0:
                        for i in range(CT):
                            nc.sync.dma_start(out=dbg_o["xt0"][i * 128:(i + 1) * 128, sl],
                                              in_=xt[i])

                    # ---- t5 ----
                    mx = [k2.tile([128, TTW], F32R, tag=f"xf{i}", name=f"xf{i}") for i in range(CT)]
                    for i in range(CT):
                        nc.vector.scalar_tensor_tensor(
                            out=mx[i], in0=xx[i], scalar=vcol(vecs, i, V_MAAX),
                            in1=xt[i], op0=OP.mult, op1=OP.add)
                    p160a = psC.tile([128, TTW], F32, tag="stA", name="stA")
                    p160b = psC.tile([32, TTW], F32, tag="stB", name="stB")
                    wtmtd = wp.tile([128, CT, 224], F32R, tag="wtmtd", name="wtmtd",
                                    bufs=1)
                    nc.sync.dma_start(
                        out=wtmtd,
                        in_=wbig[l][:, WB_TM1:WB_TM1 + 224]
                        .rearrange("(k p) n -> p k n", p=128).bitcast(F32R))
                    for i in range(CT):
                        nc.tensor.matmul(out=p160a, lhsT=wtmtd[:, i, 0:128], rhs=mx[i],
                                         start=(i == 0), stop=(i == CT - 1))
                        nc.tensor.matmul(out=p160b, lhsT=wtmtd[:, i, 128:160], rhs=mx[i],
                                         start=(i == 0), stop=(i == CT - 1))
                    t5 = [k2.tile([32, TTW], F32R, tag=f"t5{f}", name=f"t5{f}", bufs=1) for f in range(5)]
                    for f in range(4):
                        tanh_route(p160a[f * 32:(f + 1) * 32, :], t5[f])
                    tanh_route(p160b, t5[4])

                    def build_xf(fidx, maa_i):
                        w2 = load_w(tmw2[l][fidx * TM:(fidx + 1) * TM, :],
                                    [TM, C], "wtm2")
                        xft = []
                        for i in range(CT):
                            dlp = psB.tile([128, TTW], F32, tag="pw", name="pw")
                            nc.tensor.matmul(out=dlp,
                                             lhsT=w2[:, i * 128:(i + 1) * 128],
                                             rhs=t5[fidx], start=True, stop=True)
                            a = k2.tile([128, TTW], F32, tag="lnt1", name="lnt1")
                            nc.vector.scalar_tensor_tensor(
                                out=a, in0=dlp, scalar=vcol(vecs, i, maa_i),
                                in1=xx[i], op0=OP.add, op1=OP.mult)
                            xf = k2.tile([128, TTW], F32R, tag=f"xf{i}", name=f"xf{i}")
                            nc.vector.tensor_add(out=xf, in0=a, in1=xt[i])
                            xft.append(xf)
                        return xft

                    def wmm(col_off, xft):
                        accs = [psA.tile([128, TTW], F32, tag=f"acc{m}",
                                         name=f"acc{m}") for m in range(4)]
                        wt = load_wblk(wbig[l][:, col_off:col_off + 512],
                                       [128, CT, 512])
                        for i in range(CT):
                            for m in range(4):
                                nc.tensor.matmul(out=accs[m],
                                                 lhsT=wt[:, i, m * 128:(m + 1) * 128],
                                                 rhs=xft[i], start=(i == 0),
                                                 stop=(i == CT - 1))
                        return accs

                    def wmm_tm(col_off, xft):
                        accs = [psA.tile([128, TTW], F32, tag=f"acc{m}",
                                         name=f"acc{m}") for m in range(4)]
                        wt = load_wblk(wbig[l][:, col_off:col_off + 512],
                                       [128, CT, 512])
                        for i in range(CT):
                            for ci in range(NCPT):
                                nc.tensor.matmul(out=accs[ci],
                                                 lhsT=xft[i][:, ci * L:(ci + 1) * L],
                                                 rhs=wt[:, i, :], start=(i == 0),
                                                 stop=(i == CT - 1))
                        return accs

                    # k
                    xf = build_xf(1, V_MAAK)
                    accs = wmm(WB_K, xf)
                    k_sb = [kp.tile([128, TTW], F32, tag=f"ksb{i}", name=f"ksb{i}") for i in range(CT)]
                    for m in range(4):
                        nc.scalar.activation(out=k_sb[m], in_=accs[m], func=AF.Copy)
                    # v token-major
                    xf = build_xf(2, V_MAAV)
                    accs = wmm_tm(WB_V, xf)
                    v_tm = [kp.tile([128, C], F32, tag=f"vtm{ci}", name=f"vtm{ci}") for ci in range(NCPT)]
                    for ci in range(NCPT):
                        nc.scalar.activation(out=v_tm[ci], in_=accs[ci], func=AF.Copy)
                    # r
                    xf = build_xf(3, V_MAAR)
                    accs = wmm(WB_R, xf)
                    r_sb = [kp.tile([128, TTW], F32, tag=f"rsb{i}", name=f"rsb{i}") for i in range(CT)]
                    for m in range(4):
                        nc.scalar.activation(out=r_sb[m], in_=accs[m], func=AF.Copy)
                    if dbg and l == 0:
                        for i in range(CT):
                            nc.sync.dma_start(out=dbg_o["r0"][i * 128:(i + 1) * 128, sl],
                                              in_=r_sb[i])
                    # g token-major, silu
                    xf = build_xf(4, V_MAAG)
                    accs = wmm_tm(WB_G, xf)
                    g_tm = [kp.tile([128, C], F32, tag=f"gtm{ci}", name=f"gtm{ci}") for ci in range(NCPT)]
                    for ci in range(NCPT):
                        e = k2.tile([128, C], F32, tag="er0", name="er0", bufs=2)
                        nc.scalar.activation(out=e, in_=accs[ci], func=AF.Exp, scale=-1.0)
                        den = k2.tile([128, C], F32, tag="er1", name="er1", bufs=2)
                        nc.vector.tensor_scalar_add(out=den, in0=e, scalar1=1.0)
                        nc.vector.reciprocal(out=den, in_=den)
                        nc.vector.tensor_mul(out=g_tm[ci], in0=den, in1=accs[ci])
                    # w -> wacc -> lai
                    xf = build_xf(0, V_MAAW)
                    tdp = psC.tile([TD, TTW], F32, tag="stA", name="stA")
                    for i in range(CT):
                        nc.tensor.matmul(out=tdp, lhsT=wtmtd[:, i, 160:224], rhs=xf[i],
                                         start=(i == 0), stop=(i == CT - 1))
                    tdt = k2.tile([TD, TTW], F32R, tag="tdt", name="tdt", bufs=1)
                    tanh_route(tdp, tdt)
                    w2t = load_w(tdw2[l][:, :], [TD, C], "wtd2", bufs=1)
                    lai = [kp.tile([128, 1 + TTW], F32, tag=f"lai{i}", name=f"lai{i}") for i in range(CT)]
                    for i in range(CT):
                        wwp = psB.tile([128, TTW], F32, tag="pw", name="pw")
                        nc.tensor.matmul(out=wwp, lhsT=w2t[:, i * 128:(i + 1) * 128],
                                         rhs=tdt, start=True, stop=True)
                        wacc = k2.tile([128, TTW], F32, tag="lnt1", name="lnt1")
                        nc.scalar.activation(out=wacc, in_=wwp, func=AF.Exp,
                                             bias=vcol(vecs, i, V_TDCY))
                        nc.gpsimd.memset(lai[i][:, 0:1], 0.0)
                        nc.vector.tensor_tensor_scan(
                            out=lai[i][:, 1:1 + TTW], data0=wacc, data1=wacc,
                            initial=0.0, op0=OP.add, op1=OP.bypass)
                        if dbg and l == 0:
                            nc.sync.dma_start(
                                out=dbg_o["w0"][i * 128:(i + 1) * 128, sl], in_=wacc)
                            nc.sync.dma_start(
                                out=dbg_o["lai0"][i * 128:(i + 1) * 128, sl],
                                in_=lai[i][:, 1:1 + TTW])

                    # ---- wkv chunks ----
                    ztc = [kp.tile([128, TTW], F32R, tag=f"xt{i}", name=f"ztc{i}") for i in range(CT)]
                    for ci in range(NCPT):
                        gc = tt * NCPT + ci
                        c0 = ci * L
                        fpc, fnc, rt_t, kt_t, kh_tm, m_t = [], [], [], [], [], []
                        for i in range(CT):
                            ngc = k2.tile([128, 1], F32, tag="ngc", name="ngc", bufs=4)
                            nc.vector.tensor_scalar_mul(out=ngc,
                                                        in0=lai[i][:, c0:c0 + 1],
                                                        scalar1=-1.0)
                            fp = k2.tile([128, 1 + L], F32, tag="fp", name="fp", bufs=4)
                            nc.scalar.activation(out=fp, in_=lai[i][:, c0:c0 + 1 + L],
                                                 func=AF.Exp, bias=ngc)
                            fn = k2.tile([128, 1 + L], F32, tag="fn", name="fn", bufs=4)
                            nc.scalar.activation(out=fn, in_=lai[i][:, c0:c0 + 1 + L],
                                                 func=AF.Exp, scale=-1.0,
                                                 bias=lai[i][:, c0:c0 + 1])
                            fpc.append(fp)
                            fnc.append(fn)
                            rt = k2.tile([128, L], F32, tag="rt", name="rt", bufs=4)
                            nc.vector.tensor_mul(out=rt, in0=r_sb[i][:, c0:c0 + L],
                                                 in1=fn[:, 0:L])
                            kt = k2.tile([128, L], F32, tag="kt", name="kt", bufs=4)
                            nc.vector.tensor_mul(out=kt, in0=k_sb[i][:, c0:c0 + L],
                                                 in1=fp[:, 1:1 + L])
                            kh = k2.tile([128, L], F32, tag="kh", name="kh", bufs=4)
                            nc.vector.tensor_scalar_mul(out=kh, in0=kt,
                                                        scalar1=fn[:, L:L + 1])
                            mt = k2.tile([128, L], F32, tag="mt", name="mt", bufs=4)
                            nc.gpsimd.tensor_mul(out=mt, in0=r_sb[i][:, c0:c0 + L],
                                                 in1=k_sb[i][:, c0:c0 + L])
                            rt_t.append(rt)
                            kt_t.append(kt)
                            m_t.append(mt)
                            trp = psB.tile([128, L], F32, tag="pw", name="pw")
                            nc.tensor.transpose(out=trp, in_=kh, identity=ident)
                            kht = k2.tile([128, L], F32, tag="khtm", name="khtm", bufs=4)
                            nc.scalar.activation(out=kht, in_=trp, func=AF.Copy)
                            kh_tm.append(kht)
                        dall = psC.tile([128, 8], F32, tag="stB", name="stB")
                        for i in range(CT):
                            nc.tensor.matmul(out=dall, lhsT=m_t[i], rhs=hmu[i],
                                             start=(i == 0), stop=(i == CT - 1))
                        yps = psA.tile([128, C], F32, tag="acc0", name="acc0")
                        S_new = [k2.tile([128, HN], F32, tag=f"Sn{i}", name=f"Sn{i}") for i in range(CT)]
                        for i in range(CT):
                            sup = psC.tile([128, HN], F32, tag="stA", name="stA")
                            for hh in range(2):
                                h = 2 * i + hh
                                hb = hh * HN
                                pt = psB.tile([L, L], F32, tag="pw", name="pw")
                                nc.tensor.matmul(out=pt, lhsT=kt_t[i][hb:hb + HN, :],
                                                 rhs=rt_t[i][hb:hb + HN, :],
                                                 start=True, stop=True)
                                pts = k2.tile([L, L], F32, tag="pts", name="pts")
                                nc.vector.tensor_mul(out=pts, in0=pt, in1=triu)
                                nc.tensor.matmul(
                                    out=yps[:, h * HN:(h + 1) * HN], lhsT=pts,
                                    rhs=v_tm[ci][:, h * HN:(h + 1) * HN],
                                    start=True, stop=(gc == 0), skip_group_check=True)
                                if gc > 0:
                                    nc.tensor.matmul(
                                        out=yps[:, h * HN:(h + 1) * HN],
                                        lhsT=rt_t[i][hb:hb + HN, :],
                                        rhs=S_cur[i][hb:hb + HN, :],
                                        start=False, stop=True, skip_group_check=True)
                                nc.tensor.matmul(
                                    out=sup[hb:hb + HN, :],
                                    lhsT=kh_tm[i][:, hb:hb + HN],
                                    rhs=v_tm[ci][:, h * HN:(h + 1) * HN],
                                    start=True, stop=True, skip_group_check=True)
                            t0 = k2.tile([128, HN], F32, tag="ssc", name="ssc", bufs=4)
                            nc.vector.tensor_scalar_mul(out=t0, in0=S_cur[i],
                                                        scalar1=fnc[i][:, L:L + 1])
                            nc.vector.tensor_add(out=S_new[i], in0=t0, in1=sup)
                        S_cur = S_new
                        ysb = k2.tile([128, C], F32, tag="ysb", name="ysb")
                        for h in range(H):
                            nc.vector.scalar_tensor_tensor(
                                out=ysb[:, h * HN:(h + 1) * HN],
                                in0=v_tm[ci][:, h * HN:(h + 1) * HN],
                                scalar=dall[:, h:h + 1],
                                in1=yps[:, h * HN:(h + 1) * HN],
                                op0=OP.mult, op1=OP.add)
                        if dbg and l == 0:
                            nc.sync.dma_start(
                                out=dbg_o["y0tm"][tt * TTW + c0:tt * TTW + c0 + L, :],
                                in_=ysb)
                            nc.sync.dma_start(
                                out=dbg_o["g0tm"][tt * TTW + c0:tt * TTW + c0 + L, :],
                                in_=g_tm[ci])
                        # groupnorm + affine + *g  (token-major)
                        mv = k2.tile([128, 16], F32, tag="gnmv", name="gnmv")
                        for h in range(H):
                            st = k2.tile([128, 6], F32, tag="gnst", name="gnst")
                            nc.vector.bn_stats(out=st, in_=ysb[:, h * HN:(h + 1) * HN])
                            nc.vector.bn_aggr(out=mv[:, 2 * h:2 * h + 2], in_=st)
                        lnv = k2.tile([128, 8], F32, tag="gnln", name="gnln")
                        var_view = bass.AP(tensor=mv.tensor, offset=mv.offset + 1,
                                           ap=[mv.ap[0], [2, 8]])
                        nc.scalar.activation(out=lnv, in_=var_view, func=AF.Ln,
                                             bias=eps_gn_t)
                        rsg = k2.tile([128, 8], F32, tag="gnrs", name="gnrs")
                        nc.scalar.activation(out=rsg, in_=lnv, func=AF.Exp, scale=-0.5)
                        for h in range(H):
                            nc.vector.tensor_scalar(
                                out=ysb[:, h * HN:(h + 1) * HN],
                                in0=ysb[:, h * HN:(h + 1) * HN],
                                scalar1=mv[:, 2 * h:2 * h + 1],
                                scalar2=rsg[:, h:h + 1],
                                op0=OP.subtract, op1=OP.mult)
                        nc.gpsimd.tensor_mul(out=ysb, in0=ysb, in1=lnxt[:, 0:512])
                        nc.gpsimd.tensor_add(out=ysb, in0=ysb, in1=lnxt[:, 512:1024])
                        nc.vector.tensor_mul(out=ysb, in0=ysb, in1=g_tm[ci])
                        for i in range(CT):
                            trp = psB.tile([128, L], F32, tag="pw", name="pw")
                            nc.tensor.transpose(out=trp,
                                                in_=ysb[:, i * 128:(i + 1) * 128],
                                                identity=ident)
                            nc.scalar.activation(out=ztc[i][:, c0:c0 + L], in_=trp,
                                                 func=AF.Copy)
                    # ---- Wo ----
                    accs = [psA.tile([128, TTW], F32, tag=f"acc{m}",
                                     name=f"acc{m}") for m in range(4)]
                    wt = load_wblk(wbig[l][:, WB_O:WB_O + 512], [128, CT, 512])
                    for i in range(CT):
                        for m in range(4):
                            nc.tensor.matmul(out=accs[m],
                                             lhsT=wt[:, i, m * 128:(m + 1) * 128],
                                             rhs=ztc[i], start=(i == 0),
                                             stop=(i == CT - 1))
                    for m in range(4):
                        nc.vector.tensor_add(out=xb[m][:, sl], in0=xb[m][:, sl],
                                             in1=accs[m])
                    if dbg and l == 0:
                        for i in range(CT):
                            nc.sync.dma_start(
                                out=dbg_o["xbtm0"][i * 128:(i + 1) * 128, sl],
                                in_=xb[i][:, sl].bitcast(F32))

                # ================= channel mix =================
                for tt in range(NTT):
                    sl = slice(tt * TTW, (tt + 1) * TTW)
                    xbs = [xb[i][:, sl] for i in range(CT)]
                    bc0, bc1 = ln_stats(xbs)
                    xc = [kp.tile([128, TTW], F32, tag=f"xt{i}", name=f"xt{i}") for i in range(CT)]
                    ln_apply(xbs, bc0, bc1, vecs, V_LN2W, V_LN2B, xc)
                    xx2 = [kp.tile([128, TTW], F32, tag=f"xx{i}", name=f"xx{i}") for i in range(CT)]
                    for i in range(CT):
                        nc.vector.tensor_sub(out=xx2[i][:, 1:TTW],
                                             in0=xc[i][:, 0:TTW - 1],
                                             in1=xc[i][:, 1:TTW])
                        nc.vector.scalar_tensor_tensor(
                            out=xx2[i][:, 0:1], in0=carry2[i], scalar=1.0,
                            in1=xc[i][:, 0:1], op0=OP.mult, op1=OP.subtract)
                        nc.gpsimd.tensor_copy(out=carry2[i], in_=xc[i][:, TTW - 1:TTW])
                    xk2 = [kp.tile([128, TTW], F32R, tag=f"ksb{i}", name=f"ksb{i}") for i in range(CT)]
                    xr2 = [kp.tile([128, TTW], F32R, tag=f"rsb{i}", name=f"rsb{i}") for i in range(CT)]
                    for i in range(CT):
                        nc.vector.scalar_tensor_tensor(
                            out=xk2[i], in0=xx2[i], scalar=vcol(vecs, i, V_CMK),
                            in1=xc[i], op0=OP.mult, op1=OP.add)
                        nc.vector.scalar_tensor_tensor(
                            out=xr2[i], in0=xx2[i], scalar=vcol(vecs, i, V_CMR),
                            in1=xc[i], op0=OP.mult, op1=OP.add)
                    # cm_Wr -> sigmoid
                    accs = [psA.tile([128, TTW], F32, tag=f"acc{m}", name=f"acc{m}") for m in range(4)]
                    wt = load_wblk(cmrg[l][:, 0:512], [128, CT, 512])
                    for i in range(CT):
                        for m in range(4):
                            nc.tensor.matmul(out=accs[m],
                                             lhsT=wt[:, i, m * 128:(m + 1) * 128],
                                             rhs=xr2[i], start=(i == 0),
                                             stop=(i == CT - 1))
                    sig = [kp.tile([128, TTW], F32, tag=f"gtm{m}", name=f"sig{m}") for m in range(4)]
                    for m in range(4):
                        e = k2.tile([128, TTW], F32, tag="er0", name="er0", bufs=2)
                        nc.scalar.activation(out=e, in_=accs[m], func=AF.Exp, scale=-1.0)
                        nc.vector.tensor_scalar_add(out=e, in0=e, scalar1=1.0)
                        nc.vector.reciprocal(out=sig[m], in_=e)
                    # kk loop with cm_Wv accumulation
                    accs = [psA.tile([128, TTW], F32, tag=f"acc{m}", name=f"acc{m}") for m in range(4)]
                    wfq = wvq = None
                    for f in range(NFF):
                        fq, fr2 = f // 4, f % 4
                        nq = min(4, NFF - 4 * fq)
                        if fr2 == 0:
                            wfq = wp.tile([128, nq, CT, 128], F32R, tag="wblk",
                                          name="wfq", bufs=2)
                            nc.sync.dma_start(
                                out=wfq,
                                in_=cmkp[l][4 * fq:4 * fq + nq]
                                .rearrange("f (k p) n -> p f k n", p=128)
                                .bitcast(F32R))
                            wvq = wp.tile([128, nq, C], F32R, tag="wblk",
                                          name="wvq", bufs=2)
                            nc.sync.dma_start(
                                out=wvq,
                                in_=cmvp[l][4 * fq * 128:(4 * fq + nq) * 128, :]
                                .rearrange("(f p) n -> p f n", p=128)
                                .bitcast(F32R))
                        kkp = psB.tile([128, TTW], F32, tag="pw", name="pw")
                        for i in range(CT):
                            nc.tensor.matmul(out=kkp, lhsT=wfq[:, fr2, i, :],
                                             rhs=xk2[i],
                                             start=(i == 0), stop=(i == CT - 1))
                        rl = k2.tile([128, TTW], F32, tag="lnt1", name="lnt1")
                        nc.vector.tensor_scalar_max(out=rl, in0=kkp, scalar1=0.0)
                        kkf = k2.tile([128, TTW], F32R, tag="lnt2", name="lnt2")
                        nc.vector.tensor_mul(out=kkf, in0=rl, in1=kkp)
                        for m in range(4):
                            nc.tensor.matmul(out=accs[m],
                                             lhsT=wvq[:, fr2, m * 128:(m + 1) * 128],
                                             rhs=kkf, start=(f == 0),
                                             stop=(f == NFF - 1))
                    for m in range(4):
                        nc.vector.tensor_mul(out=sig[m], in0=sig[m], in1=accs[m])
                        nc.vector.tensor_add(out=xb[m][:, sl], in0=xb[m][:, sl],
                                             in1=sig[m])
                if dbg and l == 0:
                    for i in range(CT):
                        nc.sync.dma_start(out=dbg_o["p0"][i * 128:(i + 1) * 128, :],
                                          in_=xb[i].bitcast(F32))

                # ================= exchange + join =================
                send = dp.tile([C, T], F32, tag="send", name="send")
                recv = dp.tile([2 * C, T], F32, tag="recv", name="recv")
                for i in range(CT):
                    nc.sync.dma_start(out=_revap(send[i * 128:(i + 1) * 128, :]),
                                      in_=xb[i].bitcast(F32))
                if solo:
                    for i in range(CT):
                        nc.sync.dma_start(out=recv[i * 128:(i + 1) * 128, :],
                                          in_=xb[i].bitcast(F32))
                        nc.sync.dma_start(out=recv[C + i * 128:C + (i + 1) * 128, :],
                                          in_=xb[i].bitcast(F32))
                else:
                    nc.gpsimd.collective_compute(
                        "AllGather", OP.bypass, replica_groups=groups,
                        ins=[send.opt()], outs=[recv.opt()])
                for tt in range(NTT):
                    sl = slice(tt * TTW, (tt + 1) * TTW)
                    # conv residue, all ct (reuse ksb tags)
                    cv = [kp.tile([128, TTW], F32R, tag=f"ksb{i}", name=f"ksb{i}") for i in range(CT)]
                    a = tt * TTW
                    for i in range(CT):
                        nc.vector.tensor_scalar_mul(out=cv[i], in0=xres[i][:, sl],
                                                    scalar1=vcol(vecs, i, V_CW1))
                        lo = 1 if tt == 0 else 0
                        nc.vector.scalar_tensor_tensor(
                            out=cv[i][:, lo:TTW],
                            in0=xres[i][:, a + lo - 1:a + TTW - 1],
                            scalar=vcol(vecs, i, V_CW0),
                            in1=cv[i][:, lo:TTW], op0=OP.mult, op1=OP.add)
                        hi = TTW - 1 if tt == NTT - 1 else TTW
                        nc.vector.scalar_tensor_tensor(
                            out=cv[i][:, 0:hi],
                            in0=xres[i][:, a + 1:a + hi + 1],
                            scalar=vcol(vecs, i, V_CW2),
                            in1=cv[i][:, 0:hi], op0=OP.mult, op1=OP.add)
                    accs = [psA.tile([128, TTW], F32, tag=f"acc{m}", name=f"acc{m}") for m in range(4)]
                    wt = load_wblk(cmrg[l][:, 512:1024], [128, CT, 512])
                    for i in range(CT):
                        for m in range(4):
                            nc.tensor.matmul(out=accs[m],
                                             lhsT=wt[:, i, m * 128:(m + 1) * 128],
                                             rhs=cv[i], start=(i == 0),
                                             stop=(i == CT - 1))
                    for m in range(4):
                        jr0 = kp.tile([128, TTW], F32, tag="vtm0", name="vtm0")
                        jr1 = kp.tile([128, TTW], F32, tag="vtm1", name="vtm1")
                        nc.sync.dma_start(out=jr0, in_=recv[m * 128:(m + 1) * 128, sl])
                        nc.sync.dma_start(out=jr1,
                                          in_=recv[C + m * 128:C + (m + 1) * 128, sl])
                        nc.vector.tensor_scalar_mul(
                            out=jr0, in0=jr0, scalar1=selt[:, S_ALPHA:S_ALPHA + 1])
                        nc.vector.scalar_tensor_tensor(
                            out=jr1, in0=jr1, scalar=selt[:, S_BETA:S_BETA + 1],
                            in1=jr0, op0=OP.mult, op1=OP.add)
                        e = k2.tile([128, TTW], F32, tag="er0", name="er0", bufs=2)
                        nc.scalar.activation(out=e, in_=accs[m], func=AF.Exp,
                                             scale=selt[:, S_NEGS:S_NEGS + 1],
                                             bias=vcol(vecs, m, V_GBM))
                        wown = kp.tile([128, TTW], F32, tag="vtm3", name="vtm3")
                        nc.vector.tensor_scalar_add(out=wown, in0=e, scalar1=1.0)
                        nc.vector.reciprocal(out=wown, in_=wown)
                        nc.vector.tensor_mul(out=e, in0=wown, in1=e)  # w_recv
                        jsum = kp.tile([128, TTW], F32, tag="vtm2", name="vtm2")
                        nc.vector.tensor_mul(out=jsum, in0=wown, in1=xb[m][:, sl])
                        nc.vector.tensor_mul(out=jr1, in0=e, in1=jr1)
                        nc.vector.tensor_add(out=jsum, in0=jsum, in1=jr1)
                        # note: host mask05 already includes the 2x factor fold:
                        # mask05 = mask (not 0.5*mask) since sigmoid form used.
                        nc.vector.tensor_mul(out=xres[m][:, sl], in0=jsum,
                                             in1=maskt[:, sl])
            # ---- output ----
            for i in range(CT):
                nc.sync.dma_start(out=xout[i * 128:(i + 1) * 128, :],
                                  in_=xres[i].bitcast(F32))
    nc.compile()
    return nc


def _host_inputs(inputs):
    x = np.asarray(inputs["x"], np.float32)
    lengths = np.asarray(inputs["lengths"]).astype(np.int64)
    pos = np.arange(T, dtype=np.float32)[:, None]
    div = np.exp(np.arange(0, C, 2, dtype=np.float32) * (-np.log(10000.0) / C))
    pe = np.zeros((T, C), np.float32)
    pe[:, 0::2] = np.sin(pos * div)
    pe[:, 1::2] = np.cos(pos * div)
    mask = (np.arange(T)[None, :] < lengths[:, None]).astype(np.float32)

    consts = np.zeros((128, 384), np.float32)
    consts[:, 0:128] = np.eye(128, dtype=np.float32)
    consts[:, 128:256] = np.triu(np.ones((128, 128), np.float32), 1)
    consts[:, 256:384] = 1.0

    gw = np.asarray(inputs["gate_w"], np.float32)
    gb = np.asarray(inputs["gate_b"], np.float32)
    cw = np.asarray(inputs["conv_w"], np.float32)
    cb = np.asarray(inputs["conv_b"], np.float32)

    in_maps = []
    for c in range(8):
        b, d = c % 4, c // 4
        rev = d == 1
        s = -1.0 if rev else 1.0
        xin = (x[b] + pe)
        mrow = mask[b]
        if rev:
            xin = xin[::-1]
            mrow = mrow[::-1]
        m = {
            "x0": np.ascontiguousarray(xin.T),
            "mask05": np.ascontiguousarray(np.broadcast_to(mrow, (128, T))),
            "consts": consts,
        }
        sel = np.zeros((128, 8), np.float32)
        sel[:, S_LN0] = 0.0 if rev else 1.0
        sel[:, S_NEGS] = -s
        sel[:, S_ALPHA] = 1.0 if rev else 0.0
        sel[:, S_BETA] = 0.0 if rev else 1.0
        m["sel"] = sel
        for l in range(NL):
            W = {k: np.asarray(inputs[k], np.float32)[d, l]
                 for k in ["ln1_w", "ln1_b", "ln2_w", "ln2_b", "maa_x", "maa_w",
                           "maa_k", "maa_v", "maa_r", "maa_g", "tm_w1", "tm_w2",
                           "td_w1", "td_w2", "time_decay", "Wr", "Wk", "Wv",
                           "Wg", "Wo", "lnx_w", "lnx_b", "cm_maa_k", "cm_maa_r",
                           "cm_Wk", "cm_Wv", "cm_Wr", "time_faaaa"]}
            m[f"wbig{l}"] = np.ascontiguousarray(np.concatenate(
                [W["Wr"], W["Wk"], W["Wv"], W["Wg"], W["tm_w1"], W["td_w1"],
                 W["Wo"]], axis=1))
            m[f"tmw2{l}"] = np.ascontiguousarray(W["tm_w2"].reshape(5 * TM, C))
            m[f"tdw2{l}"] = np.ascontiguousarray(W["td_w2"])
            m[f"cmk{l}"] = np.ascontiguousarray(
                W["cm_Wk"].reshape(C, NFF, 128).transpose(1, 0, 2))
            m[f"cmv{l}"] = np.ascontiguousarray(W["cm_Wv"])
            m[f"cmrg{l}"] = np.ascontiguousarray(
                np.concatenate([W["cm_Wr"], gw[l]], axis=1))
            cwe = cw[l] if not rev else cw[l][:, ::-1]
            gbe = cb[l] @ gw[l] + gb[l]
            vec = np.zeros((C, NV), np.float32)
            vec[:, V_LN1W] = W["ln1_w"]; vec[:, V_LN1B] = W["ln1_b"]
            vec[:, V_LN2W] = W["ln2_w"]; vec[:, V_LN2B] = W["ln2_b"]
            vec[:, V_MAAX] = W["maa_x"]; vec[:, V_MAAW] = W["maa_w"]
            vec[:, V_MAAK] = W["maa_k"]; vec[:, V_MAAV] = W["maa_v"]
            vec[:, V_MAAR] = W["maa_r"]; vec[:, V_MAAG] = W["maa_g"]
            vec[:, V_TDCY] = W["time_decay"]
            vec[:, V_CMK] = W["cm_maa_k"]; vec[:, V_CMR] = W["cm_maa_r"]
            vec[:, V_GBM] = -s * gbe
            vec[:, V_CW0] = cwe[:, 0]
            vec[:, V_CW1] = cwe[:, 1] - 1.0
            vec[:, V_CW2] = cwe[:, 2]
            vec[:, V_LN0W] = np.asarray(inputs["ln0_w"], np.float32)
            vec[:, V_LN0B] = np.asarray(inputs["ln0_b"], np.float32)
            m[f"vecs{l}"] = vec
            lnx = np.zeros((128, 1024), np.float32)
            lnx[:, 0:512] = W["lnx_w"][None, :]
            lnx[:, 512:1024] = W["lnx_b"][None, :]
            m[f"lnx{l}"] = lnx
            u = W["time_faaaa"].reshape(C)
            hmu = np.zeros((C, 8), np.float32)
            for h in range(H):
                hmu[h * HN:(h + 1) * HN, h] = u[h * HN:(h + 1) * HN]
            m[f"hmu{l}"] = hmu
        in_maps.append(m)
    return in_maps


def kernel(**inputs):
    if "nc" not in _CACHE:
        _CACHE["nc"] = _build(dbg=False)
    nc = _CACHE["nc"]
    in_maps = _host_inputs(inputs)
    res = run_bass_kernel_spmd(nc, in_maps, list(range(8)))
    out = np.empty((B, T, C), np.float32)
    for b in range(B):
        out[b] = res.results[b]["xout"].T
    return out


if __name__ == "__main__":
    rng = np.random.default_rng(0)
    demo = None



# revision 2
# speedup vs baseline: 7.5093x; 7.5093x over previous
"""BiWKV6 encoder kernel for 8 Trainium2 NeuronCores.

Sharding: (batch, direction) -> 8 units, one per core; core c handles
batch c % 4, direction c // 4. Backward cores run the identical SPMD
program on time-reversed inputs; the only cross-core communication is a
pairwise AllGather of each layer's block output, written time-reversed
into the partner's domain. Within a core activations are channel-major
[C, T]; the WKV scan uses the chunked linear-attention formulation
(chunk 128) with log-space cumulative decay from the DVE prefix scan.
"""
import numpy as np

import concourse.bass as bass
import concourse.tile as tile
from concourse import bacc, mybir
from concourse.bass_utils import run_bass_kernel_spmd

B, T, C = 4, 1024, 512
H, HN = 8, 64
L = 128
TTW = 512
NTT = T // TTW
NCPT = TTW // L
CT = C // 128
TM, TD, FFN, NL = 32, 64, 1792, 2
NFF = FFN // 128
EPS_LN, EPS_GN = 1e-5, 64e-5
NV = 19

F32 = mybir.dt.float32
F32R = mybir.dt.float32r
AF = mybir.ActivationFunctionType
OP = mybir.AluOpType

WB_R, WB_K, WB_V, WB_G, WB_TM1, WB_TD1, WB_O = 0, 512, 1024, 1536, 2048, 2208, 2272
WB_COLS = 2784
(V_LN1W, V_LN1B, V_LN2W, V_LN2B, V_MAAX, V_MAAW, V_MAAK, V_MAAV, V_MAAR,
 V_MAAG, V_TDCY, V_CMK, V_CMR, V_GBM, V_CW0, V_CW1, V_CW2, V_LN0W,
 V_LN0B) = range(NV)
S_LN0, S_NEGS, S_ALPHA, S_BETA = 0, 1, 2, 3

_CACHE = {}


def _revap(ap):
    n = ap.ap[-1][1]
    return bass.AP(tensor=ap.tensor, offset=ap.offset + (n - 1) * ap.ap[-1][0],
                   ap=[ap.ap[0], [-ap.ap[-1][0], n]])


def _build(dbg=False, solo=False):
    nc = bacc.Bacc("TRN2", target_bir_lowering=False, debug=False, num_devices=8)

    x0 = nc.declare_dram_parameter("x0", [C, T], F32, isOutput=False)
    mask05 = nc.declare_dram_parameter("mask05", [128, T], F32, isOutput=False)
    sel_in = nc.declare_dram_parameter("sel", [128, 8], F32, isOutput=False)
    consts = nc.declare_dram_parameter("consts", [128, 384], F32, isOutput=False)
    wbig, tmw2, tdw2, cmkp, cmvp, cmrg, vecs_in, lnx_in, hmu_in = \
        [], [], [], [], [], [], [], [], []
    for l in range(NL):
        wbig.append(nc.declare_dram_parameter(f"wbig{l}", [C, WB_COLS], F32, isOutput=False))
        tmw2.append(nc.declare_dram_parameter(f"tmw2{l}", [5 * TM, C], F32, isOutput=False))
        tdw2.append(nc.declare_dram_parameter(f"tdw2{l}", [TD, C], F32, isOutput=False))
        cmkp.append(nc.declare_dram_parameter(f"cmk{l}", [NFF, C, 128], F32, isOutput=False))
        cmvp.append(nc.declare_dram_parameter(f"cmv{l}", [FFN, C], F32, isOutput=False))
        cmrg.append(nc.declare_dram_parameter(f"cmrg{l}", [C, 1024], F32, isOutput=False))
        vecs_in.append(nc.declare_dram_parameter(f"vecs{l}", [C, NV], F32, isOutput=False))
        lnx_in.append(nc.declare_dram_parameter(f"lnx{l}", [128, 1024], F32, isOutput=False))
        hmu_in.append(nc.declare_dram_parameter(f"hmu{l}", [C, 8], F32, isOutput=False))
    xout = nc.declare_dram_parameter("xout", [C, T], F32, isOutput=True)
    dbg_o = {}
    if dbg:
        for nm in ["xt0", "r0", "lai0", "xbtm0", "p0", "w0"]:
            dbg_o[nm] = nc.declare_dram_parameter(nm, [C, T], F32, isOutput=True)
        for nm in ["y0tm", "g0tm"]:
            dbg_o[nm] = nc.declare_dram_parameter(nm, [T, C], F32, isOutput=True)

    groups = [[0, 4], [1, 5], [2, 6], [3, 7]]

    with tile.TileContext(nc) as tc:
        with (
            tc.tile_pool(name="pp", bufs=1) as pp,
            tc.tile_pool(name="wp", bufs=2) as wp,
            tc.tile_pool(name="kp", bufs=1) as kp,
            tc.tile_pool(name="k2", bufs=2) as k2,
            tc.tile_pool(name="psA", bufs=1, space="PSUM") as psA,
            tc.tile_pool(name="psB", bufs=2, space="PSUM") as psB,
            tc.tile_pool(name="psC", bufs=1, space="PSUM") as psC,
            tc.tile_pool(name="dp", bufs=2, space="DRAM") as dp,
        ):
            # ------------- persistent loads -------------
            xres = [pp.tile([128, T], F32R, tag=f"xres{i}", name=f"xres{i}") for i in range(CT)]
            xb = [pp.tile([128, T], F32R, tag=f"xb{i}", name=f"xb{i}") for i in range(CT)]
            for i in range(CT):
                nc.sync.dma_start(out=xres[i],
                                  in_=x0[i * 128:(i + 1) * 128, :].bitcast(F32R))
            maskt = pp.tile([128, T], F32, tag="mask", name="mask")
            nc.sync.dma_start(out=maskt, in_=mask05[:, :])
            selt = pp.tile([128, 8], F32, tag="sel", name="sel")
            nc.sync.dma_start(out=selt, in_=sel_in[:, :])
            cst = pp.tile([128, 384], F32, tag="consts", name="consts")
            nc.sync.dma_start(out=cst, in_=consts[:, :])
            eps_ln_t = pp.tile([128, 1], F32, tag="epsln", name="epsln")
            nc.vector.memset(eps_ln_t, EPS_LN)
            eps_gn_t = pp.tile([128, 1], F32, tag="epsgn", name="epsgn")
            nc.vector.memset(eps_gn_t, EPS_GN)
            ident = cst[:, 0:128]
            triu = cst[:, 128:256]
            onesr_t = pp.tile([128, 128], F32R, tag="onesr", name="onesr")
            nc.sync.dma_start(out=onesr_t, in_=consts[:, 256:384].bitcast(F32R))
            ones_r = onesr_t

            def vcol(vt, i, j):
                return vt[i][:, j:j + 1]

            def load_w(dram_ap, shape, tag, bufs=2):
                t = wp.tile(shape, F32R, tag=tag, name="wld", bufs=bufs)
                nc.sync.dma_start(out=t, in_=dram_ap.bitcast(F32R))
                return t

            def load_wblk(dram_2d, shape):
                # one DMA for a [C, w] weight block -> SBUF [128, CT, w]
                t = wp.tile(shape, F32R, tag="wblk", name="wblk", bufs=2)
                nc.sync.dma_start(
                    out=t, in_=dram_2d.rearrange("(k p) n -> p k n", p=128).bitcast(F32R))
                return t

            def ln_stats(src_sl):
                ssum = psC.tile([1, TTW], F32, tag="stA", name="stA")
                ssq = psC.tile([1, TTW], F32, tag="stB", name="stB")
                for i in range(CT):
                    sq = k2.tile([128, TTW], F32R, tag="lnt1", name="lnt1")
                    nc.vector.tensor_mul(out=sq, in0=src_sl[i], in1=src_sl[i])
                    nc.tensor.matmul(out=ssum, lhsT=ones_r[:, 0:1], rhs=src_sl[i],
                                     start=(i == 0), stop=(i == CT - 1))
                    nc.tensor.matmul(out=ssq, lhsT=ones_r[:, 0:1], rhs=sq,
                                     start=(i == 0), stop=(i == CT - 1))
                rows = k2.tile([128, TTW], F32, tag="lnrows", name="lnrows", bufs=1)
                srow, s2, varu, lnv = (rows[j:j + 1, :] for j in (0, 32, 64, 96))
                nc.scalar.activation(out=srow, in_=ssum, func=AF.Copy)
                nc.vector.tensor_mul(out=s2, in0=srow, in1=srow)
                nc.vector.scalar_tensor_tensor(out=varu, in0=s2, scalar=-1.0 / C,
                                               in1=ssq, op0=OP.mult, op1=OP.add)
                nc.scalar.activation(out=lnv, in_=varu, func=AF.Ln, scale=1.0 / C,
                                     bias=eps_ln_t[0:1, :])
                rs = k2.tile([1, TTW], F32R, tag="lnrs", name="lnrs", bufs=1)
                nc.scalar.activation(out=rs, in_=lnv, func=AF.Exp, scale=-0.5)
                murs = k2.tile([1, TTW], F32R, tag="lnmu", name="lnmu", bufs=1)
                nc.vector.scalar_tensor_tensor(out=murs, in0=srow, scalar=1.0 / C,
                                               in1=rs, op0=OP.mult, op1=OP.mult)
                bc0 = psB.tile([128, TTW], F32, tag="pw", name="pw")
                nc.tensor.matmul(out=bc0, lhsT=ones_r[0:1, 0:128], rhs=rs,
                                 start=True, stop=True)
                bc1 = psB.tile([128, TTW], F32, tag="pw", name="pw")
                nc.tensor.matmul(out=bc1, lhsT=ones_r[0:1, 0:128], rhs=murs,
                                 start=True, stop=True)
                return bc0, bc1

            def ln_apply(src_sl, bc0, bc1, vt, wi, bi, out_tiles):
                for i in range(CT):
                    t1 = k2.tile([128, TTW], F32, tag="lnt1", name="lnt1")
                    nc.vector.tensor_mul(out=t1, in0=src_sl[i], in1=bc0)
                    t2 = k2.tile([128, TTW], F32, tag="lnt2", name="lnt2")
                    nc.vector.tensor_sub(out=t2, in0=t1, in1=bc1)
                    nc.vector.tensor_scalar(out=out_tiles[i], in0=t2,
                                            scalar1=vcol(vt, i, wi),
                                            scalar2=vcol(vt, i, bi),
                                            op0=OP.mult, op1=OP.add)

            def tanh_route(psum_ap, out_tile, scale=2.0):
                p, q = psum_ap.shape[0], psum_ap.shape[1]
                e = k2.tile([p, q], F32, tag="er0", name="er0", bufs=2)
                nc.scalar.activation(out=e, in_=psum_ap, func=AF.Exp, scale=scale)
                den = k2.tile([p, q], F32, tag="er1", name="er1", bufs=2)
                nc.vector.tensor_scalar_add(out=den, in0=e, scalar1=1.0)
                nc.vector.reciprocal(out=den, in_=den)
                nc.vector.tensor_scalar_add(out=e, in0=e, scalar1=-1.0)
                nc.vector.tensor_mul(out=out_tile, in0=e, in1=den)

            # ================= layers =================
            for l in range(NL):
                vecs = []
                for i in range(CT):
                    vt = pp.tile([128, NV], F32, tag=f"vecs{i}", name=f"vecs{i}")
                    nc.sync.dma_start(out=vt, in_=vecs_in[l][i * 128:(i + 1) * 128, :])
                    vecs.append(vt)
                lnxt = pp.tile([128, 1024], F32, tag="lnx", name="lnx")
                nc.sync.dma_start(out=lnxt, in_=lnx_in[l][:, :])
                hmu = []
                for i in range(CT):
                    ht = pp.tile([128, 8], F32, tag=f"hmu{i}", name=f"hmu{i}")
                    nc.sync.dma_start(out=ht, in_=hmu_in[l][i * 128:(i + 1) * 128, :])
                    hmu.append(ht)

                # ---- xb init ----
                if l == 0:
                    for tt in range(NTT):
                        sl = slice(tt * TTW, (tt + 1) * TTW)
                        src = [xres[i][:, sl] for i in range(CT)]
                        bc0, bc1 = ln_stats(src)
                        xbs = [xb[i][:, sl] for i in range(CT)]
                        ln_apply(src, bc0, bc1, vecs, V_LN0W, V_LN0B, xbs)
                        for i in range(CT):
                            d = k2.tile([128, TTW], F32, tag="lnt2", name="lnt2")
                            nc.vector.tensor_sub(out=d, in0=xb[i][:, sl],
                                                 in1=xres[i][:, sl])
                            nc.vector.scalar_tensor_tensor(
                                out=xb[i][:, sl], in0=d,
                                scalar=selt[:, S_LN0:S_LN0 + 1],
                                in1=xres[i][:, sl], op0=OP.mult, op1=OP.add)
                else:
                    for i in range(CT):
                        nc.gpsimd.tensor_copy(out=xb[i], in_=xres[i])

                S_cur = [pp.tile([128, HN], F32, tag=f"S{i}", name=f"S{i}") for i in range(CT)]
                for i in range(CT):
                    nc.vector.memset(S_cur[i], 0.0)
                carry = [pp.tile([128, 1], F32, tag=f"ca{i}", name=f"ca{i}") for i in range(CT)]
                carry2 = [pp.tile([128, 1], F32, tag=f"cb{i}", name=f"cb{i}") for i in range(CT)]
                for i in range(CT):
                    nc.gpsimd.memset(carry[i], 0.0)
                    nc.gpsimd.memset(carry2[i], 0.0)

                # ================= time mix =================
                for tt in range(NTT):
                    sl = slice(tt * TTW, (tt + 1) * TTW)
                    xbs = [xb[i][:, sl] for i in range(CT)]
                    bc0, bc1 = ln_stats(xbs)
                    xt = [kp.tile([128, TTW], F32, tag=f"xt{i}", name=f"xt{i}") for i in range(CT)]
                    ln_apply(xbs, bc0, bc1, vecs, V_LN1W, V_LN1B, xt)
                    xx = [kp.tile([128, TTW], F32, tag=f"xx{i}", name=f"xx{i}") for i in range(CT)]
                    for i in range(CT):
                        nc.vector.tensor_sub(out=xx[i][:, 1:TTW],
                                             in0=xt[i][:, 0:TTW - 1],
                                             in1=xt[i][:, 1:TTW])
                        nc.vector.scalar_tensor_tensor(
                            out=xx[i][:, 0:1], in0=carry[i], scalar=1.0,
                            in1=xt[i][:, 0:1], op0=OP.mult, op1=OP.subtract)
                        nc.gpsimd.tensor_copy(out=carry[i], in_=xt[i][:, TTW - 1:TTW])
                    if dbg and l == 0:
                        for i in range(CT):
                            nc.sync.dma_start(out=dbg_o["xt0"][i * 128:(i + 1) * 128, sl],
                                              in_=xt[i])

                    # ---- t5 ----
                    mx = [k2.tile([128, TTW], F32R, tag=f"xf{i}", name=f"xf{i}") for i in range(CT)]
                    for i in range(CT):
                        nc.vector.scalar_tensor_tensor(
                            out=mx[i], in0=xx[i], scalar=vcol(vecs, i, V_MAAX),
                            in1=xt[i], op0=OP.mult, op1=OP.add)
                    p160a = psC.tile([128, TTW], F32, tag="stA", name="stA")
                    p160b = psC.tile([32, TTW], F32, tag="stB", name="stB")
                    wtmtd = wp.tile([128, CT, 224], F32R, tag="wtmtd", name="wtmtd",
                                    bufs=1)
                    nc.sync.dma_start(
                        out=wtmtd,
                        in_=wbig[l][:, WB_TM1:WB_TM1 + 224]
                        .rearrange("(k p) n -> p k n", p=128).bitcast(F32R))
                    for i in range(CT):
                        nc.tensor.matmul(out=p160a, lhsT=wtmtd[:, i, 0:128], rhs=mx[i],
                                         start=(i == 0), stop=(i == CT - 1))
                        nc.tensor.matmul(out=p160b, lhsT=wtmtd[:, i, 128:160], rhs=mx[i],
                                         start=(i == 0), stop=(i == CT - 1))
                    t5 = [k2.tile([32, TTW], F32R, tag=f"t5{f}", name=f"t5{f}", bufs=1) for f in range(5)]
                    for f in range(4):
                        tanh_route(p160a[f * 32:(f + 1) * 32, :], t5[f])
                    tanh_route(p160b, t5[4])

                    def build_xf(fidx, maa_i):
                        w2 = load_w(tmw2[l][fidx * TM:(fidx + 1) * TM, :],
                                    [TM, C], "wtm2")
                        xft = []
                        for i in range(CT):
                            dlp = psB.tile([128, TTW], F32, tag="pw", name="pw")
                            nc.tensor.matmul(out=dlp,
                                             lhsT=w2[:, i * 128:(i + 1) * 128],
                                             rhs=t5[fidx], start=True, stop=True)
                            a = k2.tile([128, TTW], F32, tag="lnt1", name="lnt1")
                            nc.vector.scalar_tensor_tensor(
                                out=a, in0=dlp, scalar=vcol(vecs, i, maa_i),
                                in1=xx[i], op0=OP.add, op1=OP.mult)
                            xf = k2.tile([128, TTW], F32R, tag=f"xf{i}", name=f"xf{i}")
                            nc.vector.tensor_add(out=xf, in0=a, in1=xt[i])
                            xft.append(xf)
                        return xft

                    def wmm(col_off, xft):
                        accs = [psA.tile([128, TTW], F32, tag=f"acc{m}",
                                         name=f"acc{m}") for m in range(4)]
                        wt = load_wblk(wbig[l][:, col_off:col_off + 512],
                                       [128, CT, 512])
                        for i in range(CT):
                            for m in range(4):
                                nc.tensor.matmul(out=accs[m],
                                                 lhsT=wt[:, i, m * 128:(m + 1) * 128],
                                                 rhs=xft[i], start=(i == 0),
                                                 stop=(i == CT - 1))
                        return accs

                    def wmm_tm(col_off, xft):
                        accs = [psA.tile([128, TTW], F32, tag=f"acc{m}",
                                         name=f"acc{m}") for m in range(4)]
                        wt = load_wblk(wbig[l][:, col_off:col_off + 512],
                                       [128, CT, 512])
                        for i in range(CT):
                            for ci in range(NCPT):
                                nc.tensor.matmul(out=accs[ci],
                                                 lhsT=xft[i][:, ci * L:(ci + 1) * L],
                                                 rhs=wt[:, i, :], start=(i == 0),
                                                 stop=(i == CT - 1))
                        return accs

                    # k
                    xf = build_xf(1, V_MAAK)
                    accs = wmm(WB_K, xf)
                    k_sb = [kp.tile([128, TTW], F32, tag=f"ksb{i}", name=f"ksb{i}") for i in range(CT)]
                    for m in range(4):
                        nc.scalar.activation(out=k_sb[m], in_=accs[m], func=AF.Copy)
                    # v token-major
                    xf = build_xf(2, V_MAAV)
                    accs = wmm_tm(WB_V, xf)
                    v_tm = [kp.tile([128, C], F32, tag=f"vtm{ci}", name=f"vtm{ci}") for ci in range(NCPT)]
                    for ci in range(NCPT):
                        nc.scalar.activation(out=v_tm[ci], in_=accs[ci], func=AF.Copy)
                    # r
                    xf = build_xf(3, V_MAAR)
                    accs = wmm(WB_R, xf)
                    r_sb = [kp.tile([128, TTW], F32, tag=f"rsb{i}", name=f"rsb{i}") for i in range(CT)]
                    for m in range(4):
                        nc.scalar.activation(out=r_sb[m], in_=accs[m], func=AF.Copy)
                    if dbg and l == 0:
                        for i in range(CT):
                            nc.sync.dma_start(out=dbg_o["r0"][i * 128:(i + 1) * 128, sl],
                                              in_=r_sb[i])
                    # g token-major, silu
                    xf = build_xf(4, V_MAAG)
                    accs = wmm_tm(WB_G, xf)
                    g_tm = [kp.tile([128, C], F32, tag=f"gtm{ci}", name=f"gtm{ci}") for ci in range(NCPT)]
                    for ci in range(NCPT):
                        e = k2.tile([128, C], F32, tag="er0", name="er0", bufs=2)
                        nc.scalar.activation(out=e, in_=accs[ci], func=AF.Exp, scale=-1.0)
                        den = k2.tile([128, C], F32, tag="er1", name="er1", bufs=2)
                        nc.vector.tensor_scalar_add(out=den, in0=e, scalar1=1.0)
                        nc.vector.reciprocal(out=den, in_=den)
                        nc.vector.tensor_mul(out=g_tm[ci], in0=den, in1=accs[ci])
                    # w -> wacc -> lai
                    xf = build_xf(0, V_MAAW)
                    tdp = psC.tile([TD, TTW], F32, tag="stA", name="stA")
                    for i in range(CT):
                        nc.tensor.matmul(out=tdp, lhsT=wtmtd[:, i, 160:224], rhs=xf[i],
                                         start=(i == 0), stop=(i == CT - 1))
                    tdt = k2.tile([TD, TTW], F32R, tag="tdt", name="tdt", bufs=1)
                    tanh_route(tdp, tdt)
                    w2t = load_w(tdw2[l][:, :], [TD, C], "wtd2", bufs=1)
                    lai = [kp.tile([128, 1 + TTW], F32, tag=f"lai{i}", name=f"lai{i}") for i in range(CT)]
                    for i in range(CT):
                        wwp = psB.tile([128, TTW], F32, tag="pw", name="pw")
                        nc.tensor.matmul(out=wwp, lhsT=w2t[:, i * 128:(i + 1) * 128],
                                         rhs=tdt, start=True, stop=True)
                        wacc = k2.tile([128, TTW], F32, tag="lnt1", name="lnt1")
                        nc.scalar.activation(out=wacc, in_=wwp, func=AF.Exp,
                                             bias=vcol(vecs, i, V_TDCY))
                        nc.gpsimd.memset(lai[i][:, 0:1], 0.0)
                        nc.vector.tensor_tensor_scan(
                            out=lai[i][:, 1:1 + TTW], data0=wacc, data1=wacc,
                            initial=0.0, op0=OP.add, op1=OP.bypass)
                        if dbg and l == 0:
                            nc.sync.dma_start(
                                out=dbg_o["w0"][i * 128:(i + 1) * 128, sl], in_=wacc)
                            nc.sync.dma_start(
                                out=dbg_o["lai0"][i * 128:(i + 1) * 128, sl],
                                in_=lai[i][:, 1:1 + TTW])

                    # ---- wkv chunks ----
                    ztc = [kp.tile([128, TTW], F32R, tag=f"xt{i}", name=f"ztc{i}") for i in range(CT)]
                    for ci in range(NCPT):
                        gc = tt * NCPT + ci
                        c0 = ci * L
                        fpc, fnc, rt_t, kt_t, kh_tm, m_t = [], [], [], [], [], []
                        for i in range(CT):
                            ngc = k2.tile([128, 1], F32, tag="ngc", name="ngc", bufs=4)
                            nc.vector.tensor_scalar_mul(out=ngc,
                                                        in0=lai[i][:, c0:c0 + 1],
                                                        scalar1=-1.0)
                            fp = k2.tile([128, 1 + L], F32, tag="fp", name="fp", bufs=4)
                            nc.scalar.activation(out=fp, in_=lai[i][:, c0:c0 + 1 + L],
                                                 func=AF.Exp, bias=ngc)
                            fn = k2.tile([128, 1 + L], F32, tag="fn", name="fn", bufs=4)
                            nc.scalar.activation(out=fn, in_=lai[i][:, c0:c0 + 1 + L],
                                                 func=AF.Exp, scale=-1.0,
                                                 bias=lai[i][:, c0:c0 + 1])
                            fpc.append(fp)
                            fnc.append(fn)
                            rt = k2.tile([128, L], F32, tag="rt", name="rt", bufs=4)
                            nc.vector.tensor_mul(out=rt, in0=r_sb[i][:, c0:c0 + L],
                                                 in1=fn[:, 0:L])
                            kt = k2.tile([128, L], F32, tag="kt", name="kt", bufs=4)
                            nc.vector.tensor_mul(out=kt, in0=k_sb[i][:, c0:c0 + L],
                                                 in1=fp[:, 1:1 + L])
                            kh = k2.tile([128, L], F32, tag="kh", name="kh", bufs=4)
                            nc.vector.tensor_scalar_mul(out=kh, in0=kt,
                                                        scalar1=fn[:, L:L + 1])
                            mt = k2.tile([128, L], F32, tag="mt", name="mt", bufs=4)
                            nc.gpsimd.tensor_mul(out=mt, in0=r_sb[i][:, c0:c0 + L],
                                                 in1=k_sb[i][:, c0:c0 + L])
                            rt_t.append(rt)
                            kt_t.append(kt)
                            m_t.append(mt)
                            trp = psB.tile([128, L], F32, tag="pw", name="pw")
                            nc.tensor.transpose(out=trp, in_=kh, identity=ident)
                            kht = k2.tile([128, L], F32, tag="khtm", name="khtm", bufs=4)
                            nc.scalar.activation(out=kht, in_=trp, func=AF.Copy)
                            kh_tm.append(kht)
                        dall = psC.tile([128, 8], F32, tag="stB", name="stB")
                        for i in range(CT):
                            nc.tensor.matmul(out=dall, lhsT=m_t[i], rhs=hmu[i],
                                             start=(i == 0), stop=(i == CT - 1))
                        yps = psA.tile([128, C], F32, tag="acc0", name="acc0")
                        S_new = [k2.tile([128, HN], F32, tag=f"Sn{i}", name=f"Sn{i}") for i in range(CT)]
                        for i in range(CT):
                            sup = psC.tile([128, HN], F32, tag="stA", name="stA")
                            for hh in range(2):
                                h = 2 * i + hh
                                hb = hh * HN
                                pt = psB.tile([L, L], F32, tag="pw", name="pw")
                                nc.tensor.matmul(out=pt, lhsT=kt_t[i][hb:hb + HN, :],
                                                 rhs=rt_t[i][hb:hb + HN, :],
                                                 start=True, stop=True)
                                pts = k2.tile([L, L], F32, tag="pts", name="pts")
                                nc.vector.tensor_mul(out=pts, in0=pt, in1=triu)
                                nc.tensor.matmul(
                                    out=yps[:, h * HN:(h + 1) * HN], lhsT=pts,
                                    rhs=v_tm[ci][:, h * HN:(h + 1) * HN],
                                    start=True, stop=(gc == 0), skip_group_check=True)
                                if gc > 0:
                                    nc.tensor.matmul(
                                        out=yps[:, h * HN:(h + 1) * HN],
                                        lhsT=rt_t[i][hb:hb + HN, :],
                                        rhs=S_cur[i][hb:hb + HN, :],
                                        start=False, stop=True, skip_group_check=True)
                                nc.tensor.matmul(
                                    out=sup[hb:hb + HN, :],
                                    lhsT=kh_tm[i][:, hb:hb + HN],
                                    rhs=v_tm[ci][:, h * HN:(h + 1) * HN],
                                    start=True, stop=True, skip_group_check=True)
                            t0 = k2.tile([128, HN], F32, tag="ssc", name="ssc", bufs=4)
                            nc.vector.tensor_scalar_mul(out=t0, in0=S_cur[i],
                                                        scalar1=fnc[i][:, L:L + 1])
                            nc.vector.tensor_add(out=S_new[i], in0=t0, in1=sup)
                        S_cur = S_new
                        ysb = k2.tile([128, C], F32, tag="ysb", name="ysb")
                        for h in range(H):
                            nc.vector.scalar_tensor_tensor(
                                out=ysb[:, h * HN:(h + 1) * HN],
                                in0=v_tm[ci][:, h * HN:(h + 1) * HN],
                                scalar=dall[:, h:h + 1],
                                in1=yps[:, h * HN:(h + 1) * HN],
                                op0=OP.mult, op1=OP.add)
                        if dbg and l == 0:
                            nc.sync.dma_start(
                                out=dbg_o["y0tm"][tt * TTW + c0:tt * TTW + c0 + L, :],
                                in_=ysb)
                            nc.sync.dma_start(
                                out=dbg_o["g0tm"][tt * TTW + c0:tt * TTW + c0 + L, :],
                                in_=g_tm[ci])
                        # groupnorm + affine + *g  (token-major)
                        mv = k2.tile([128, 16], F32, tag="gnmv", name="gnmv")
                        for h in range(H):
                            st = k2.tile([128, 6], F32, tag="gnst", name="gnst")
                            nc.vector.bn_stats(out=st, in_=ysb[:, h * HN:(h + 1) * HN])
                            nc.vector.bn_aggr(out=mv[:, 2 * h:2 * h + 2], in_=st)
                        lnv = k2.tile([128, 8], F32, tag="gnln", name="gnln")
                        var_view = bass.AP(tensor=mv.tensor, offset=mv.offset + 1,
                                           ap=[mv.ap[0], [2, 8]])
                        nc.scalar.activation(out=lnv, in_=var_view, func=AF.Ln,
                                             bias=eps_gn_t)
                        rsg = k2.tile([128, 8], F32, tag="gnrs", name="gnrs")
                        nc.scalar.activation(out=rsg, in_=lnv, func=AF.Exp, scale=-0.5)
                        for h in range(H):
                            nc.vector.tensor_scalar(
                                out=ysb[:, h * HN:(h + 1) * HN],
                                in0=ysb[:, h * HN:(h + 1) * HN],
                                scalar1=mv[:, 2 * h:2 * h + 1],
                                scalar2=rsg[:, h:h + 1],
                                op0=OP.subtract, op1=OP.mult)
                        nc.gpsimd.tensor_mul(out=ysb, in0=ysb, in1=lnxt[:, 0:512])
                        nc.gpsimd.tensor_add(out=ysb, in0=ysb, in1=lnxt[:, 512:1024])
                        nc.vector.tensor_mul(out=ysb, in0=ysb, in1=g_tm[ci])
                        for i in range(CT):
                            trp = psB.tile([128, L], F32, tag="pw", name="pw")
                            nc.tensor.transpose(out=trp,
                                                in_=ysb[:, i * 128:(i + 1) * 128],
                                                identity=ident)
                            nc.scalar.activation(out=ztc[i][:, c0:c0 + L], in_=trp,
                                                 func=AF.Copy)
                    # ---- Wo ----
                    accs = [psA.tile([128, TTW], F32, tag=f"acc{m}",
                                     name=f"acc{m}") for m in range(4)]
                    wt = load_wblk(wbig[l][:, WB_O:WB_O + 512], [128, CT, 512])
                    for i in range(CT):
                        for m in range(4):
                            nc.tensor.matmul(out=accs[m],
                                             lhsT=wt[:, i, m * 128:(m + 1) * 128],
                                             rhs=ztc[i], start=(i == 0),
                                             stop=(i == CT - 1))
                    for m in range(4):
                        nc.vector.tensor_add(out=xb[m][:, sl], in0=xb[m][:, sl],
                                             in1=accs[m])
                    if dbg and l == 0:
                        for i in range(CT):
                            nc.sync.dma_start(
                                out=dbg_o["xbtm0"][i * 128:(i + 1) * 128, sl],
                                in_=xb[i][:, sl].bitcast(F32))

                # ================= channel mix =================
                for tt in range(NTT):
                    sl = slice(tt * TTW, (tt + 1) * TTW)
                    xbs = [xb[i][:, sl] for i in range(CT)]
                    bc0, bc1 = ln_stats(xbs)
                    xc = [kp.tile([128, TTW], F32, tag=f"xt{i}", name=f"xt{i}") for i in range(CT)]
                    ln_apply(xbs, bc0, bc1, vecs, V_LN2W, V_LN2B, xc)
                    xx2 = [kp.tile([128, TTW], F32, tag=f"xx{i}", name=f"xx{i}") for i in range(CT)]
                    for i in range(CT):
                        nc.vector.tensor_sub(out=xx2[i][:, 1:TTW],
                                             in0=xc[i][:, 0:TTW - 1],
                                             in1=xc[i][:, 1:TTW])
                        nc.vector.scalar_tensor_tensor(
                            out=xx2[i][:, 0:1], in0=carry2[i], scalar=1.0,
                            in1=xc[i][:, 0:1], op0=OP.mult, op1=OP.subtract)
                        nc.gpsimd.tensor_copy(out=carry2[i], in_=xc[i][:, TTW - 1:TTW])
                    xk2 = [kp.tile([128, TTW], F32R, tag=f"ksb{i}", name=f"ksb{i}") for i in range(CT)]
                    xr2 = [kp.tile([128, TTW], F32R, tag=f"rsb{i}", name=f"rsb{i}") for i in range(CT)]
                    for i in range(CT):
                        nc.vector.scalar_tensor_tensor(
                            out=xk2[i], in0=xx2[i], scalar=vcol(vecs, i, V_CMK),
                            in1=xc[i], op0=OP.mult, op1=OP.add)
                        nc.vector.scalar_tensor_tensor(
                            out=xr2[i], in0=xx2[i], scalar=vcol(vecs, i, V_CMR),
                            in1=xc[i], op0=OP.mult, op1=OP.add)
                    # cm_Wr -> sigmoid
                    accs = [psA.tile([128, TTW], F32, tag=f"acc{m}", name=f"acc{m}") for m in range(4)]
                    wt = load_wblk(cmrg[l][:, 0:512], [128, CT, 512])
                    for i in range(CT):
                        for m in range(4):
                            nc.tensor.matmul(out=accs[m],
                                             lhsT=wt[:, i, m * 128:(m + 1) * 128],
                                             rhs=xr2[i], start=(i == 0),
                                             stop=(i == CT - 1))
                    sig = [kp.tile([128, TTW], F32, tag=f"gtm{m}", name=f"sig{m}") for m in range(4)]
                    for m in range(4):
                        e = k2.tile([128, TTW], F32, tag="er0", name="er0", bufs=2)
                        nc.scalar.activation(out=e, in_=accs[m], func=AF.Exp, scale=-1.0)
                        nc.vector.tensor_scalar_add(out=e, in0=e, scalar1=1.0)
                        nc.vector.reciprocal(out=sig[m], in_=e)
                    # kk loop with cm_Wv accumulation
                    accs = [psA.tile([128, TTW], F32, tag=f"acc{m}", name=f"acc{m}") for m in range(4)]
                    wfq = wvq = None
                    for f in range(NFF):
                        fq, fr2 = f // 4, f % 4
                        nq = min(4, NFF - 4 * fq)
                        if fr2 == 0:
                            wfq = wp.tile([128, nq, CT, 128], F32R, tag="wblk",
                                          name="wfq", bufs=2)
                            nc.sync.dma_start(
                                out=wfq,
                                in_=cmkp[l][4 * fq:4 * fq + nq]
                                .rearrange("f (k p) n -> p f k n", p=128)
                                .bitcast(F32R))
                            wvq = wp.tile([128, nq, C], F32R, tag="wblk",
                                          name="wvq", bufs=2)
                            nc.sync.dma_start(
                                out=wvq,
                                in_=cmvp[l][4 * fq * 128:(4 * fq + nq) * 128, :]
                                .rearrange("(f p) n -> p f n", p=128)
                                .bitcast(F32R))
                        kkp = psB.tile([128, TTW], F32, tag="pw", name="pw")
                        for i in range(CT):
                            nc.tensor.matmul(out=kkp, lhsT=wfq[:, fr2, i, :],
                                             rhs=xk2[i],
                                             start=(i == 0), stop=(i == CT - 1))
                        rl = k2.tile([128, TTW], F32, tag="lnt1", name="lnt1")
                        nc.vector.tensor_scalar_max(out=rl, in0=kkp, scalar1=0.0)
                        kkf = k2.tile([128, TTW], F32R, tag="lnt2", name="lnt2")
                        nc.vector.tensor_mul(out=kkf, in0=rl, in1=kkp)
                        for m in range(4):
                            nc.tensor.matmul(out=accs[m],
                                             lhsT=wvq[:, fr2, m * 128:(m + 1) * 128],
                                             rhs=kkf, start=(f == 0),
                                             stop=(f == NFF - 1))
                    for m in range(4):
                        nc.vector.tensor_mul(out=sig[m], in0=sig[m], in1=accs[m])
                        nc.vector.tensor_add(out=xb[m][:, sl], in0=xb[m][:, sl],
                                             in1=sig[m])
                if dbg and l == 0:
                    for i in range(CT):
                        nc.sync.dma_start(out=dbg_o["p0"][i * 128:(i + 1) * 128, :],
                                          in_=xb[i].bitcast(F32))

                # ================= exchange + join =================
                send = dp.tile([C, T], F32, tag="send", name="send")
                recv = dp.tile([2 * C, T], F32, tag="recv", name="recv")
                for i in range(CT):
                    # time-reverse in SBUF (DVE handles the negative stride),
                    # then one contiguous DMA — a reversed DRAM AP would shatter
                    # into 128*1024 4-byte descriptors and stall every queue.
                    rv = k2.tile([128, T], F32, tag="revst", name="revst")
                    nc.vector.tensor_copy(out=rv, in_=_revap(xb[i][:, :].bitcast(F32)))
                    nc.sync.dma_start(out=send[i * 128:(i + 1) * 128, :], in_=rv)
                if solo:
                    for i in range(CT):
                        nc.sync.dma_start(out=recv[i * 128:(i + 1) * 128, :],
                                          in_=xb[i].bitcast(F32))
                        nc.sync.dma_start(out=recv[C + i * 128:C + (i + 1) * 128, :],
                                          in_=xb[i].bitcast(F32))
                else:
                    nc.gpsimd.collective_compute(
                        "AllGather", OP.bypass, replica_groups=groups,
                        ins=[send.opt()], outs=[recv.opt()])
                for tt in range(NTT):
                    sl = slice(tt * TTW, (tt + 1) * TTW)
                    # conv residue, all ct (reuse ksb tags)
                    cv = [kp.tile([128, TTW], F32R, tag=f"ksb{i}", name=f"ksb{i}") for i in range(CT)]
                    a = tt * TTW
                    for i in range(CT):
                        nc.vector.tensor_scalar_mul(out=cv[i], in0=xres[i][:, sl],
                                                    scalar1=vcol(vecs, i, V_CW1))
                        lo = 1 if tt == 0 else 0
                        nc.vector.scalar_tensor_tensor(
                            out=cv[i][:, lo:TTW],
                            in0=xres[i][:, a + lo - 1:a + TTW - 1],
                            scalar=vcol(vecs, i, V_CW0),
                            in1=cv[i][:, lo:TTW], op0=OP.mult, op1=OP.add)
                        hi = TTW - 1 if tt == NTT - 1 else TTW
                        nc.vector.scalar_tensor_tensor(
                            out=cv[i][:, 0:hi],
                            in0=xres[i][:, a + 1:a + hi + 1],
                            scalar=vcol(vecs, i, V_CW2),
                            in1=cv[i][:, 0:hi], op0=OP.mult, op1=OP.add)
                    accs = [psA.tile([128, TTW], F32, tag=f"acc{m}", name=f"acc{m}") for m in range(4)]
                    wt = load_wblk(cmrg[l][:, 512:1024], [128, CT, 512])
                    for i in range(CT):
                        for m in range(4):
                            nc.tensor.matmul(out=accs[m],
                                             lhsT=wt[:, i, m * 128:(m + 1) * 128],
                                             rhs=cv[i], start=(i == 0),
                                             stop=(i == CT - 1))
                    for m in range(4):
                        jr0 = kp.tile([128, TTW], F32, tag="vtm0", name="vtm0")
                        jr1 = kp.tile([128, TTW], F32, tag="vtm1", name="vtm1")
                        nc.sync.dma_start(out=jr0, in_=recv[m * 128:(m + 1) * 128, sl])
                        nc.sync.dma_start(out=jr1,
                                          in_=recv[C + m * 128:C + (m + 1) * 128, sl])
                        nc.vector.tensor_scalar_mul(
                            out=jr0, in0=jr0, scalar1=selt[:, S_ALPHA:S_ALPHA + 1])
                        nc.vector.scalar_tensor_tensor(
                            out=jr1, in0=jr1, scalar=selt[:, S_BETA:S_BETA + 1],
                            in1=jr0, op0=OP.mult, op1=OP.add)
                        e = k2.tile([128, TTW], F32, tag="er0", name="er0", bufs=2)
                        nc.scalar.activation(out=e, in_=accs[m], func=AF.Exp,
                                             scale=selt[:, S_NEGS:S_NEGS + 1],
                                             bias=vcol(vecs, m, V_GBM))
                        wown = kp.tile([128, TTW], F32, tag="vtm3", name="vtm3")
                        nc.vector.tensor_scalar_add(out=wown, in0=e, scalar1=1.0)
                        nc.vector.reciprocal(out=wown, in_=wown)
                        nc.vector.tensor_mul(out=e, in0=wown, in1=e)  # w_recv
                        jsum = kp.tile([128, TTW], F32, tag="vtm2", name="vtm2")
                        nc.vector.tensor_mul(out=jsum, in0=wown, in1=xb[m][:, sl])
                        nc.vector.tensor_mul(out=jr1, in0=e, in1=jr1)
                        nc.vector.tensor_add(out=jsum, in0=jsum, in1=jr1)
                        # note: host mask05 already includes the 2x factor fold:
                        # mask05 = mask (not 0.5*mask) since sigmoid form used.
                        nc.vector.tensor_mul(out=xres[m][:, sl], in0=jsum,
                                             in1=maskt[:, sl])
            # ---- output ----
            for i in range(CT):
                nc.sync.dma_start(out=xout[i * 128:(i + 1) * 128, :],
                                  in_=xres[i].bitcast(F32))
    nc.compile()
    return nc


def _host_inputs(inputs):
    x = np.asarray(inputs["x"], np.float32)
    lengths = np.asarray(inputs["lengths"]).astype(np.int64)
    pos = np.arange(T, dtype=np.float32)[:, None]
    div = np.exp(np.arange(0, C, 2, dtype=np.float32) * (-np.log(10000.0) / C))
    pe = np.zeros((T, C), np.float32)
    pe[:, 0::2] = np.sin(pos * div)
    pe[:, 1::2] = np.cos(pos * div)
    mask = (np.arange(T)[None, :] < lengths[:, None]).astype(np.float32)

    consts = np.zeros((128, 384), np.float32)
    consts[:, 0:128] = np.eye(128, dtype=np.float32)
    consts[:, 128:256] = np.triu(np.ones((128, 128), np.float32), 1)
    consts[:, 256:384] = 1.0

    gw = np.asarray(inputs["gate_w"], np.float32)
    gb = np.asarray(inputs["gate_b"], np.float32)
    cw = np.asarray(inputs["conv_w"], np.float32)
    cb = np.asarray(inputs["conv_b"], np.float32)

    in_maps = []
    for c in range(8):
        b, d = c % 4, c // 4
        rev = d == 1
        s = -1.0 if rev else 1.0
        xin = (x[b] + pe)
        mrow = mask[b]
        if rev:
            xin = xin[::-1]
            mrow = mrow[::-1]
        m = {
            "x0": np.ascontiguousarray(xin.T),
            "mask05": np.ascontiguousarray(np.broadcast_to(mrow, (128, T))),
            "consts": consts,
        }
        sel = np.zeros((128, 8), np.float32)
        sel[:, S_LN0] = 0.0 if rev else 1.0
        sel[:, S_NEGS] = -s
        sel[:, S_ALPHA] = 1.0 if rev else 0.0
        sel[:, S_BETA] = 0.0 if rev else 1.0
        m["sel"] = sel
        for l in range(NL):
            W = {k: np.asarray(inputs[k], np.float32)[d, l]
                 for k in ["ln1_w", "ln1_b", "ln2_w", "ln2_b", "maa_x", "maa_w",
                           "maa_k", "maa_v", "maa_r", "maa_g", "tm_w1", "tm_w2",
                           "td_w1", "td_w2", "time_decay", "Wr", "Wk", "Wv",
                           "Wg", "Wo", "lnx_w", "lnx_b", "cm_maa_k", "cm_maa_r",
                           "cm_Wk", "cm_Wv", "cm_Wr", "time_faaaa"]}
            m[f"wbig{l}"] = np.ascontiguousarray(np.concatenate(
                [W["Wr"], W["Wk"], W["Wv"], W["Wg"], W["tm_w1"], W["td_w1"],
                 W["Wo"]], axis=1))
            m[f"tmw2{l}"] = np.ascontiguousarray(W["tm_w2"].reshape(5 * TM, C))
            m[f"tdw2{l}"] = np.ascontiguousarray(W["td_w2"])
            m[f"cmk{l}"] = np.ascontiguousarray(
                W["cm_Wk"].reshape(C, NFF, 128).transpose(1, 0, 2))
            m[f"cmv{l}"] = np.ascontiguousarray(W["cm_Wv"])
            m[f"cmrg{l}"] = np.ascontiguousarray(
                np.concatenate([W["cm_Wr"], gw[l]], axis=1))
            cwe = cw[l] if not rev else cw[l][:, ::-1]
            gbe = cb[l] @ gw[l] + gb[l]
            vec = np.zeros((C, NV), np.float32)
            vec[:, V_LN1W] = W["ln1_w"]; vec[:, V_LN1B] = W["ln1_b"]
            vec[:, V_LN2W] = W["ln2_w"]; vec[:, V_LN2B] = W["ln2_b"]
            vec[:, V_MAAX] = W["maa_x"]; vec[:, V_MAAW] = W["maa_w"]
            vec[:, V_MAAK] = W["maa_k"]; vec[:, V_MAAV] = W["maa_v"]
            vec[:, V_MAAR] = W["maa_r"]; vec[:, V_MAAG] = W["maa_g"]
            vec[:, V_TDCY] = W["time_decay"]
            vec[:, V_CMK] = W["cm_maa_k"]; vec[:, V_CMR] = W["cm_maa_r"]
            vec[:, V_GBM] = -s * gbe
            vec[:, V_CW0] = cwe[:, 0]
            vec[:, V_CW1] = cwe[:, 1] - 1.0
            vec[:, V_CW2] = cwe[:, 2]
            vec[:, V_LN0W] = np.asarray(inputs["ln0_w"], np.float32)
            vec[:, V_LN0B] = np.asarray(inputs["ln0_b"], np.float32)
            m[f"vecs{l}"] = vec
            lnx = np.zeros((128, 1024), np.float32)
            lnx[:, 0:512] = W["lnx_w"][None, :]
            lnx[:, 512:1024] = W["lnx_b"][None, :]
            m[f"lnx{l}"] = lnx
            u = W["time_faaaa"].reshape(C)
            hmu = np.zeros((C, 8), np.float32)
            for h in range(H):
                hmu[h * HN:(h + 1) * HN, h] = u[h * HN:(h + 1) * HN]
            m[f"hmu{l}"] = hmu
        in_maps.append(m)
    return in_maps


def kernel(**inputs):
    if "nc" not in _CACHE:
        _CACHE["nc"] = _build(dbg=False)
    nc = _CACHE["nc"]
    in_maps = _host_inputs(inputs)
    res = run_bass_kernel_spmd(nc, in_maps, list(range(8)))
    out = np.empty((B, T, C), np.float32)
    for b in range(B):
        out[b] = res.results[b]["xout"].T
    return out


if __name__ == "__main__":
    rng = np.random.default_rng(0)
    demo = None



# revision 18
# speedup vs baseline: 9.5647x; 1.2737x over previous
"""BiWKV6 encoder kernel for 8 Trainium2 NeuronCores.

Sharding: (batch, direction) -> 8 units, one per core; core c handles
batch c % 4, direction c // 4. Backward cores run the identical SPMD
program on time-reversed inputs; the only cross-core communication is a
pairwise AllGather of each layer's block output, written time-reversed
into the partner's domain. Within a core activations are channel-major
[C, T]; the WKV scan uses the chunked linear-attention formulation
(chunk 128) with log-space cumulative decay from the DVE prefix scan.
"""
import numpy as np

import concourse.bass as bass
import concourse.tile as tile
from concourse import bacc, mybir
from concourse.bass_utils import run_bass_kernel_spmd

B, T, C = 4, 1024, 512
H, HN = 8, 64
L = 128
TTW = 512
NTT = T // TTW
NCPT = TTW // L
CT = C // 128
TM, TD, FFN, NL = 32, 64, 1792, 2
NFF = FFN // 128
EPS_LN, EPS_GN = 1e-5, 64e-5
NV = 19

F32 = mybir.dt.float32
F32R = mybir.dt.float32r
BF16 = mybir.dt.bfloat16
AF = mybir.ActivationFunctionType
OP = mybir.AluOpType

WB_R, WB_K, WB_V, WB_G, WB_TM1, WB_TD1, WB_O = 0, 512, 1024, 1536, 2048, 2208, 2272
WB_COLS = 2784
(V_LN1W, V_LN1B, V_LN2W, V_LN2B, V_MAAX, V_MAAW, V_MAAK, V_MAAV, V_MAAR,
 V_MAAG, V_TDCY, V_CMK, V_CMR, V_GBM, V_CW0, V_CW1, V_CW2, V_LN0W,
 V_LN0B) = range(NV)
S_LN0, S_NEGS, S_ALPHA, S_BETA = 0, 1, 2, 3

_CACHE = {}


def _revap(ap):
    n = ap.ap[-1][1]
    return bass.AP(tensor=ap.tensor, offset=ap.offset + (n - 1) * ap.ap[-1][0],
                   ap=[ap.ap[0], [-ap.ap[-1][0], n]])


def _build(dbg=False, solo=False):
    nc = bacc.Bacc("TRN2", target_bir_lowering=False, debug=False, num_devices=8)

    x0 = nc.declare_dram_parameter("x0", [C, T], F32, isOutput=False)
    mask05 = nc.declare_dram_parameter("mask05", [128, T], F32, isOutput=False)
    sel_in = nc.declare_dram_parameter("sel", [128, 8], F32, isOutput=False)
    consts = nc.declare_dram_parameter("consts", [128, 384], F32, isOutput=False)
    wbig, tmw2, tdw2, cmkp, cmvp, cmrg, vecs_in, lnx_in, hmu_in = \
        [], [], [], [], [], [], [], [], []
    for l in range(NL):
        wbig.append(nc.declare_dram_parameter(f"wbig{l}", [C, WB_COLS], BF16, isOutput=False))
        tmw2.append(nc.declare_dram_parameter(f"tmw2{l}", [5 * TM, C], BF16, isOutput=False))
        tdw2.append(nc.declare_dram_parameter(f"tdw2{l}", [TD, C], BF16, isOutput=False))
        cmkp.append(nc.declare_dram_parameter(f"cmk{l}", [NFF, C, 128], BF16, isOutput=False))
        cmvp.append(nc.declare_dram_parameter(f"cmv{l}", [FFN, C], BF16, isOutput=False))
        cmrg.append(nc.declare_dram_parameter(f"cmrg{l}", [C, 1024], BF16, isOutput=False))
        vecs_in.append(nc.declare_dram_parameter(f"vecs{l}", [C, NV], F32, isOutput=False))
        lnx_in.append(nc.declare_dram_parameter(f"lnx{l}", [128, 1024], F32, isOutput=False))
        hmu_in.append(nc.declare_dram_parameter(f"hmu{l}", [C, 8], F32, isOutput=False))
    xout = nc.declare_dram_parameter("xout", [C, T], F32, isOutput=True)
    dbg_o = {}
    if dbg:
        for nm in ["xt0", "r0", "lai0", "xbtm0", "p0", "w0"]:
            dbg_o[nm] = nc.declare_dram_parameter(nm, [C, T], F32, isOutput=True)
        for nm in ["y0tm", "g0tm"]:
            dbg_o[nm] = nc.declare_dram_parameter(nm, [T, C], F32, isOutput=True)

    groups = [[0, 4], [1, 5], [2, 6], [3, 7]]

    with tile.TileContext(nc) as tc:
        with (
            tc.tile_pool(name="pp", bufs=1) as pp,
            tc.tile_pool(name="wp", bufs=2) as wp,
            tc.tile_pool(name="kp", bufs=1) as kp,
            tc.tile_pool(name="k2", bufs=2) as k2,
            tc.tile_pool(name="psA", bufs=1, space="PSUM") as psA,
            tc.tile_pool(name="psB", bufs=2, space="PSUM") as psB,
            tc.tile_pool(name="psC", bufs=1, space="PSUM") as psC,
            tc.tile_pool(name="dp", bufs=2, space="DRAM") as dp,
        ):
            # ------------- persistent loads -------------
            xres = [pp.tile([128, T], F32R, tag=f"xres{i}", name=f"xres{i}") for i in range(CT)]
            xb = [pp.tile([128, T], F32R, tag=f"xb{i}", name=f"xb{i}") for i in range(CT)]
            for i in range(CT):
                nc.sync.dma_start(out=xres[i],
                                  in_=x0[i * 128:(i + 1) * 128, :].bitcast(F32R))
            maskt = pp.tile([128, T], F32, tag="mask", name="mask")
            nc.sync.dma_start(out=maskt, in_=mask05[:, :])
            selt = pp.tile([128, 8], F32, tag="sel", name="sel")
            nc.sync.dma_start(out=selt, in_=sel_in[:, :])
            cst = pp.tile([128, 384], F32, tag="consts", name="consts")
            nc.sync.dma_start(out=cst, in_=consts[:, :])
            eps_ln_t = pp.tile([128, 1], F32, tag="epsln", name="epsln")
            nc.vector.memset(eps_ln_t, EPS_LN)
            eps_gn_t = pp.tile([128, 1], F32, tag="epsgn", name="epsgn")
            nc.vector.memset(eps_gn_t, EPS_GN)
            ident = cst[:, 0:128]
            triu = cst[:, 128:256]
            onesr_t = pp.tile([128, 128], F32R, tag="onesr", name="onesr")
            nc.sync.dma_start(out=onesr_t, in_=consts[:, 256:384].bitcast(F32R))
            ones_r = onesr_t

            def vcol(vt, i, j):
                return vt[i][:, j:j + 1]

            def load_w(dram_ap, shape, tag, bufs=2):
                t = wp.tile(shape, BF16, tag=tag, name="wld", bufs=bufs)
                nc.sync.dma_start(out=t, in_=dram_ap)
                return t

            def load_wblk(dram_2d, shape):
                # one DMA for a [C, w] weight block -> SBUF [128, CT, w]
                t = wp.tile(shape, BF16, tag="wblk", name="wblk", bufs=2)
                nc.sync.dma_start(
                    out=t, in_=dram_2d.rearrange("(k p) n -> p k n", p=128))
                return t

            def ln_stats(src_sl):
                ssum = psC.tile([1, TTW], F32, tag="stA", name="stA")
                ssq = psC.tile([1, TTW], F32, tag="stB", name="stB")
                for i in range(CT):
                    sq = k2.tile([128, TTW], F32R, tag="lnt1", name="lnt1")
                    nc.vector.tensor_mul(out=sq, in0=src_sl[i], in1=src_sl[i])
                    nc.tensor.matmul(out=ssum, lhsT=ones_r[:, 0:1], rhs=src_sl[i],
                                     start=(i == 0), stop=(i == CT - 1))
                    nc.tensor.matmul(out=ssq, lhsT=ones_r[:, 0:1], rhs=sq,
                                     start=(i == 0), stop=(i == CT - 1))
                rows = k2.tile([128, TTW], F32, tag="lnrows", name="lnrows", bufs=1)
                srow, s2, varu, lnv = (rows[j:j + 1, :] for j in (0, 32, 64, 96))
                nc.scalar.activation(out=srow, in_=ssum, func=AF.Copy)
                nc.vector.tensor_mul(out=s2, in0=srow, in1=srow)
                nc.vector.scalar_tensor_tensor(out=varu, in0=s2, scalar=-1.0 / C,
                                               in1=ssq, op0=OP.mult, op1=OP.add)
                nc.scalar.activation(out=lnv, in_=varu, func=AF.Ln, scale=1.0 / C,
                                     bias=eps_ln_t[0:1, :])
                rs = k2.tile([1, TTW], F32R, tag="lnrs", name="lnrs", bufs=1)
                nc.scalar.activation(out=rs, in_=lnv, func=AF.Exp, scale=-0.5)
                murs = k2.tile([1, TTW], F32R, tag="lnmu", name="lnmu", bufs=1)
                nc.vector.scalar_tensor_tensor(out=murs, in0=srow, scalar=1.0 / C,
                                               in1=rs, op0=OP.mult, op1=OP.mult)
                bc0 = psB.tile([128, TTW], F32, tag="pw", name="pw")
                nc.tensor.matmul(out=bc0, lhsT=ones_r[0:1, 0:128], rhs=rs,
                                 start=True, stop=True)
                bc1 = psB.tile([128, TTW], F32, tag="pw", name="pw")
                nc.tensor.matmul(out=bc1, lhsT=ones_r[0:1, 0:128], rhs=murs,
                                 start=True, stop=True)
                return bc0, bc1

            def ln_apply(src_sl, bc0, bc1, vt, wi, bi, out_tiles):
                for i in range(CT):
                    t1 = k2.tile([128, TTW], F32, tag="lnt1", name="lnt1")
                    nc.vector.tensor_mul(out=t1, in0=src_sl[i], in1=bc0)
                    t2 = k2.tile([128, TTW], F32, tag="lnt2", name="lnt2")
                    nc.vector.tensor_sub(out=t2, in0=t1, in1=bc1)
                    nc.vector.tensor_scalar(out=out_tiles[i], in0=t2,
                                            scalar1=vcol(vt, i, wi),
                                            scalar2=vcol(vt, i, bi),
                                            op0=OP.mult, op1=OP.add)

            def tanh_route(psum_ap, out_tile):
                nc.scalar.activation(out=out_tile, in_=psum_ap, func=AF.Tanh)

            # ================= layers =================
            for l in range(NL):
                vecs = []
                for i in range(CT):
                    vt = pp.tile([128, NV], F32, tag=f"vecs{i}", name=f"vecs{i}")
                    nc.sync.dma_start(out=vt, in_=vecs_in[l][i * 128:(i + 1) * 128, :])
                    vecs.append(vt)
                lnxt = pp.tile([128, 1024], F32, tag="lnx", name="lnx")
                nc.sync.dma_start(out=lnxt, in_=lnx_in[l][:, :])
                hmu = []
                for i in range(CT):
                    ht = pp.tile([128, 8], F32, tag=f"hmu{i}", name=f"hmu{i}")
                    nc.sync.dma_start(out=ht, in_=hmu_in[l][i * 128:(i + 1) * 128, :])
                    hmu.append(ht)

                # ---- xb init ----
                if l == 0:
                    for tt in range(NTT):
                        sl = slice(tt * TTW, (tt + 1) * TTW)
                        src = [xres[i][:, sl] for i in range(CT)]
                        bc0, bc1 = ln_stats(src)
                        xbs = [xb[i][:, sl] for i in range(CT)]
                        ln_apply(src, bc0, bc1, vecs, V_LN0W, V_LN0B, xbs)
                        for i in range(CT):
                            d = k2.tile([128, TTW], F32, tag="lnt2", name="lnt2")
                            nc.vector.tensor_sub(out=d, in0=xb[i][:, sl],
                                                 in1=xres[i][:, sl])
                            nc.vector.scalar_tensor_tensor(
                                out=xb[i][:, sl], in0=d,
                                scalar=selt[:, S_LN0:S_LN0 + 1],
                                in1=xres[i][:, sl], op0=OP.mult, op1=OP.add)
                else:
                    for i in range(CT):
                        nc.gpsimd.tensor_copy(out=xb[i], in_=xres[i])

                S_cur = [pp.tile([128, HN], F32, tag=f"S{i}", name=f"S{i}") for i in range(CT)]
                for i in range(CT):
                    nc.vector.memset(S_cur[i], 0.0)
                carry = [pp.tile([128, 1], F32, tag=f"ca{i}", name=f"ca{i}") for i in range(CT)]
                carry2 = [pp.tile([128, 1], F32, tag=f"cb{i}", name=f"cb{i}") for i in range(CT)]
                for i in range(CT):
                    nc.gpsimd.memset(carry[i], 0.0)
                    nc.gpsimd.memset(carry2[i], 0.0)

                # ================= time mix =================
                for tt in range(NTT):
                    sl = slice(tt * TTW, (tt + 1) * TTW)
                    xbs = [xb[i][:, sl] for i in range(CT)]
                    bc0, bc1 = ln_stats(xbs)
                    xt = [kp.tile([128, TTW], F32, tag=f"xt{i}", name=f"xt{i}") for i in range(CT)]
                    ln_apply(xbs, bc0, bc1, vecs, V_LN1W, V_LN1B, xt)
                    xx = [kp.tile([128, TTW], F32, tag=f"xx{i}", name=f"xx{i}") for i in range(CT)]
                    for i in range(CT):
                        nc.vector.tensor_sub(out=xx[i][:, 1:TTW],
                                             in0=xt[i][:, 0:TTW - 1],
                                             in1=xt[i][:, 1:TTW])
                        nc.vector.scalar_tensor_tensor(
                            out=xx[i][:, 0:1], in0=carry[i], scalar=1.0,
                            in1=xt[i][:, 0:1], op0=OP.mult, op1=OP.subtract)
                        nc.gpsimd.tensor_copy(out=carry[i], in_=xt[i][:, TTW - 1:TTW])
                    if dbg and l == 0:
                        for i in range(CT):
                            nc.sync.dma_start(out=dbg_o["xt0"][i * 128:(i + 1) * 128, sl],
                                              in_=xt[i])

                    # ---- t5 ----
                    mx = [k2.tile([128, TTW], BF16, tag=f"xf{i}", name=f"xf{i}") for i in range(CT)]
                    for i in range(CT):
                        nc.vector.scalar_tensor_tensor(
                            out=mx[i], in0=xx[i], scalar=vcol(vecs, i, V_MAAX),
                            in1=xt[i], op0=OP.mult, op1=OP.add)
                    p160a = psC.tile([128, TTW], F32, tag="stA", name="stA")
                    p160b = psC.tile([32, TTW], F32, tag="stB", name="stB")
                    wtmtd = wp.tile([128, CT, 224], BF16, tag="wtmtd", name="wtmtd",
                                    bufs=1)
                    nc.sync.dma_start(
                        out=wtmtd,
                        in_=wbig[l][:, WB_TM1:WB_TM1 + 224]
                        .rearrange("(k p) n -> p k n", p=128))
                    for i in range(CT):
                        nc.tensor.matmul(out=p160a, lhsT=wtmtd[:, i, 0:128], rhs=mx[i],
                                         start=(i == 0), stop=(i == CT - 1))
                        nc.tensor.matmul(out=p160b, lhsT=wtmtd[:, i, 128:160], rhs=mx[i],
                                         start=(i == 0), stop=(i == CT - 1))
                    t5 = [k2.tile([32, TTW], BF16, tag=f"t5{f}", name=f"t5{f}", bufs=1) for f in range(5)]
                    for f in range(4):
                        tanh_route(p160a[f * 32:(f + 1) * 32, :], t5[f])
                    tanh_route(p160b, t5[4])

                    def build_xf(fidx, maa_i):
                        w2 = load_w(tmw2[l][fidx * TM:(fidx + 1) * TM, :],
                                    [TM, C], "wtm2")
                        xft = []
                        for i in range(CT):
                            dlp = psB.tile([128, TTW], F32, tag="pw", name="pw")
                            nc.tensor.matmul(out=dlp,
                                             lhsT=w2[:, i * 128:(i + 1) * 128],
                                             rhs=t5[fidx], start=True, stop=True)
                            a = k2.tile([128, TTW], F32, tag="lnt1", name="lnt1")
                            nc.vector.scalar_tensor_tensor(
                                out=a, in0=dlp, scalar=vcol(vecs, i, maa_i),
                                in1=xx[i], op0=OP.add, op1=OP.mult)
                            xf = k2.tile([128, TTW], BF16, tag=f"xf{i}", name=f"xf{i}")
                            nc.vector.tensor_add(out=xf, in0=a, in1=xt[i])
                            xft.append(xf)
                        return xft

                    def wmm(col_off, xft):
                        accs = [psA.tile([128, TTW], F32, tag=f"acc{m}",
                                         name=f"acc{m}") for m in range(4)]
                        wt = load_wblk(wbig[l][:, col_off:col_off + 512],
                                       [128, CT, 512])
                        for i in range(CT):
                            for m in range(4):
                                nc.tensor.matmul(out=accs[m],
                                                 lhsT=wt[:, i, m * 128:(m + 1) * 128],
                                                 rhs=xft[i], start=(i == 0),
                                                 stop=(i == CT - 1))
                        return accs

                    def wmm_tm(col_off, xft):
                        accs = [psA.tile([128, TTW], F32, tag=f"acc{m}",
                                         name=f"acc{m}") for m in range(4)]
                        wt = load_wblk(wbig[l][:, col_off:col_off + 512],
                                       [128, CT, 512])
                        for i in range(CT):
                            for ci in range(NCPT):
                                nc.tensor.matmul(out=accs[ci],
                                                 lhsT=xft[i][:, ci * L:(ci + 1) * L],
                                                 rhs=wt[:, i, :], start=(i == 0),
                                                 stop=(i == CT - 1))
                        return accs

                    # k
                    xf = build_xf(1, V_MAAK)
                    accs = wmm(WB_K, xf)
                    k_sb = [kp.tile([128, TTW], F32, tag=f"ksb{i}", name=f"ksb{i}") for i in range(CT)]
                    for m in range(4):
                        nc.scalar.activation(out=k_sb[m], in_=accs[m], func=AF.Copy)
                    # v token-major
                    xf = build_xf(2, V_MAAV)
                    accs = wmm_tm(WB_V, xf)
                    v_tm = [kp.tile([128, C], F32, tag=f"vtm{ci}", name=f"vtm{ci}") for ci in range(NCPT)]
                    for ci in range(NCPT):
                        nc.scalar.activation(out=v_tm[ci], in_=accs[ci], func=AF.Copy)
                    # r
                    xf = build_xf(3, V_MAAR)
                    accs = wmm(WB_R, xf)
                    r_sb = [kp.tile([128, TTW], F32, tag=f"rsb{i}", name=f"rsb{i}") for i in range(CT)]
                    for m in range(4):
                        nc.scalar.activation(out=r_sb[m], in_=accs[m], func=AF.Copy)
                    if dbg and l == 0:
                        for i in range(CT):
                            nc.sync.dma_start(out=dbg_o["r0"][i * 128:(i + 1) * 128, sl],
                                              in_=r_sb[i])
                    # g token-major, silu
                    xf = build_xf(4, V_MAAG)
                    accs = wmm_tm(WB_G, xf)
                    g_tm = [kp.tile([128, C], F32, tag=f"gtm{ci}", name=f"gtm{ci}") for ci in range(NCPT)]
                    for ci in range(NCPT):
                        nc.scalar.activation(out=g_tm[ci], in_=accs[ci], func=AF.Silu)
                    # w -> wacc -> lai
                    xf = build_xf(0, V_MAAW)
                    tdp = psC.tile([TD, TTW], F32, tag="stA", name="stA")
                    for i in range(CT):
                        nc.tensor.matmul(out=tdp, lhsT=wtmtd[:, i, 160:224], rhs=xf[i],
                                         start=(i == 0), stop=(i == CT - 1))
                    tdt = k2.tile([TD, TTW], BF16, tag="tdt", name="tdt", bufs=1)
                    tanh_route(tdp, tdt)
                    w2t = load_w(tdw2[l][:, :], [TD, C], "wtd2", bufs=1)
                    lai = [kp.tile([128, 1 + TTW], F32, tag=f"lai{i}", name=f"lai{i}") for i in range(CT)]
                    for i in range(CT):
                        wwp = psB.tile([128, TTW], F32, tag="pw", name="pw")
                        nc.tensor.matmul(out=wwp, lhsT=w2t[:, i * 128:(i + 1) * 128],
                                         rhs=tdt, start=True, stop=True)
                        wacc = k2.tile([128, TTW], F32, tag="lnt1", name="lnt1")
                        nc.scalar.activation(out=wacc, in_=wwp, func=AF.Exp,
                                             bias=vcol(vecs, i, V_TDCY))
                        nc.gpsimd.memset(lai[i][:, 0:1], 0.0)
                        nc.vector.tensor_tensor_scan(
                            out=lai[i][:, 1:1 + TTW], data0=wacc, data1=wacc,
                            initial=0.0, op0=OP.add, op1=OP.bypass)
                        if dbg and l == 0:
                            nc.sync.dma_start(
                                out=dbg_o["w0"][i * 128:(i + 1) * 128, sl], in_=wacc)
                            nc.sync.dma_start(
                                out=dbg_o["lai0"][i * 128:(i + 1) * 128, sl],
                                in_=lai[i][:, 1:1 + TTW])

                    # ---- wkv chunks ----
                    ztc = [kp.tile([128, TTW], BF16, tag=f"ztc{i}", name=f"ztc{i}") for i in range(CT)]
                    for ci in range(NCPT):
                        gc = tt * NCPT + ci
                        c0 = ci * L
                        fpc, fnc, rt_t, kt_t, kh_tm, m_t = [], [], [], [], [], []
                        for i in range(CT):
                            ngc = k2.tile([128, 1], F32, tag="ngc", name="ngc", bufs=4)
                            nc.vector.tensor_scalar_mul(out=ngc,
                                                        in0=lai[i][:, c0:c0 + 1],
                                                        scalar1=-1.0)
                            fp = k2.tile([128, 1 + L], F32, tag="fp", name="fp", bufs=4)
                            nc.scalar.activation(out=fp, in_=lai[i][:, c0:c0 + 1 + L],
                                                 func=AF.Exp, bias=ngc)
                            fn = k2.tile([128, 1 + L], F32, tag="fn", name="fn", bufs=4)
                            nc.scalar.activation(out=fn, in_=lai[i][:, c0:c0 + 1 + L],
                                                 func=AF.Exp, scale=-1.0,
                                                 bias=lai[i][:, c0:c0 + 1])
                            fpc.append(fp)
                            fnc.append(fn)
                            rt = k2.tile([128, L], F32, tag="rt", name="rt", bufs=4)
                            nc.vector.tensor_mul(out=rt, in0=r_sb[i][:, c0:c0 + L],
                                                 in1=fn[:, 0:L])
                            kt = k2.tile([128, L], F32, tag="kt", name="kt", bufs=4)
                            nc.vector.tensor_mul(out=kt, in0=k_sb[i][:, c0:c0 + L],
                                                 in1=fp[:, 1:1 + L])
                            kh = k2.tile([128, L], F32, tag="kh", name="kh", bufs=4)
                            nc.vector.tensor_scalar_mul(out=kh, in0=kt,
                                                        scalar1=fn[:, L:L + 1])
                            mt = k2.tile([128, L], F32, tag="mt", name="mt", bufs=4)
                            nc.gpsimd.tensor_mul(out=mt, in0=r_sb[i][:, c0:c0 + L],
                                                 in1=k_sb[i][:, c0:c0 + L])
                            rt_t.append(rt)
                            kt_t.append(kt)
                            m_t.append(mt)
                            trp = psB.tile([128, L], F32, tag="pw", name="pw")
                            nc.tensor.transpose(out=trp, in_=kh, identity=ident)
                            kht = k2.tile([128, L], F32, tag="khtm", name="khtm", bufs=4)
                            nc.scalar.activation(out=kht, in_=trp, func=AF.Copy)
                            kh_tm.append(kht)
                        dall = psC.tile([128, 8], F32, tag="stB", name="stB")
                        for i in range(CT):
                            nc.tensor.matmul(out=dall, lhsT=m_t[i], rhs=hmu[i],
                                             start=(i == 0), stop=(i == CT - 1))
                        yps = psA.tile([128, C], F32, tag="acc0", name="acc0")
                        S_new = [k2.tile([128, HN], F32, tag=f"Sn{i}", name=f"Sn{i}") for i in range(CT)]
                        for i in range(CT):
                            sup = psC.tile([128, HN], F32, tag="stA", name="stA")
                            for hh in range(2):
                                h = 2 * i + hh
                                hb = hh * HN
                                pt = psB.tile([L, L], F32, tag="pw", name="pw")
                                nc.tensor.matmul(out=pt, lhsT=kt_t[i][hb:hb + HN, :],
                                                 rhs=rt_t[i][hb:hb + HN, :],
                                                 start=True, stop=True)
                                pts = k2.tile([L, L], F32, tag="pts", name="pts")
                                nc.vector.tensor_mul(out=pts, in0=pt, in1=triu)
                                nc.tensor.matmul(
                                    out=yps[:, h * HN:(h + 1) * HN], lhsT=pts,
                                    rhs=v_tm[ci][:, h * HN:(h + 1) * HN],
                                    start=True, stop=(gc == 0), skip_group_check=True)
                                if gc > 0:
                                    nc.tensor.matmul(
                                        out=yps[:, h * HN:(h + 1) * HN],
                                        lhsT=rt_t[i][hb:hb + HN, :],
                                        rhs=S_cur[i][hb:hb + HN, :],
                                        start=False, stop=True, skip_group_check=True)
                                nc.tensor.matmul(
                                    out=sup[hb:hb + HN, :],
                                    lhsT=kh_tm[i][:, hb:hb + HN],
                                    rhs=v_tm[ci][:, h * HN:(h + 1) * HN],
                                    start=True, stop=True, skip_group_check=True)
                            t0 = k2.tile([128, HN], F32, tag="ssc", name="ssc", bufs=4)
                            nc.vector.tensor_scalar_mul(out=t0, in0=S_cur[i],
                                                        scalar1=fnc[i][:, L:L + 1])
                            nc.vector.tensor_add(out=S_new[i], in0=t0, in1=sup)
                        S_cur = S_new
                        ysb = k2.tile([128, C], F32, tag="ysb", name="ysb")
                        for h in range(H):
                            nc.vector.scalar_tensor_tensor(
                                out=ysb[:, h * HN:(h + 1) * HN],
                                in0=v_tm[ci][:, h * HN:(h + 1) * HN],
                                scalar=dall[:, h:h + 1],
                                in1=yps[:, h * HN:(h + 1) * HN],
                                op0=OP.mult, op1=OP.add)
                        if dbg and l == 0:
                            nc.sync.dma_start(
                                out=dbg_o["y0tm"][tt * TTW + c0:tt * TTW + c0 + L, :],
                                in_=ysb)
                            nc.sync.dma_start(
                                out=dbg_o["g0tm"][tt * TTW + c0:tt * TTW + c0 + L, :],
                                in_=g_tm[ci])
                        # groupnorm + affine + *g  (token-major)
                        mv = k2.tile([128, 16], F32, tag="gnmv", name="gnmv")
                        for h in range(H):
                            st = k2.tile([128, 6], F32, tag="gnst", name="gnst")
                            nc.vector.bn_stats(out=st, in_=ysb[:, h * HN:(h + 1) * HN])
                            nc.vector.bn_aggr(out=mv[:, 2 * h:2 * h + 2], in_=st)
                        lnv = k2.tile([128, 8], F32, tag="gnln", name="gnln")
                        var_view = bass.AP(tensor=mv.tensor, offset=mv.offset + 1,
                                           ap=[mv.ap[0], [2, 8]])
                        nc.scalar.activation(out=lnv, in_=var_view, func=AF.Ln,
                                             bias=eps_gn_t)
                        rsg = k2.tile([128, 8], F32, tag="gnrs", name="gnrs")
                        nc.scalar.activation(out=rsg, in_=lnv, func=AF.Exp, scale=-0.5)
                        for h in range(H):
                            nc.vector.tensor_scalar(
                                out=ysb[:, h * HN:(h + 1) * HN],
                                in0=ysb[:, h * HN:(h + 1) * HN],
                                scalar1=mv[:, 2 * h:2 * h + 1],
                                scalar2=rsg[:, h:h + 1],
                                op0=OP.subtract, op1=OP.mult)
                        nc.gpsimd.tensor_mul(out=ysb, in0=ysb, in1=lnxt[:, 0:512])
                        nc.gpsimd.tensor_add(out=ysb, in0=ysb, in1=lnxt[:, 512:1024])
                        nc.vector.tensor_mul(out=ysb, in0=ysb, in1=g_tm[ci])
                        for i in range(CT):
                            trp = psB.tile([128, L], F32, tag="pw", name="pw")
                            nc.tensor.transpose(out=trp,
                                                in_=ysb[:, i * 128:(i + 1) * 128],
                                                identity=ident)
                            nc.scalar.activation(out=ztc[i][:, c0:c0 + L], in_=trp,
                                                 func=AF.Copy)
                    # ---- Wo ----
                    accs = [psA.tile([128, TTW], F32, tag=f"acc{m}",
                                     name=f"acc{m}") for m in range(4)]
                    wt = load_wblk(wbig[l][:, WB_O:WB_O + 512], [128, CT, 512])
                    for i in range(CT):
                        for m in range(4):
                            nc.tensor.matmul(out=accs[m],
                                             lhsT=wt[:, i, m * 128:(m + 1) * 128],
                                             rhs=ztc[i], start=(i == 0),
                                             stop=(i == CT - 1))
                    for m in range(4):
                        nc.vector.tensor_add(out=xb[m][:, sl], in0=xb[m][:, sl],
                                             in1=accs[m])
                    if dbg and l == 0:
                        for i in range(CT):
                            nc.sync.dma_start(
                                out=dbg_o["xbtm0"][i * 128:(i + 1) * 128, sl],
                                in_=xb[i][:, sl].bitcast(F32))

                # ================= channel mix =================
                for tt in range(NTT):
                    sl = slice(tt * TTW, (tt + 1) * TTW)
                    xbs = [xb[i][:, sl] for i in range(CT)]
                    bc0, bc1 = ln_stats(xbs)
                    xc = [kp.tile([128, TTW], F32, tag=f"xt{i}", name=f"xt{i}") for i in range(CT)]
                    ln_apply(xbs, bc0, bc1, vecs, V_LN2W, V_LN2B, xc)
                    xx2 = [kp.tile([128, TTW], F32, tag=f"xx{i}", name=f"xx{i}") for i in range(CT)]
                    for i in range(CT):
                        nc.vector.tensor_sub(out=xx2[i][:, 1:TTW],
                                             in0=xc[i][:, 0:TTW - 1],
                                             in1=xc[i][:, 1:TTW])
                        nc.vector.scalar_tensor_tensor(
                            out=xx2[i][:, 0:1], in0=carry2[i], scalar=1.0,
                            in1=xc[i][:, 0:1], op0=OP.mult, op1=OP.subtract)
                        nc.gpsimd.tensor_copy(out=carry2[i], in_=xc[i][:, TTW - 1:TTW])
                    xk2 = [kp.tile([128, TTW], BF16, tag=f"xk2{i}", name=f"xk2{i}") for i in range(CT)]
                    xr2 = [kp.tile([128, TTW], BF16, tag=f"xr2{i}", name=f"xr2{i}") for i in range(CT)]
                    for i in range(CT):
                        nc.vector.scalar_tensor_tensor(
                            out=xk2[i], in0=xx2[i], scalar=vcol(vecs, i, V_CMK),
                            in1=xc[i], op0=OP.mult, op1=OP.add)
                        nc.vector.scalar_tensor_tensor(
                            out=xr2[i], in0=xx2[i], scalar=vcol(vecs, i, V_CMR),
                            in1=xc[i], op0=OP.mult, op1=OP.add)
                    # cm_Wr -> sigmoid
                    accs = [psA.tile([128, TTW], F32, tag=f"acc{m}", name=f"acc{m}") for m in range(4)]
                    wt = load_wblk(cmrg[l][:, 0:512], [128, CT, 512])
                    for i in range(CT):
                        for m in range(4):
                            nc.tensor.matmul(out=accs[m],
                                             lhsT=wt[:, i, m * 128:(m + 1) * 128],
                                             rhs=xr2[i], start=(i == 0),
                                             stop=(i == CT - 1))
                    sig = [kp.tile([128, TTW], F32, tag=f"gtm{m}", name=f"sig{m}") for m in range(4)]
                    for m in range(4):
                        nc.scalar.activation(out=sig[m], in_=accs[m], func=AF.Sigmoid)
                    # kk loop with cm_Wv accumulation
                    accs = [psA.tile([128, TTW], F32, tag=f"acc{m}", name=f"acc{m}") for m in range(4)]
                    wfq = wvq = None
                    for f in range(NFF):
                        fq, fr2 = f // 4, f % 4
                        nq = min(4, NFF - 4 * fq)
                        if fr2 == 0:
                            wfq = wp.tile([128, nq, CT, 128], BF16, tag="wblk",
                                          name="wfq", bufs=2)
                            nc.sync.dma_start(
                                out=wfq,
                                in_=cmkp[l][4 * fq:4 * fq + nq]
                                .rearrange("f (k p) n -> p f k n", p=128))
                            wvq = wp.tile([128, nq, C], BF16, tag="wblk",
                                          name="wvq", bufs=2)
                            nc.sync.dma_start(
                                out=wvq,
                                in_=cmvp[l][4 * fq * 128:(4 * fq + nq) * 128, :]
                                .rearrange("(f p) n -> p f n", p=128))
                        kkp = psB.tile([128, TTW], F32, tag="pw", name="pw")
                        for i in range(CT):
                            nc.tensor.matmul(out=kkp, lhsT=wfq[:, fr2, i, :],
                                             rhs=xk2[i],
                                             start=(i == 0), stop=(i == CT - 1))
                        rl = k2.tile([128, TTW], F32, tag="lnt1", name="lnt1")
                        nc.scalar.activation(out=rl, in_=kkp, func=AF.Relu)
                        kkf = k2.tile([128, TTW], BF16, tag="lnt2k", name="lnt2k")
                        nc.scalar.activation(out=kkf, in_=rl, func=AF.Square)
                        for m in range(4):
                            nc.tensor.matmul(out=accs[m],
                                             lhsT=wvq[:, fr2, m * 128:(m + 1) * 128],
                                             rhs=kkf, start=(f == 0),
                                             stop=(f == NFF - 1))
                    for m in range(4):
                        nc.vector.tensor_mul(out=sig[m], in0=sig[m], in1=accs[m])
                        nc.vector.tensor_add(out=xb[m][:, sl], in0=xb[m][:, sl],
                                             in1=sig[m])
                if dbg and l == 0:
                    for i in range(CT):
                        nc.sync.dma_start(out=dbg_o["p0"][i * 128:(i + 1) * 128, :],
                                          in_=xb[i].bitcast(F32))

                # ================= exchange + join =================
                send = dp.tile([C, T], F32, tag="send", name="send")
                recv = dp.tile([2 * C, T], F32, tag="recv", name="recv")
                for i in range(CT):
                    # time-reverse in SBUF (DVE handles the negative stride),
                    # then one contiguous DMA — a reversed DRAM AP would shatter
                    # into 128*1024 4-byte descriptors and stall every queue.
                    rv = k2.tile([128, T], F32, tag="revst", name="revst")
                    nc.vector.tensor_copy(out=rv, in_=_revap(xb[i][:, :].bitcast(F32)))
                    nc.sync.dma_start(out=send[i * 128:(i + 1) * 128, :], in_=rv)
                if solo:
                    for i in range(CT):
                        nc.sync.dma_start(out=recv[i * 128:(i + 1) * 128, :],
                                          in_=xb[i].bitcast(F32))
                        nc.sync.dma_start(out=recv[C + i * 128:C + (i + 1) * 128, :],
                                          in_=xb[i].bitcast(F32))
                else:
                    nc.gpsimd.collective_compute(
                        "AllGather", OP.bypass, replica_groups=groups,
                        ins=[send.opt()], outs=[recv.opt()])
                for tt in range(NTT):
                    sl = slice(tt * TTW, (tt + 1) * TTW)
                    # conv residue, all ct (fp32 accumulation, bf16 final for matmul)
                    cv32 = [kp.tile([128, TTW], F32, tag=f"ksb{i}", name=f"ksb{i}") for i in range(CT)]
                    cv = [k2.tile([128, TTW], BF16, tag=f"cvb{i}", name=f"cvb{i}") for i in range(CT)]
                    a = tt * TTW
                    for i in range(CT):
                        nc.vector.tensor_scalar_mul(out=cv32[i], in0=xres[i][:, sl],
                                                    scalar1=vcol(vecs, i, V_CW1))
                        lo = 1 if tt == 0 else 0
                        nc.vector.scalar_tensor_tensor(
                            out=cv32[i][:, lo:TTW],
                            in0=xres[i][:, a + lo - 1:a + TTW - 1],
                            scalar=vcol(vecs, i, V_CW0),
                            in1=cv32[i][:, lo:TTW], op0=OP.mult, op1=OP.add)
                        hi = TTW - 1 if tt == NTT - 1 else TTW
                        nc.vector.scalar_tensor_tensor(
                            out=cv[i][:, 0:hi],
                            in0=xres[i][:, a + 1:a + hi + 1],
                            scalar=vcol(vecs, i, V_CW2),
                            in1=cv32[i][:, 0:hi], op0=OP.mult, op1=OP.add)
                        if hi < TTW:
                            nc.scalar.activation(out=cv[i][:, hi:TTW],
                                                 in_=cv32[i][:, hi:TTW], func=AF.Copy)
                    accs = [psA.tile([128, TTW], F32, tag=f"acc{m}", name=f"acc{m}") for m in range(4)]
                    wt = load_wblk(cmrg[l][:, 512:1024], [128, CT, 512])
                    for i in range(CT):
                        for m in range(4):
                            nc.tensor.matmul(out=accs[m],
                                             lhsT=wt[:, i, m * 128:(m + 1) * 128],
                                             rhs=cv[i], start=(i == 0),
                                             stop=(i == CT - 1))
                    for m in range(4):
                        jr0 = kp.tile([128, TTW], F32, tag="vtm0", name="vtm0")
                        jr1 = kp.tile([128, TTW], F32, tag="vtm1", name="vtm1")
                        nc.sync.dma_start(out=jr0, in_=recv[m * 128:(m + 1) * 128, sl])
                        nc.sync.dma_start(out=jr1,
                                          in_=recv[C + m * 128:C + (m + 1) * 128, sl])
                        nc.vector.tensor_scalar_mul(
                            out=jr0, in0=jr0, scalar1=selt[:, S_ALPHA:S_ALPHA + 1])
                        nc.vector.scalar_tensor_tensor(
                            out=jr1, in0=jr1, scalar=selt[:, S_BETA:S_BETA + 1],
                            in1=jr0, op0=OP.mult, op1=OP.add)
                        sg = k2.tile([128, TTW], F32, tag="er0", name="er0", bufs=2)
                        nc.scalar.activation(out=sg, in_=accs[m], func=AF.Sigmoid,
                                             scale=selt[:, S_NEGS:S_NEGS + 1],
                                             bias=vcol(vecs, m, V_GBM))  # w_recv
                        wown = kp.tile([128, TTW], F32, tag="vtm3", name="vtm3")
                        nc.vector.tensor_scalar(out=wown, in0=sg, scalar1=-1.0,
                                                scalar2=1.0, op0=OP.mult, op1=OP.add)
                        jsum = kp.tile([128, TTW], F32, tag="vtm2", name="vtm2")
                        nc.vector.tensor_mul(out=jsum, in0=wown, in1=xb[m][:, sl])
                        nc.vector.tensor_mul(out=jr1, in0=sg, in1=jr1)
                        nc.vector.tensor_add(out=jsum, in0=jsum, in1=jr1)
                        # note: host mask05 already includes the 2x factor fold:
                        # mask05 = mask (not 0.5*mask) since sigmoid form used.
                        nc.vector.tensor_mul(out=xres[m][:, sl], in0=jsum,
                                             in1=maskt[:, sl])
            # ---- output ----
            for i in range(CT):
                nc.sync.dma_start(out=xout[i * 128:(i + 1) * 128, :],
                                  in_=xres[i].bitcast(F32))
    nc.compile()
    return nc


def _host_inputs(inputs):
    x = np.asarray(inputs["x"], np.float32)
    lengths = np.asarray(inputs["lengths"]).astype(np.int64)
    pos = np.arange(T, dtype=np.float32)[:, None]
    div = np.exp(np.arange(0, C, 2, dtype=np.float32) * (-np.log(10000.0) / C))
    pe = np.zeros((T, C), np.float32)
    pe[:, 0::2] = np.sin(pos * div)
    pe[:, 1::2] = np.cos(pos * div)
    mask = (np.arange(T)[None, :] < lengths[:, None]).astype(np.float32)

    consts = np.zeros((128, 384), np.float32)
    consts[:, 0:128] = np.eye(128, dtype=np.float32)
    consts[:, 128:256] = np.triu(np.ones((128, 128), np.float32), 1)
    consts[:, 256:384] = 1.0

    gw = np.asarray(inputs["gate_w"], np.float32)
    gb = np.asarray(inputs["gate_b"], np.float32)
    cw = np.asarray(inputs["conv_w"], np.float32)
    cb = np.asarray(inputs["conv_b"], np.float32)

    in_maps = []
    for c in range(8):
        b, d = c % 4, c // 4
        rev = d == 1
        s = -1.0 if rev else 1.0
        xin = (x[b] + pe)
        mrow = mask[b]
        if rev:
            xin = xin[::-1]
            mrow = mrow[::-1]
        m = {
            "x0": np.ascontiguousarray(xin.T),
            "mask05": np.ascontiguousarray(np.broadcast_to(mrow, (128, T))),
            "consts": consts,
        }
        sel = np.zeros((128, 8), np.float32)
        sel[:, S_LN0] = 0.0 if rev else 1.0
        sel[:, S_NEGS] = -s
        sel[:, S_ALPHA] = 1.0 if rev else 0.0
        sel[:, S_BETA] = 0.0 if rev else 1.0
        m["sel"] = sel
        for l in range(NL):
            W = {k: np.asarray(inputs[k], np.float32)[d, l]
                 for k in ["ln1_w", "ln1_b", "ln2_w", "ln2_b", "maa_x", "maa_w",
                           "maa_k", "maa_v", "maa_r", "maa_g", "tm_w1", "tm_w2",
                           "td_w1", "td_w2", "time_decay", "Wr", "Wk", "Wv",
                           "Wg", "Wo", "lnx_w", "lnx_b", "cm_maa_k", "cm_maa_r",
                           "cm_Wk", "cm_Wv", "cm_Wr", "time_faaaa"]}
            import ml_dtypes
            bf16 = ml_dtypes.bfloat16
            m[f"wbig{l}"] = np.ascontiguousarray(np.concatenate(
                [W["Wr"], W["Wk"], W["Wv"], W["Wg"], W["tm_w1"], W["td_w1"],
                 W["Wo"]], axis=1).astype(bf16))
            m[f"tmw2{l}"] = np.ascontiguousarray(
                W["tm_w2"].reshape(5 * TM, C).astype(bf16))
            m[f"tdw2{l}"] = np.ascontiguousarray(W["td_w2"].astype(bf16))
            m[f"cmk{l}"] = np.ascontiguousarray(
                W["cm_Wk"].reshape(C, NFF, 128).transpose(1, 0, 2).astype(bf16))
            m[f"cmv{l}"] = np.ascontiguousarray(W["cm_Wv"].astype(bf16))
            m[f"cmrg{l}"] = np.ascontiguousarray(
                np.concatenate([W["cm_Wr"], gw[l]], axis=1).astype(bf16))
            cwe = cw[l] if not rev else cw[l][:, ::-1]
            gbe = cb[l] @ gw[l] + gb[l]
            vec = np.zeros((C, NV), np.float32)
            vec[:, V_LN1W] = W["ln1_w"]; vec[:, V_LN1B] = W["ln1_b"]
            vec[:, V_LN2W] = W["ln2_w"]; vec[:, V_LN2B] = W["ln2_b"]
            vec[:, V_MAAX] = W["maa_x"]; vec[:, V_MAAW] = W["maa_w"]
            vec[:, V_MAAK] = W["maa_k"]; vec[:, V_MAAV] = W["maa_v"]
            vec[:, V_MAAR] = W["maa_r"]; vec[:, V_MAAG] = W["maa_g"]
            vec[:, V_TDCY] = W["time_decay"]
            vec[:, V_CMK] = W["cm_maa_k"]; vec[:, V_CMR] = W["cm_maa_r"]
            vec[:, V_GBM] = -s * gbe
            vec[:, V_CW0] = cwe[:, 0]
            vec[:, V_CW1] = cwe[:, 1] - 1.0
            vec[:, V_CW2] = cwe[:, 2]
            vec[:, V_LN0W] = np.asarray(inputs["ln0_w"], np.float32)
            vec[:, V_LN0B] = np.asarray(inputs["ln0_b"], np.float32)
            m[f"vecs{l}"] = vec
            lnx = np.zeros((128, 1024), np.float32)
            lnx[:, 0:512] = W["lnx_w"][None, :]
            lnx[:, 512:1024] = W["lnx_b"][None, :]
            m[f"lnx{l}"] = lnx
            u = W["time_faaaa"].reshape(C)
            hmu = np.zeros((C, 8), np.float32)
            for h in range(H):
                hmu[h * HN:(h + 1) * HN, h] = u[h * HN:(h + 1) * HN]
            m[f"hmu{l}"] = hmu
        in_maps.append(m)
    return in_maps


def kernel(**inputs):
    if "nc" not in _CACHE:
        _CACHE["nc"] = _build(dbg=False)
    nc = _CACHE["nc"]
    in_maps = _host_inputs(inputs)
    res = run_bass_kernel_spmd(nc, in_maps, list(range(8)))
    out = np.empty((B, T, C), np.float32)
    for b in range(B):
        out[b] = res.results[b]["xout"].T
    return out


if __name__ == "__main__":
    rng = np.random.default_rng(0)
    demo = None



# revision 28
# speedup vs baseline: 11.0209x; 1.1522x over previous
"""BiWKV6 encoder kernel for 8 Trainium2 NeuronCores.

Sharding: (batch, direction) -> 8 units, one per core; core c handles
batch c % 4, direction c // 4. Backward cores run the identical SPMD
program on time-reversed inputs; the only cross-core communication is a
pairwise AllGather of each layer's block output, written time-reversed
into the partner's domain. Within a core activations are channel-major
[C, T]; the WKV scan uses the chunked linear-attention formulation
(chunk 128) with log-space cumulative decay from the DVE prefix scan.
"""
import numpy as np

import concourse.bass as bass
import concourse.tile as tile
from concourse import bacc, mybir
from concourse.bass_utils import run_bass_kernel_spmd

B, T, C = 4, 1024, 512
H, HN = 8, 64
L = 128
TTW = 512
NTT = T // TTW
NCPT = TTW // L
CT = C // 128
TM, TD, FFN, NL = 32, 64, 1792, 2
NFF = FFN // 128
EPS_LN, EPS_GN = 1e-5, 64e-5
NV = 19

F32 = mybir.dt.float32
F32R = mybir.dt.float32r
BF16 = mybir.dt.bfloat16
AF = mybir.ActivationFunctionType
OP = mybir.AluOpType

WB_R, WB_K, WB_V, WB_G, WB_TM1, WB_TD1, WB_O = 0, 512, 1024, 1536, 2048, 2208, 2272
WB_COLS = 2784
(V_LN1W, V_LN1B, V_LN2W, V_LN2B, V_MAAX, V_MAAW, V_MAAK, V_MAAV, V_MAAR,
 V_MAAG, V_TDCY, V_CMK, V_CMR, V_GBM, V_CW0, V_CW1, V_CW2, V_LN0W,
 V_LN0B) = range(NV)
S_LN0, S_NEGS, S_ALPHA, S_BETA = 0, 1, 2, 3

_CACHE = {}


def _revap(ap):
    n = ap.ap[-1][1]
    return bass.AP(tensor=ap.tensor, offset=ap.offset + (n - 1) * ap.ap[-1][0],
                   ap=[ap.ap[0], [-ap.ap[-1][0], n]])


def _build(dbg=False, solo=False):
    nc = bacc.Bacc("TRN2", target_bir_lowering=False, debug=False, num_devices=8)

    x0 = nc.declare_dram_parameter("x0", [C, T], F32, isOutput=False)
    mask05 = nc.declare_dram_parameter("mask05", [128, T], F32, isOutput=False)
    sel_in = nc.declare_dram_parameter("sel", [128, 8], F32, isOutput=False)
    consts = nc.declare_dram_parameter("consts", [128, 384], F32, isOutput=False)
    wbig, tmw2, tdw2, cmkp, cmvp, cmrg, vecs_in, lnx_in, hmu_in = \
        [], [], [], [], [], [], [], [], []
    for l in range(NL):
        wbig.append(nc.declare_dram_parameter(f"wbig{l}", [C, WB_COLS], BF16, isOutput=False))
        tmw2.append(nc.declare_dram_parameter(f"tmw2{l}", [5 * TM, C], BF16, isOutput=False))
        tdw2.append(nc.declare_dram_parameter(f"tdw2{l}", [TD, C], BF16, isOutput=False))
        cmkp.append(nc.declare_dram_parameter(f"cmk{l}", [NFF, C, 128], BF16, isOutput=False))
        cmvp.append(nc.declare_dram_parameter(f"cmv{l}", [FFN, C], BF16, isOutput=False))
        cmrg.append(nc.declare_dram_parameter(f"cmrg{l}", [C, 1024], BF16, isOutput=False))
        vecs_in.append(nc.declare_dram_parameter(f"vecs{l}", [C, NV], F32, isOutput=False))
        lnx_in.append(nc.declare_dram_parameter(f"lnx{l}", [128, 1024], F32, isOutput=False))
        hmu_in.append(nc.declare_dram_parameter(f"hmu{l}", [C, 8], BF16, isOutput=False))
    xout = nc.declare_dram_parameter("xout", [C, T], F32, isOutput=True)
    dbg_o = {}
    if dbg:
        for nm in ["xt0", "r0", "lai0", "xbtm0", "p0", "w0"]:
            dbg_o[nm] = nc.declare_dram_parameter(nm, [C, T], F32, isOutput=True)
        for nm in ["y0tm", "g0tm"]:
            dbg_o[nm] = nc.declare_dram_parameter(nm, [T, C], F32, isOutput=True)

    groups = [[0, 4], [1, 5], [2, 6], [3, 7]]

    with tile.TileContext(nc) as tc:
        with (
            tc.tile_pool(name="pp", bufs=1) as pp,
            tc.tile_pool(name="wp", bufs=2) as wp,
            tc.tile_pool(name="kp", bufs=1) as kp,
            tc.tile_pool(name="k2", bufs=2) as k2,
            tc.tile_pool(name="psA", bufs=1, space="PSUM") as psA,
            tc.tile_pool(name="psB", bufs=2, space="PSUM") as psB,
            tc.tile_pool(name="psC", bufs=1, space="PSUM") as psC,
            tc.tile_pool(name="dp", bufs=2, space="DRAM") as dp,
        ):
            # ------------- persistent loads -------------
            xres = [pp.tile([128, T], F32R, tag=f"xres{i}", name=f"xres{i}") for i in range(CT)]
            xb = [pp.tile([128, T], F32R, tag=f"xb{i}", name=f"xb{i}") for i in range(CT)]
            for i in range(CT):
                nc.sync.dma_start(out=xres[i],
                                  in_=x0[i * 128:(i + 1) * 128, :].bitcast(F32R))
            maskt = pp.tile([128, T], F32, tag="mask", name="mask")
            nc.sync.dma_start(out=maskt, in_=mask05[:, :])
            selt = pp.tile([128, 8], F32, tag="sel", name="sel")
            nc.sync.dma_start(out=selt, in_=sel_in[:, :])
            cst = pp.tile([128, 384], F32, tag="consts", name="consts")
            nc.sync.dma_start(out=cst, in_=consts[:, :])
            eps_ln_t = pp.tile([128, 1], F32, tag="epsln", name="epsln")
            nc.vector.memset(eps_ln_t, EPS_LN)
            eps_gn_t = pp.tile([128, 1], F32, tag="epsgn", name="epsgn")
            nc.vector.memset(eps_gn_t, EPS_GN)
            ident = cst[:, 0:128]
            triu = cst[:, 128:256]
            identb = pp.tile([128, 128], BF16, tag="identb", name="identb")
            nc.scalar.activation(out=identb, in_=ident, func=AF.Copy)
            onesr_t = pp.tile([128, 128], F32R, tag="onesr", name="onesr")
            nc.sync.dma_start(out=onesr_t, in_=consts[:, 256:384].bitcast(F32R))
            ones_r = onesr_t

            def vcol(vt, i, j):
                return vt[i][:, j:j + 1]

            def load_w(dram_ap, shape, tag, bufs=2):
                t = wp.tile(shape, BF16, tag=tag, name="wld", bufs=bufs)
                nc.sync.dma_start(out=t, in_=dram_ap)
                return t

            def load_wblk(dram_2d, shape):
                # one DMA for a [C, w] weight block -> SBUF [128, CT, w]
                t = wp.tile(shape, BF16, tag="wblk", name="wblk", bufs=2)
                nc.sync.dma_start(
                    out=t, in_=dram_2d.rearrange("(k p) n -> p k n", p=128))
                return t

            def ln_stats(src_sl):
                ssum = psC.tile([1, TTW], F32, tag="stA", name="stA")
                ssq = psC.tile([1, TTW], F32, tag="stB", name="stB")
                for i in range(CT):
                    sq = k2.tile([128, TTW], F32R, tag="lnt1", name="lnt1")
                    nc.vector.tensor_mul(out=sq, in0=src_sl[i], in1=src_sl[i])
                    nc.tensor.matmul(out=ssum, lhsT=ones_r[:, 0:1], rhs=src_sl[i],
                                     start=(i == 0), stop=(i == CT - 1))
                    nc.tensor.matmul(out=ssq, lhsT=ones_r[:, 0:1], rhs=sq,
                                     start=(i == 0), stop=(i == CT - 1))
                rows = k2.tile([128, TTW], F32, tag="lnrows", name="lnrows", bufs=1)
                srow, s2, varu, lnv = (rows[j:j + 1, :] for j in (0, 32, 64, 96))
                nc.scalar.activation(out=srow, in_=ssum, func=AF.Copy)
                nc.vector.tensor_mul(out=s2, in0=srow, in1=srow)
                nc.vector.scalar_tensor_tensor(out=varu, in0=s2, scalar=-1.0 / C,
                                               in1=ssq, op0=OP.mult, op1=OP.add)
                nc.scalar.activation(out=lnv, in_=varu, func=AF.Ln, scale=1.0 / C,
                                     bias=eps_ln_t[0:1, :])
                rs = k2.tile([1, TTW], F32R, tag="lnrs", name="lnrs", bufs=1)
                nc.scalar.activation(out=rs, in_=lnv, func=AF.Exp, scale=-0.5)
                murs = k2.tile([1, TTW], F32R, tag="lnmu", name="lnmu", bufs=1)
                nc.vector.scalar_tensor_tensor(out=murs, in0=srow, scalar=1.0 / C,
                                               in1=rs, op0=OP.mult, op1=OP.mult)
                bc0 = psB.tile([128, TTW], F32, tag="pw", name="pw")
                nc.tensor.matmul(out=bc0, lhsT=ones_r[0:1, 0:128], rhs=rs,
                                 start=True, stop=True)
                bc1 = psB.tile([128, TTW], F32, tag="pw", name="pw")
                nc.tensor.matmul(out=bc1, lhsT=ones_r[0:1, 0:128], rhs=murs,
                                 start=True, stop=True)
                return bc0, bc1

            def ln_apply(src_sl, bc0, bc1, vt, wi, bi, out_tiles):
                for i in range(CT):
                    t1 = k2.tile([128, TTW], F32, tag="lnt1", name="lnt1")
                    nc.vector.tensor_mul(out=t1, in0=src_sl[i], in1=bc0)
                    t2 = k2.tile([128, TTW], F32, tag="lnt2", name="lnt2")
                    nc.vector.tensor_sub(out=t2, in0=t1, in1=bc1)
                    nc.vector.tensor_scalar(out=out_tiles[i], in0=t2,
                                            scalar1=vcol(vt, i, wi),
                                            scalar2=vcol(vt, i, bi),
                                            op0=OP.mult, op1=OP.add)

            def tanh_route(psum_ap, out_tile):
                nc.scalar.activation(out=out_tile, in_=psum_ap, func=AF.Tanh)

            # ================= layers =================
            for l in range(NL):
                vecs = []
                for i in range(CT):
                    vt = pp.tile([128, NV], F32, tag=f"vecs{i}", name=f"vecs{i}")
                    nc.sync.dma_start(out=vt, in_=vecs_in[l][i * 128:(i + 1) * 128, :])
                    vecs.append(vt)
                lnxt = pp.tile([128, 1024], F32, tag="lnx", name="lnx")
                nc.sync.dma_start(out=lnxt, in_=lnx_in[l][:, :])
                hmu = []
                for i in range(CT):
                    ht = pp.tile([128, 8], BF16, tag=f"hmu{i}", name=f"hmu{i}")
                    nc.sync.dma_start(out=ht, in_=hmu_in[l][i * 128:(i + 1) * 128, :])
                    hmu.append(ht)

                # ---- xb init ----
                if l == 0:
                    for tt in range(NTT):
                        sl = slice(tt * TTW, (tt + 1) * TTW)
                        src = [xres[i][:, sl] for i in range(CT)]
                        bc0, bc1 = ln_stats(src)
                        xbs = [xb[i][:, sl] for i in range(CT)]
                        ln_apply(src, bc0, bc1, vecs, V_LN0W, V_LN0B, xbs)
                        for i in range(CT):
                            d = k2.tile([128, TTW], F32, tag="lnt2", name="lnt2")
                            nc.vector.tensor_sub(out=d, in0=xb[i][:, sl],
                                                 in1=xres[i][:, sl])
                            nc.vector.scalar_tensor_tensor(
                                out=xb[i][:, sl], in0=d,
                                scalar=selt[:, S_LN0:S_LN0 + 1],
                                in1=xres[i][:, sl], op0=OP.mult, op1=OP.add)
                else:
                    for i in range(CT):
                        nc.gpsimd.tensor_copy(out=xb[i], in_=xres[i])

                S_cur = [pp.tile([128, HN], BF16, tag=f"S{i}", name=f"S{i}") for i in range(CT)]
                for i in range(CT):
                    nc.vector.memset(S_cur[i], 0.0)
                carry = [pp.tile([128, 1], F32, tag=f"ca{i}", name=f"ca{i}") for i in range(CT)]
                carry2 = [pp.tile([128, 1], F32, tag=f"cb{i}", name=f"cb{i}") for i in range(CT)]
                for i in range(CT):
                    nc.gpsimd.memset(carry[i], 0.0)
                    nc.gpsimd.memset(carry2[i], 0.0)

                # ================= time mix =================
                for tt in range(NTT):
                    sl = slice(tt * TTW, (tt + 1) * TTW)
                    xbs = [xb[i][:, sl] for i in range(CT)]
                    bc0, bc1 = ln_stats(xbs)
                    xt = [kp.tile([128, TTW], F32, tag=f"xt{i}", name=f"xt{i}") for i in range(CT)]
                    ln_apply(xbs, bc0, bc1, vecs, V_LN1W, V_LN1B, xt)
                    xx = [kp.tile([128, TTW], F32, tag=f"xx{i}", name=f"xx{i}") for i in range(CT)]
                    for i in range(CT):
                        nc.vector.tensor_sub(out=xx[i][:, 1:TTW],
                                             in0=xt[i][:, 0:TTW - 1],
                                             in1=xt[i][:, 1:TTW])
                        nc.vector.scalar_tensor_tensor(
                            out=xx[i][:, 0:1], in0=carry[i], scalar=1.0,
                            in1=xt[i][:, 0:1], op0=OP.mult, op1=OP.subtract)
                        nc.gpsimd.tensor_copy(out=carry[i], in_=xt[i][:, TTW - 1:TTW])
                    if dbg and l == 0:
                        for i in range(CT):
                            nc.sync.dma_start(out=dbg_o["xt0"][i * 128:(i + 1) * 128, sl],
                                              in_=xt[i])

                    # ---- t5 ----
                    mx = [k2.tile([128, TTW], BF16, tag=f"xf{i}", name=f"xf{i}") for i in range(CT)]
                    for i in range(CT):
                        nc.vector.scalar_tensor_tensor(
                            out=mx[i], in0=xx[i], scalar=vcol(vecs, i, V_MAAX),
                            in1=xt[i], op0=OP.mult, op1=OP.add)
                    p160a = psC.tile([128, TTW], F32, tag="stA", name="stA")
                    p160b = psC.tile([32, TTW], F32, tag="stB", name="stB")
                    wtmtd = wp.tile([128, CT, 224], BF16, tag="wtmtd", name="wtmtd",
                                    bufs=1)
                    nc.sync.dma_start(
                        out=wtmtd,
                        in_=wbig[l][:, WB_TM1:WB_TM1 + 224]
                        .rearrange("(k p) n -> p k n", p=128))
                    for i in range(CT):
                        nc.tensor.matmul(out=p160a, lhsT=wtmtd[:, i, 0:128], rhs=mx[i],
                                         start=(i == 0), stop=(i == CT - 1))
                        nc.tensor.matmul(out=p160b, lhsT=wtmtd[:, i, 128:160], rhs=mx[i],
                                         start=(i == 0), stop=(i == CT - 1))
                    t5 = [k2.tile([32, TTW], BF16, tag=f"t5{f}", name=f"t5{f}", bufs=1) for f in range(5)]
                    for f in range(4):
                        tanh_route(p160a[f * 32:(f + 1) * 32, :], t5[f])
                    tanh_route(p160b, t5[4])

                    def build_xf(fidx, maa_i):
                        w2 = load_w(tmw2[l][fidx * TM:(fidx + 1) * TM, :],
                                    [TM, C], "wtm2")
                        xft = []
                        for i in range(CT):
                            dlp = psB.tile([128, TTW], F32, tag="pw", name="pw")
                            nc.tensor.matmul(out=dlp,
                                             lhsT=w2[:, i * 128:(i + 1) * 128],
                                             rhs=t5[fidx], start=True, stop=True)
                            a = k2.tile([128, TTW], F32, tag="lnt1", name="lnt1")
                            nc.vector.scalar_tensor_tensor(
                                out=a, in0=dlp, scalar=vcol(vecs, i, maa_i),
                                in1=xx[i], op0=OP.add, op1=OP.mult)
                            xf = k2.tile([128, TTW], BF16, tag=f"xf{i}", name=f"xf{i}")
                            nc.vector.tensor_add(out=xf, in0=a, in1=xt[i])
                            xft.append(xf)
                        return xft

                    def wmm(col_off, xft):
                        accs = [psA.tile([128, TTW], F32, tag=f"acc{m}",
                                         name=f"acc{m}") for m in range(4)]
                        wt = load_wblk(wbig[l][:, col_off:col_off + 512],
                                       [128, CT, 512])
                        for i in range(CT):
                            for m in range(4):
                                nc.tensor.matmul(out=accs[m],
                                                 lhsT=wt[:, i, m * 128:(m + 1) * 128],
                                                 rhs=xft[i], start=(i == 0),
                                                 stop=(i == CT - 1))
                        return accs

                    def wmm_tm(col_off, xft):
                        accs = [psA.tile([128, TTW], F32, tag=f"acc{m}",
                                         name=f"acc{m}") for m in range(4)]
                        wt = load_wblk(wbig[l][:, col_off:col_off + 512],
                                       [128, CT, 512])
                        for i in range(CT):
                            for ci in range(NCPT):
                                nc.tensor.matmul(out=accs[ci],
                                                 lhsT=xft[i][:, ci * L:(ci + 1) * L],
                                                 rhs=wt[:, i, :], start=(i == 0),
                                                 stop=(i == CT - 1))
                        return accs

                    # k
                    xf = build_xf(1, V_MAAK)
                    accs = wmm(WB_K, xf)
                    k_sb = [kp.tile([128, TTW], F32, tag=f"ksb{i}", name=f"ksb{i}") for i in range(CT)]
                    for m in range(4):
                        nc.scalar.activation(out=k_sb[m], in_=accs[m], func=AF.Copy)
                    # v token-major
                    xf = build_xf(2, V_MAAV)
                    accs = wmm_tm(WB_V, xf)
                    v_tm = [kp.tile([128, C], F32, tag=f"vtm{ci}", name=f"vtm{ci}") for ci in range(NCPT)]
                    v_tmb = [kp.tile([128, C], BF16, tag=f"vtb{ci}", name=f"vtb{ci}") for ci in range(NCPT)]
                    for ci in range(NCPT):
                        nc.scalar.activation(out=v_tm[ci], in_=accs[ci], func=AF.Copy)
                        nc.scalar.activation(out=v_tmb[ci], in_=accs[ci], func=AF.Copy)
                    # r
                    xf = build_xf(3, V_MAAR)
                    accs = wmm(WB_R, xf)
                    r_sb = [kp.tile([128, TTW], F32, tag=f"rsb{i}", name=f"rsb{i}") for i in range(CT)]
                    for m in range(4):
                        nc.scalar.activation(out=r_sb[m], in_=accs[m], func=AF.Copy)
                    if dbg and l == 0:
                        for i in range(CT):
                            nc.sync.dma_start(out=dbg_o["r0"][i * 128:(i + 1) * 128, sl],
                                              in_=r_sb[i])
                    # g token-major, silu
                    xf = build_xf(4, V_MAAG)
                    accs = wmm_tm(WB_G, xf)
                    g_tm = [kp.tile([128, C], F32, tag=f"gtm{ci}", name=f"gtm{ci}") for ci in range(NCPT)]
                    for ci in range(NCPT):
                        nc.scalar.activation(out=g_tm[ci], in_=accs[ci], func=AF.Silu)
                    # w -> wacc -> lai
                    xf = build_xf(0, V_MAAW)
                    tdp = psC.tile([TD, TTW], F32, tag="stA", name="stA")
                    for i in range(CT):
                        nc.tensor.matmul(out=tdp, lhsT=wtmtd[:, i, 160:224], rhs=xf[i],
                                         start=(i == 0), stop=(i == CT - 1))
                    tdt = k2.tile([TD, TTW], BF16, tag="tdt", name="tdt", bufs=1)
                    tanh_route(tdp, tdt)
                    w2t = load_w(tdw2[l][:, :], [TD, C], "wtd2", bufs=1)
                    lai = [kp.tile([128, 1 + TTW], F32, tag=f"lai{i}", name=f"lai{i}") for i in range(CT)]
                    for i in range(CT):
                        wwp = psB.tile([128, TTW], F32, tag="pw", name="pw")
                        nc.tensor.matmul(out=wwp, lhsT=w2t[:, i * 128:(i + 1) * 128],
                                         rhs=tdt, start=True, stop=True)
                        wacc = k2.tile([128, TTW], F32, tag="lnt1", name="lnt1")
                        nc.scalar.activation(out=wacc, in_=wwp, func=AF.Exp,
                                             bias=vcol(vecs, i, V_TDCY))
                        nc.gpsimd.memset(lai[i][:, 0:1], 0.0)
                        nc.vector.tensor_tensor_scan(
                            out=lai[i][:, 1:1 + TTW], data0=wacc, data1=wacc,
                            initial=0.0, op0=OP.add, op1=OP.bypass)
                        if dbg and l == 0:
                            nc.sync.dma_start(
                                out=dbg_o["w0"][i * 128:(i + 1) * 128, sl], in_=wacc)
                            nc.sync.dma_start(
                                out=dbg_o["lai0"][i * 128:(i + 1) * 128, sl],
                                in_=lai[i][:, 1:1 + TTW])

                    # ---- wkv chunks ----
                    ztc = [kp.tile([128, TTW], BF16, tag=f"ztc{i}", name=f"ztc{i}") for i in range(CT)]
                    for ci in range(NCPT):
                        gc = tt * NCPT + ci
                        c0 = ci * L
                        fpc, fnc, rt_t, kt_t, kh_tm, m_t = [], [], [], [], [], []
                        for i in range(CT):
                            ngc = k2.tile([128, 1], F32, tag="ngc", name="ngc", bufs=4)
                            nc.vector.tensor_scalar_mul(out=ngc,
                                                        in0=lai[i][:, c0:c0 + 1],
                                                        scalar1=-1.0)
                            fp = k2.tile([128, 1 + L], F32, tag="fp", name="fp", bufs=4)
                            nc.scalar.activation(out=fp, in_=lai[i][:, c0:c0 + 1 + L],
                                                 func=AF.Exp, bias=ngc)
                            fn = k2.tile([128, 1 + L], F32, tag="fn", name="fn", bufs=4)
                            nc.scalar.activation(out=fn, in_=lai[i][:, c0:c0 + 1 + L],
                                                 func=AF.Exp, scale=-1.0,
                                                 bias=lai[i][:, c0:c0 + 1])
                            fpc.append(fp)
                            fnc.append(fn)
                            rt = k2.tile([128, L], BF16, tag="rt", name="rt", bufs=4)
                            nc.vector.tensor_mul(out=rt, in0=r_sb[i][:, c0:c0 + L],
                                                 in1=fn[:, 0:L])
                            kt = k2.tile([128, L], BF16, tag="kt", name="kt", bufs=4)
                            nc.vector.tensor_mul(out=kt, in0=k_sb[i][:, c0:c0 + L],
                                                 in1=fp[:, 1:1 + L])
                            kh = k2.tile([128, L], F32, tag="kh", name="kh", bufs=4)
                            nc.vector.tensor_scalar_mul(out=kh, in0=kt,
                                                        scalar1=fn[:, L:L + 1])
                            mt = k2.tile([128, L], BF16, tag="mt", name="mt", bufs=4)
                            nc.gpsimd.tensor_mul(out=mt, in0=r_sb[i][:, c0:c0 + L],
                                                 in1=k_sb[i][:, c0:c0 + L])
                            rt_t.append(rt)
                            kt_t.append(kt)
                            m_t.append(mt)
                            trp = psB.tile([128, L], F32, tag="pw", name="pw")
                            nc.tensor.transpose(out=trp, in_=kh, identity=ident)
                            kht = k2.tile([128, L], BF16, tag="khtm", name="khtm", bufs=4)
                            nc.scalar.activation(out=kht, in_=trp, func=AF.Copy)
                            kh_tm.append(kht)
                        dall = psC.tile([128, 8], F32, tag="stB", name="stB")
                        for i in range(CT):
                            nc.tensor.matmul(out=dall, lhsT=m_t[i], rhs=hmu[i],
                                             start=(i == 0), stop=(i == CT - 1))
                        yps = psA.tile([128, C], F32, tag="acc0", name="acc0")
                        S_new = [k2.tile([128, HN], BF16, tag=f"Sn{i}", name=f"Sn{i}") for i in range(CT)]
                        for i in range(CT):
                            sup = psC.tile([128, HN], F32, tag="stA", name="stA")
                            for hh in range(2):
                                h = 2 * i + hh
                                hb = hh * HN
                                pt = psB.tile([L, L], F32, tag="pw", name="pw")
                                nc.tensor.matmul(out=pt, lhsT=kt_t[i][hb:hb + HN, :],
                                                 rhs=rt_t[i][hb:hb + HN, :],
                                                 start=True, stop=True)
                                pts = k2.tile([L, L], BF16, tag="pts", name="pts")
                                nc.vector.tensor_mul(out=pts, in0=pt, in1=triu)
                                nc.tensor.matmul(
                                    out=yps[:, h * HN:(h + 1) * HN], lhsT=pts,
                                    rhs=v_tmb[ci][:, h * HN:(h + 1) * HN],
                                    start=True, stop=(gc == 0), skip_group_check=True)
                                if gc > 0:
                                    nc.tensor.matmul(
                                        out=yps[:, h * HN:(h + 1) * HN],
                                        lhsT=rt_t[i][hb:hb + HN, :],
                                        rhs=S_cur[i][hb:hb + HN, :],
                                        start=False, stop=True, skip_group_check=True)
                                nc.tensor.matmul(
                                    out=sup[hb:hb + HN, :],
                                    lhsT=kh_tm[i][:, hb:hb + HN],
                                    rhs=v_tmb[ci][:, h * HN:(h + 1) * HN],
                                    start=True, stop=True, skip_group_check=True)
                            t0 = k2.tile([128, HN], F32, tag="ssc", name="ssc", bufs=4)
                            nc.vector.tensor_scalar_mul(out=t0, in0=S_cur[i],
                                                        scalar1=fnc[i][:, L:L + 1])
                            nc.vector.tensor_add(out=S_new[i], in0=t0, in1=sup)
                        S_cur = S_new
                        ysb = k2.tile([128, C], F32, tag="ysb", name="ysb")
                        for h in range(H):
                            nc.vector.scalar_tensor_tensor(
                                out=ysb[:, h * HN:(h + 1) * HN],
                                in0=v_tm[ci][:, h * HN:(h + 1) * HN],
                                scalar=dall[:, h:h + 1],
                                in1=yps[:, h * HN:(h + 1) * HN],
                                op0=OP.mult, op1=OP.add)
                        if dbg and l == 0:
                            nc.sync.dma_start(
                                out=dbg_o["y0tm"][tt * TTW + c0:tt * TTW + c0 + L, :],
                                in_=ysb)
                            nc.sync.dma_start(
                                out=dbg_o["g0tm"][tt * TTW + c0:tt * TTW + c0 + L, :],
                                in_=g_tm[ci])
                        # groupnorm + affine + *g  (token-major)
                        mv = k2.tile([128, 16], F32, tag="gnmv", name="gnmv")
                        for h in range(H):
                            st = k2.tile([128, 6], F32, tag="gnst", name="gnst")
                            nc.vector.bn_stats(out=st, in_=ysb[:, h * HN:(h + 1) * HN])
                            nc.vector.bn_aggr(out=mv[:, 2 * h:2 * h + 2], in_=st)
                        lnv = k2.tile([128, 8], F32, tag="gnln", name="gnln")
                        var_view = bass.AP(tensor=mv.tensor, offset=mv.offset + 1,
                                           ap=[mv.ap[0], [2, 8]])
                        nc.scalar.activation(out=lnv, in_=var_view, func=AF.Ln,
                                             bias=eps_gn_t)
                        rsg = k2.tile([128, 8], F32, tag="gnrs", name="gnrs")
                        nc.scalar.activation(out=rsg, in_=lnv, func=AF.Exp, scale=-0.5)
                        for h in range(H):
                            nc.vector.tensor_scalar(
                                out=ysb[:, h * HN:(h + 1) * HN],
                                in0=ysb[:, h * HN:(h + 1) * HN],
                                scalar1=mv[:, 2 * h:2 * h + 1],
                                scalar2=rsg[:, h:h + 1],
                                op0=OP.subtract, op1=OP.mult)
                        nc.gpsimd.tensor_mul(out=ysb, in0=ysb, in1=lnxt[:, 0:512])
                        nc.gpsimd.tensor_add(out=ysb, in0=ysb, in1=lnxt[:, 512:1024])
                        nc.vector.tensor_mul(out=ysb, in0=ysb, in1=g_tm[ci])
                        for i in range(CT):
                            trp = psB.tile([128, L], F32, tag="pw", name="pw")
                            nc.tensor.transpose(out=trp,
                                                in_=ysb[:, i * 128:(i + 1) * 128],
                                                identity=ident)
                            nc.scalar.activation(out=ztc[i][:, c0:c0 + L], in_=trp,
                                                 func=AF.Copy)
                    # ---- Wo ----
                    accs = [psA.tile([128, TTW], F32, tag=f"acc{m}",
                                     name=f"acc{m}") for m in range(4)]
                    wt = load_wblk(wbig[l][:, WB_O:WB_O + 512], [128, CT, 512])
                    for i in range(CT):
                        for m in range(4):
                            nc.tensor.matmul(out=accs[m],
                                             lhsT=wt[:, i, m * 128:(m + 1) * 128],
                                             rhs=ztc[i], start=(i == 0),
                                             stop=(i == CT - 1))
                    for m in range(4):
                        nc.vector.tensor_add(out=xb[m][:, sl], in0=xb[m][:, sl],
                                             in1=accs[m])
                    if dbg and l == 0:
                        for i in range(CT):
                            nc.sync.dma_start(
                                out=dbg_o["xbtm0"][i * 128:(i + 1) * 128, sl],
                                in_=xb[i][:, sl].bitcast(F32))

                # ================= channel mix =================
                for tt in range(NTT):
                    sl = slice(tt * TTW, (tt + 1) * TTW)
                    xbs = [xb[i][:, sl] for i in range(CT)]
                    bc0, bc1 = ln_stats(xbs)
                    xc = [kp.tile([128, TTW], F32, tag=f"xt{i}", name=f"xt{i}") for i in range(CT)]
                    ln_apply(xbs, bc0, bc1, vecs, V_LN2W, V_LN2B, xc)
                    xx2 = [kp.tile([128, TTW], F32, tag=f"xx{i}", name=f"xx{i}") for i in range(CT)]
                    for i in range(CT):
                        nc.vector.tensor_sub(out=xx2[i][:, 1:TTW],
                                             in0=xc[i][:, 0:TTW - 1],
                                             in1=xc[i][:, 1:TTW])
                        nc.vector.scalar_tensor_tensor(
                            out=xx2[i][:, 0:1], in0=carry2[i], scalar=1.0,
                            in1=xc[i][:, 0:1], op0=OP.mult, op1=OP.subtract)
                        nc.gpsimd.tensor_copy(out=carry2[i], in_=xc[i][:, TTW - 1:TTW])
                    xk2 = [kp.tile([128, TTW], BF16, tag=f"xk2{i}", name=f"xk2{i}") for i in range(CT)]
                    xr2 = [kp.tile([128, TTW], BF16, tag=f"xr2{i}", name=f"xr2{i}") for i in range(CT)]
                    for i in range(CT):
                        nc.vector.scalar_tensor_tensor(
                            out=xk2[i], in0=xx2[i], scalar=vcol(vecs, i, V_CMK),
                            in1=xc[i], op0=OP.mult, op1=OP.add)
                        nc.vector.scalar_tensor_tensor(
                            out=xr2[i], in0=xx2[i], scalar=vcol(vecs, i, V_CMR),
                            in1=xc[i], op0=OP.mult, op1=OP.add)
                    # cm_Wr -> sigmoid
                    accs = [psA.tile([128, TTW], F32, tag=f"acc{m}", name=f"acc{m}") for m in range(4)]
                    wt = load_wblk(cmrg[l][:, 0:512], [128, CT, 512])
                    for i in range(CT):
                        for m in range(4):
                            nc.tensor.matmul(out=accs[m],
                                             lhsT=wt[:, i, m * 128:(m + 1) * 128],
                                             rhs=xr2[i], start=(i == 0),
                                             stop=(i == CT - 1))
                    sig = [kp.tile([128, TTW], F32, tag=f"gtm{m}", name=f"sig{m}") for m in range(4)]
                    for m in range(4):
                        nc.scalar.activation(out=sig[m], in_=accs[m], func=AF.Sigmoid)
                    # kk loop with cm_Wv accumulation
                    accs = [psA.tile([128, TTW], F32, tag=f"acc{m}", name=f"acc{m}") for m in range(4)]
                    wfq = wvq = None
                    for f in range(NFF):
                        fq, fr2 = f // 4, f % 4
                        nq = min(4, NFF - 4 * fq)
                        if fr2 == 0:
                            wfq = wp.tile([128, nq, CT, 128], BF16, tag="wblk",
                                          name="wfq", bufs=2)
                            nc.sync.dma_start(
                                out=wfq,
                                in_=cmkp[l][4 * fq:4 * fq + nq]
                                .rearrange("f (k p) n -> p f k n", p=128))
                            wvq = wp.tile([128, nq, C], BF16, tag="wblk",
                                          name="wvq", bufs=2)
                            nc.sync.dma_start(
                                out=wvq,
                                in_=cmvp[l][4 * fq * 128:(4 * fq + nq) * 128, :]
                                .rearrange("(f p) n -> p f n", p=128))
                        kkp = psB.tile([128, TTW], F32, tag="pw", name="pw")
                        for i in range(CT):
                            nc.tensor.matmul(out=kkp, lhsT=wfq[:, fr2, i, :],
                                             rhs=xk2[i],
                                             start=(i == 0), stop=(i == CT - 1))
                        rl = k2.tile([128, TTW], F32, tag="lnt1", name="lnt1")
                        nc.scalar.activation(out=rl, in_=kkp, func=AF.Relu)
                        kkf = k2.tile([128, TTW], BF16, tag="lnt2k", name="lnt2k")
                        nc.scalar.activation(out=kkf, in_=rl, func=AF.Square)
                        for m in range(4):
                            nc.tensor.matmul(out=accs[m],
                                             lhsT=wvq[:, fr2, m * 128:(m + 1) * 128],
                                             rhs=kkf, start=(f == 0),
                                             stop=(f == NFF - 1))
                    for m in range(4):
                        nc.vector.tensor_mul(out=sig[m], in0=sig[m], in1=accs[m])
                        nc.vector.tensor_add(out=xb[m][:, sl], in0=xb[m][:, sl],
                                             in1=sig[m])
                if dbg and l == 0:
                    for i in range(CT):
                        nc.sync.dma_start(out=dbg_o["p0"][i * 128:(i + 1) * 128, :],
                                          in_=xb[i].bitcast(F32))

                # ================= exchange + join =================
                send = dp.tile([C, T], F32, tag="send", name="send")
                recv = dp.tile([2 * C, T], F32, tag="recv", name="recv")
                for i in range(CT):
                    # time-reverse in SBUF (DVE handles the negative stride),
                    # then one contiguous DMA — a reversed DRAM AP would shatter
                    # into 128*1024 4-byte descriptors and stall every queue.
                    rv = k2.tile([128, T], F32, tag="revst", name="revst")
                    nc.vector.tensor_copy(out=rv, in_=_revap(xb[i][:, :].bitcast(F32)))
                    nc.sync.dma_start(out=send[i * 128:(i + 1) * 128, :], in_=rv)
                if solo:
                    for i in range(CT):
                        nc.sync.dma_start(out=recv[i * 128:(i + 1) * 128, :],
                                          in_=xb[i].bitcast(F32))
                        nc.sync.dma_start(out=recv[C + i * 128:C + (i + 1) * 128, :],
                                          in_=xb[i].bitcast(F32))
                else:
                    nc.gpsimd.collective_compute(
                        "AllGather", OP.bypass, replica_groups=groups,
                        ins=[send.opt()], outs=[recv.opt()])
                for tt in range(NTT):
                    sl = slice(tt * TTW, (tt + 1) * TTW)
                    # conv residue, all ct (fp32 accumulation, bf16 final for matmul)
                    cv32 = [kp.tile([128, TTW], F32, tag=f"ksb{i}", name=f"ksb{i}") for i in range(CT)]
                    cv = [k2.tile([128, TTW], BF16, tag=f"cvb{i}", name=f"cvb{i}") for i in range(CT)]
                    a = tt * TTW
                    for i in range(CT):
                        nc.vector.tensor_scalar_mul(out=cv32[i], in0=xres[i][:, sl],
                                                    scalar1=vcol(vecs, i, V_CW1))
                        lo = 1 if tt == 0 else 0
                        nc.vector.scalar_tensor_tensor(
                            out=cv32[i][:, lo:TTW],
                            in0=xres[i][:, a + lo - 1:a + TTW - 1],
                            scalar=vcol(vecs, i, V_CW0),
                            in1=cv32[i][:, lo:TTW], op0=OP.mult, op1=OP.add)
                        hi = TTW - 1 if tt == NTT - 1 else TTW
                        nc.vector.scalar_tensor_tensor(
                            out=cv[i][:, 0:hi],
                            in0=xres[i][:, a + 1:a + hi + 1],
                            scalar=vcol(vecs, i, V_CW2),
                            in1=cv32[i][:, 0:hi], op0=OP.mult, op1=OP.add)
                        if hi < TTW:
                            nc.scalar.activation(out=cv[i][:, hi:TTW],
                                                 in_=cv32[i][:, hi:TTW], func=AF.Copy)
                    accs = [psA.tile([128, TTW], F32, tag=f"acc{m}", name=f"acc{m}") for m in range(4)]
                    wt = load_wblk(cmrg[l][:, 512:1024], [128, CT, 512])
                    for i in range(CT):
                        for m in range(4):
                            nc.tensor.matmul(out=accs[m],
                                             lhsT=wt[:, i, m * 128:(m + 1) * 128],
                                             rhs=cv[i], start=(i == 0),
                                             stop=(i == CT - 1))
                    for m in range(4):
                        jr0 = kp.tile([128, TTW], F32, tag="vtm0", name="vtm0")
                        jr1 = kp.tile([128, TTW], F32, tag="vtm1", name="vtm1")
                        nc.sync.dma_start(out=jr0, in_=recv[m * 128:(m + 1) * 128, sl])
                        nc.sync.dma_start(out=jr1,
                                          in_=recv[C + m * 128:C + (m + 1) * 128, sl])
                        nc.vector.tensor_scalar_mul(
                            out=jr0, in0=jr0, scalar1=selt[:, S_ALPHA:S_ALPHA + 1])
                        nc.vector.scalar_tensor_tensor(
                            out=jr1, in0=jr1, scalar=selt[:, S_BETA:S_BETA + 1],
                            in1=jr0, op0=OP.mult, op1=OP.add)
                        sg = k2.tile([128, TTW], F32, tag="er0", name="er0", bufs=2)
                        nc.scalar.activation(out=sg, in_=accs[m], func=AF.Sigmoid,
                                             scale=selt[:, S_NEGS:S_NEGS + 1],
                                             bias=vcol(vecs, m, V_GBM))  # w_recv
                        wown = kp.tile([128, TTW], F32, tag="vtm3", name="vtm3")
                        nc.vector.tensor_scalar(out=wown, in0=sg, scalar1=-1.0,
                                                scalar2=1.0, op0=OP.mult, op1=OP.add)
                        jsum = kp.tile([128, TTW], F32, tag="vtm2", name="vtm2")
                        nc.vector.tensor_mul(out=jsum, in0=wown, in1=xb[m][:, sl])
                        nc.vector.tensor_mul(out=jr1, in0=sg, in1=jr1)
                        nc.vector.tensor_add(out=jsum, in0=jsum, in1=jr1)
                        # note: host mask05 already includes the 2x factor fold:
                        # mask05 = mask (not 0.5*mask) since sigmoid form used.
                        nc.vector.tensor_mul(out=xres[m][:, sl], in0=jsum,
                                             in1=maskt[:, sl])
            # ---- output ----
            for i in range(CT):
                nc.sync.dma_start(out=xout[i * 128:(i + 1) * 128, :],
                                  in_=xres[i].bitcast(F32))
    nc.compile()
    return nc


def _host_inputs(inputs):
    x = np.asarray(inputs["x"], np.float32)
    lengths = np.asarray(inputs["lengths"]).astype(np.int64)
    pos = np.arange(T, dtype=np.float32)[:, None]
    div = np.exp(np.arange(0, C, 2, dtype=np.float32) * (-np.log(10000.0) / C))
    pe = np.zeros((T, C), np.float32)
    pe[:, 0::2] = np.sin(pos * div)
    pe[:, 1::2] = np.cos(pos * div)
    mask = (np.arange(T)[None, :] < lengths[:, None]).astype(np.float32)

    consts = np.zeros((128, 384), np.float32)
    consts[:, 0:128] = np.eye(128, dtype=np.float32)
    consts[:, 128:256] = np.triu(np.ones((128, 128), np.float32), 1)
    consts[:, 256:384] = 1.0

    gw = np.asarray(inputs["gate_w"], np.float32)
    gb = np.asarray(inputs["gate_b"], np.float32)
    cw = np.asarray(inputs["conv_w"], np.float32)
    cb = np.asarray(inputs["conv_b"], np.float32)

    in_maps = []
    for c in range(8):
        b, d = c % 4, c // 4
        rev = d == 1
        s = -1.0 if rev else 1.0
        xin = (x[b] + pe)
        mrow = mask[b]
        if rev:
            xin = xin[::-1]
            mrow = mrow[::-1]
        m = {
            "x0": np.ascontiguousarray(xin.T),
            "mask05": np.ascontiguousarray(np.broadcast_to(mrow, (128, T))),
            "consts": consts,
        }
        sel = np.zeros((128, 8), np.float32)
        sel[:, S_LN0] = 0.0 if rev else 1.0
        sel[:, S_NEGS] = -s
        sel[:, S_ALPHA] = 1.0 if rev else 0.0
        sel[:, S_BETA] = 0.0 if rev else 1.0
        m["sel"] = sel
        for l in range(NL):
            W = {k: np.asarray(inputs[k], np.float32)[d, l]
                 for k in ["ln1_w", "ln1_b", "ln2_w", "ln2_b", "maa_x", "maa_w",
                           "maa_k", "maa_v", "maa_r", "maa_g", "tm_w1", "tm_w2",
                           "td_w1", "td_w2", "time_decay", "Wr", "Wk", "Wv",
                           "Wg", "Wo", "lnx_w", "lnx_b", "cm_maa_k", "cm_maa_r",
                           "cm_Wk", "cm_Wv", "cm_Wr", "time_faaaa"]}
            import ml_dtypes
            bf16 = ml_dtypes.bfloat16
            m[f"wbig{l}"] = np.ascontiguousarray(np.concatenate(
                [W["Wr"], W["Wk"], W["Wv"], W["Wg"], W["tm_w1"], W["td_w1"],
                 W["Wo"]], axis=1).astype(bf16))
            m[f"tmw2{l}"] = np.ascontiguousarray(
                W["tm_w2"].reshape(5 * TM, C).astype(bf16))
            m[f"tdw2{l}"] = np.ascontiguousarray(W["td_w2"].astype(bf16))
            m[f"cmk{l}"] = np.ascontiguousarray(
                W["cm_Wk"].reshape(C, NFF, 128).transpose(1, 0, 2).astype(bf16))
            m[f"cmv{l}"] = np.ascontiguousarray(W["cm_Wv"].astype(bf16))
            m[f"cmrg{l}"] = np.ascontiguousarray(
                np.concatenate([W["cm_Wr"], gw[l]], axis=1).astype(bf16))
            cwe = cw[l] if not rev else cw[l][:, ::-1]
            gbe = cb[l] @ gw[l] + gb[l]
            vec = np.zeros((C, NV), np.float32)
            vec[:, V_LN1W] = W["ln1_w"]; vec[:, V_LN1B] = W["ln1_b"]
            vec[:, V_LN2W] = W["ln2_w"]; vec[:, V_LN2B] = W["ln2_b"]
            vec[:, V_MAAX] = W["maa_x"]; vec[:, V_MAAW] = W["maa_w"]
            vec[:, V_MAAK] = W["maa_k"]; vec[:, V_MAAV] = W["maa_v"]
            vec[:, V_MAAR] = W["maa_r"]; vec[:, V_MAAG] = W["maa_g"]
            vec[:, V_TDCY] = W["time_decay"]
            vec[:, V_CMK] = W["cm_maa_k"]; vec[:, V_CMR] = W["cm_maa_r"]
            vec[:, V_GBM] = -s * gbe
            vec[:, V_CW0] = cwe[:, 0]
            vec[:, V_CW1] = cwe[:, 1] - 1.0
            vec[:, V_CW2] = cwe[:, 2]
            vec[:, V_LN0W] = np.asarray(inputs["ln0_w"], np.float32)
            vec[:, V_LN0B] = np.asarray(inputs["ln0_b"], np.float32)
            m[f"vecs{l}"] = vec
            lnx = np.zeros((128, 1024), np.float32)
            lnx[:, 0:512] = W["lnx_w"][None, :]
            lnx[:, 512:1024] = W["lnx_b"][None, :]
            m[f"lnx{l}"] = lnx
            u = W["time_faaaa"].reshape(C)
            hmu = np.zeros((C, 8), np.float32)
            for h in range(H):
                hmu[h * HN:(h + 1) * HN, h] = u[h * HN:(h + 1) * HN]
            m[f"hmu{l}"] = hmu.astype(bf16)
        in_maps.append(m)
    return in_maps


def kernel(**inputs):
    if "nc" not in _CACHE:
        _CACHE["nc"] = _build(dbg=False)
    nc = _CACHE["nc"]
    in_maps = _host_inputs(inputs)
    res = run_bass_kernel_spmd(nc, in_maps, list(range(8)))
    out = np.empty((B, T, C), np.float32)
    for b in range(B):
        out[b] = res.results[b]["xout"].T
    return out


if __name__ == "__main__":
    rng = np.random.default_rng(0)
    demo = None



# revision 36
# speedup vs baseline: 11.5836x; 1.0511x over previous
"""BiWKV6 encoder kernel for 8 Trainium2 NeuronCores.

Sharding: (batch, direction) -> 8 units, one per core; core c handles
batch c % 4, direction c // 4. Backward cores run the identical SPMD
program on time-reversed inputs; the only cross-core communication is a
pairwise AllGather of each layer's block output, written time-reversed
into the partner's domain. Within a core activations are channel-major
[C, T]; the WKV scan uses the chunked linear-attention formulation
(chunk 128) with log-space cumulative decay from the DVE prefix scan.
"""
import numpy as np

import concourse.bass as bass
import concourse.tile as tile
from concourse import bacc, mybir
from concourse.bass_utils import run_bass_kernel_spmd

B, T, C = 4, 1024, 512
H, HN = 8, 64
L = 128
TTW = 512
NTT = T // TTW
NCPT = TTW // L
CT = C // 128
TM, TD, FFN, NL = 32, 64, 1792, 2
NFF = FFN // 128
EPS_LN, EPS_GN = 1e-5, 64e-5
NV = 19

F32 = mybir.dt.float32
F32R = mybir.dt.float32r
BF16 = mybir.dt.bfloat16
AF = mybir.ActivationFunctionType
OP = mybir.AluOpType

WB_R, WB_K, WB_V, WB_G, WB_TM1, WB_TD1, WB_O = 0, 512, 1024, 1536, 2048, 2208, 2272
WB_COLS = 2784
(V_LN1W, V_LN1B, V_LN2W, V_LN2B, V_MAAX, V_MAAW, V_MAAK, V_MAAV, V_MAAR,
 V_MAAG, V_TDCY, V_CMK, V_CMR, V_GBM, V_CW0, V_CW1, V_CW2, V_LN0W,
 V_LN0B) = range(NV)
S_LN0, S_NEGS, S_ALPHA, S_BETA = 0, 1, 2, 3

_CACHE = {}


def _revap(ap):
    n = ap.ap[-1][1]
    return bass.AP(tensor=ap.tensor, offset=ap.offset + (n - 1) * ap.ap[-1][0],
                   ap=[ap.ap[0], [-ap.ap[-1][0], n]])


def _build(dbg=False, solo=False):
    nc = bacc.Bacc("TRN2", target_bir_lowering=False, debug=False, num_devices=8)

    x0 = nc.declare_dram_parameter("x0", [C, T], F32, isOutput=False)
    mask05 = nc.declare_dram_parameter("mask05", [128, T], F32, isOutput=False)
    sel_in = nc.declare_dram_parameter("sel", [128, 8], F32, isOutput=False)
    consts = nc.declare_dram_parameter("consts", [128, 384], F32, isOutput=False)
    wbig, tmw2, tdw2, cmkp, cmvp, cmrg, vecs_in, lnx_in, hmu_in = \
        [], [], [], [], [], [], [], [], []
    for l in range(NL):
        wbig.append(nc.declare_dram_parameter(f"wbig{l}", [C, WB_COLS], BF16, isOutput=False))
        tmw2.append(nc.declare_dram_parameter(f"tmw2{l}", [5 * TM, C], BF16, isOutput=False))
        tdw2.append(nc.declare_dram_parameter(f"tdw2{l}", [TD, C], BF16, isOutput=False))
        cmkp.append(nc.declare_dram_parameter(f"cmk{l}", [NFF, C, 128], BF16, isOutput=False))
        cmvp.append(nc.declare_dram_parameter(f"cmv{l}", [FFN, C], BF16, isOutput=False))
        cmrg.append(nc.declare_dram_parameter(f"cmrg{l}", [C, 1024], BF16, isOutput=False))
        vecs_in.append(nc.declare_dram_parameter(f"vecs{l}", [C, NV], F32, isOutput=False))
        lnx_in.append(nc.declare_dram_parameter(f"lnx{l}", [128, 1024], F32, isOutput=False))
        hmu_in.append(nc.declare_dram_parameter(f"hmu{l}", [C, 8], BF16, isOutput=False))
    xout = nc.declare_dram_parameter("xout", [C, T], F32, isOutput=True)
    dbg_o = {}
    if dbg:
        for nm in ["xt0", "r0", "lai0", "xbtm0", "p0", "w0"]:
            dbg_o[nm] = nc.declare_dram_parameter(nm, [C, T], F32, isOutput=True)
        for nm in ["y0tm", "g0tm"]:
            dbg_o[nm] = nc.declare_dram_parameter(nm, [T, C], F32, isOutput=True)

    groups = [[0, 4], [1, 5], [2, 6], [3, 7]]

    with tile.TileContext(nc) as tc:
        with (
            tc.tile_pool(name="pp", bufs=1) as pp,
            tc.tile_pool(name="wp", bufs=2) as wp,
            tc.tile_pool(name="kp", bufs=1) as kp,
            tc.tile_pool(name="k2", bufs=2) as k2,
            tc.tile_pool(name="psA", bufs=1, space="PSUM") as psA,
            tc.tile_pool(name="psB", bufs=2, space="PSUM") as psB,
            tc.tile_pool(name="psC", bufs=1, space="PSUM") as psC,
            tc.tile_pool(name="dp", bufs=2, space="DRAM") as dp,
        ):
            # ------------- persistent loads -------------
            xres = [pp.tile([128, T], F32R, tag=f"xres{i}", name=f"xres{i}") for i in range(CT)]
            xb = [pp.tile([128, T], F32R, tag=f"xb{i}", name=f"xb{i}") for i in range(CT)]
            for i in range(CT):
                nc.sync.dma_start(out=xres[i],
                                  in_=x0[i * 128:(i + 1) * 128, :].bitcast(F32R))
            maskt = pp.tile([128, T], F32, tag="mask", name="mask")
            nc.sync.dma_start(out=maskt, in_=mask05[:, :])
            selt = pp.tile([128, 8], F32, tag="sel", name="sel")
            nc.sync.dma_start(out=selt, in_=sel_in[:, :])
            cst = pp.tile([128, 384], F32, tag="consts", name="consts")
            nc.sync.dma_start(out=cst, in_=consts[:, :])
            eps_ln_t = pp.tile([128, 1], F32, tag="epsln", name="epsln")
            nc.vector.memset(eps_ln_t, EPS_LN)
            eps_gn_t = pp.tile([128, 1], F32, tag="epsgn", name="epsgn")
            nc.vector.memset(eps_gn_t, EPS_GN)
            ident = cst[:, 0:128]
            triu = cst[:, 128:256]
            identb = pp.tile([128, 128], BF16, tag="identb", name="identb")
            nc.scalar.activation(out=identb, in_=ident, func=AF.Copy)
            onesr_t = pp.tile([128, 128], F32R, tag="onesr", name="onesr")
            nc.sync.dma_start(out=onesr_t, in_=consts[:, 256:384].bitcast(F32R))
            ones_r = onesr_t

            def vcol(vt, i, j):
                return vt[i][:, j:j + 1]

            def load_w(dram_ap, shape, tag, bufs=2):
                t = wp.tile(shape, BF16, tag=tag, name="wld", bufs=bufs)
                nc.sync.dma_start(out=t, in_=dram_ap)
                return t

            def load_wblk(dram_2d, shape):
                # one DMA for a [C, w] weight block -> SBUF [128, CT, w]
                t = wp.tile(shape, BF16, tag="wblk", name="wblk", bufs=2)
                nc.sync.dma_start(
                    out=t, in_=dram_2d.rearrange("(k p) n -> p k n", p=128))
                return t

            def ln_stats(src_sl):
                ssum = psC.tile([1, TTW], F32, tag="stA", name="stA")
                ssq = psC.tile([1, TTW], F32, tag="stB", name="stB")
                for i in range(CT):
                    sq = k2.tile([128, TTW], F32R, tag="lnt1", name="lnt1")
                    nc.vector.tensor_mul(out=sq, in0=src_sl[i], in1=src_sl[i])
                    nc.tensor.matmul(out=ssum, lhsT=ones_r[:, 0:1], rhs=src_sl[i],
                                     start=(i == 0), stop=(i == CT - 1))
                    nc.tensor.matmul(out=ssq, lhsT=ones_r[:, 0:1], rhs=sq,
                                     start=(i == 0), stop=(i == CT - 1))
                rows = k2.tile([128, TTW], F32, tag="lnrows", name="lnrows", bufs=1)
                srow, s2, varu, lnv = (rows[j:j + 1, :] for j in (0, 32, 64, 96))
                nc.scalar.activation(out=srow, in_=ssum, func=AF.Copy)
                nc.vector.tensor_mul(out=s2, in0=srow, in1=srow)
                nc.vector.scalar_tensor_tensor(out=varu, in0=s2, scalar=-1.0 / C,
                                               in1=ssq, op0=OP.mult, op1=OP.add)
                nc.scalar.activation(out=lnv, in_=varu, func=AF.Ln, scale=1.0 / C,
                                     bias=eps_ln_t[0:1, :])
                rs = k2.tile([1, TTW], F32R, tag="lnrs", name="lnrs", bufs=1)
                nc.scalar.activation(out=rs, in_=lnv, func=AF.Exp, scale=-0.5)
                murs = k2.tile([1, TTW], F32R, tag="lnmu", name="lnmu", bufs=1)
                nc.vector.scalar_tensor_tensor(out=murs, in0=srow, scalar=1.0 / C,
                                               in1=rs, op0=OP.mult, op1=OP.mult)
                bc0 = psB.tile([128, TTW], F32, tag="pw", name="pw")
                nc.tensor.matmul(out=bc0, lhsT=ones_r[0:1, 0:128], rhs=rs,
                                 start=True, stop=True)
                bc1 = psB.tile([128, TTW], F32, tag="pw", name="pw")
                nc.tensor.matmul(out=bc1, lhsT=ones_r[0:1, 0:128], rhs=murs,
                                 start=True, stop=True)
                return bc0, bc1

            def ln_apply(src_sl, bc0, bc1, vt, wi, bi, out_tiles):
                for i in range(CT):
                    t1 = k2.tile([128, TTW], F32, tag="lnt1", name="lnt1")
                    nc.vector.tensor_mul(out=t1, in0=src_sl[i], in1=bc0)
                    t2 = k2.tile([128, TTW], F32, tag="lnt2", name="lnt2")
                    nc.vector.tensor_sub(out=t2, in0=t1, in1=bc1)
                    nc.vector.tensor_scalar(out=out_tiles[i], in0=t2,
                                            scalar1=vcol(vt, i, wi),
                                            scalar2=vcol(vt, i, bi),
                                            op0=OP.mult, op1=OP.add)

            def tanh_route(psum_ap, out_tile):
                nc.scalar.activation(out=out_tile, in_=psum_ap, func=AF.Tanh)

            # ================= layers =================
            for l in range(NL):
                vecs = []
                for i in range(CT):
                    vt = pp.tile([128, NV], F32, tag=f"vecs{i}", name=f"vecs{i}")
                    nc.sync.dma_start(out=vt, in_=vecs_in[l][i * 128:(i + 1) * 128, :])
                    vecs.append(vt)
                lnxt = pp.tile([128, 1024], F32, tag="lnx", name="lnx")
                nc.sync.dma_start(out=lnxt, in_=lnx_in[l][:, :])
                hmu = []
                for i in range(CT):
                    ht = pp.tile([128, 8], BF16, tag=f"hmu{i}", name=f"hmu{i}")
                    nc.sync.dma_start(out=ht, in_=hmu_in[l][i * 128:(i + 1) * 128, :])
                    hmu.append(ht)

                # ---- xb init ----
                if l == 0:
                    for tt in range(NTT):
                        sl = slice(tt * TTW, (tt + 1) * TTW)
                        src = [xres[i][:, sl] for i in range(CT)]
                        bc0, bc1 = ln_stats(src)
                        xbs = [xb[i][:, sl] for i in range(CT)]
                        ln_apply(src, bc0, bc1, vecs, V_LN0W, V_LN0B, xbs)
                        for i in range(CT):
                            d = k2.tile([128, TTW], F32, tag="lnt2", name="lnt2")
                            nc.vector.tensor_sub(out=d, in0=xb[i][:, sl],
                                                 in1=xres[i][:, sl])
                            nc.vector.scalar_tensor_tensor(
                                out=xb[i][:, sl], in0=d,
                                scalar=selt[:, S_LN0:S_LN0 + 1],
                                in1=xres[i][:, sl], op0=OP.mult, op1=OP.add)
                else:
                    for i in range(CT):
                        nc.gpsimd.tensor_copy(out=xb[i], in_=xres[i])

                S_cur = [pp.tile([128, HN], BF16, tag=f"S{i}", name=f"S{i}") for i in range(CT)]
                for i in range(CT):
                    nc.vector.memset(S_cur[i], 0.0)
                carry = [pp.tile([128, 1], F32, tag=f"ca{i}", name=f"ca{i}") for i in range(CT)]
                carry2 = [pp.tile([128, 1], F32, tag=f"cb{i}", name=f"cb{i}") for i in range(CT)]
                for i in range(CT):
                    nc.gpsimd.memset(carry[i], 0.0)
                    nc.gpsimd.memset(carry2[i], 0.0)

                # ================= time mix =================
                for tt in range(NTT):
                    sl = slice(tt * TTW, (tt + 1) * TTW)
                    xbs = [xb[i][:, sl] for i in range(CT)]
                    bc0, bc1 = ln_stats(xbs)
                    xt = [kp.tile([128, TTW], F32, tag=f"xt{i}", name=f"xt{i}") for i in range(CT)]
                    ln_apply(xbs, bc0, bc1, vecs, V_LN1W, V_LN1B, xt)
                    xx = [kp.tile([128, TTW], F32, tag=f"xx{i}", name=f"xx{i}") for i in range(CT)]
                    for i in range(CT):
                        nc.vector.tensor_sub(out=xx[i][:, 1:TTW],
                                             in0=xt[i][:, 0:TTW - 1],
                                             in1=xt[i][:, 1:TTW])
                        nc.vector.scalar_tensor_tensor(
                            out=xx[i][:, 0:1], in0=carry[i], scalar=1.0,
                            in1=xt[i][:, 0:1], op0=OP.mult, op1=OP.subtract)
                        nc.gpsimd.tensor_copy(out=carry[i], in_=xt[i][:, TTW - 1:TTW])
                    if dbg and l == 0:
                        for i in range(CT):
                            nc.sync.dma_start(out=dbg_o["xt0"][i * 128:(i + 1) * 128, sl],
                                              in_=xt[i])

                    # ---- t5 ----
                    mx = [k2.tile([128, TTW], BF16, tag=f"xf{i}", name=f"xf{i}") for i in range(CT)]
                    for i in range(CT):
                        nc.vector.scalar_tensor_tensor(
                            out=mx[i], in0=xx[i], scalar=vcol(vecs, i, V_MAAX),
                            in1=xt[i], op0=OP.mult, op1=OP.add)
                    p160a = psC.tile([128, TTW], F32, tag="stA", name="stA")
                    p160b = psC.tile([32, TTW], F32, tag="stB", name="stB")
                    wtmtd = wp.tile([128, CT, 224], BF16, tag="wtmtd", name="wtmtd",
                                    bufs=1)
                    nc.sync.dma_start(
                        out=wtmtd,
                        in_=wbig[l][:, WB_TM1:WB_TM1 + 224]
                        .rearrange("(k p) n -> p k n", p=128))
                    for i in range(CT):
                        nc.tensor.matmul(out=p160a, lhsT=wtmtd[:, i, 0:128], rhs=mx[i],
                                         start=(i == 0), stop=(i == CT - 1))
                        nc.tensor.matmul(out=p160b, lhsT=wtmtd[:, i, 128:160], rhs=mx[i],
                                         start=(i == 0), stop=(i == CT - 1))
                    t5 = [k2.tile([32, TTW], BF16, tag=f"t5{f}", name=f"t5{f}", bufs=1) for f in range(5)]
                    for f in range(4):
                        tanh_route(p160a[f * 32:(f + 1) * 32, :], t5[f])
                    tanh_route(p160b, t5[4])

                    def build_xf(fidx, maa_i):
                        w2 = load_w(tmw2[l][fidx * TM:(fidx + 1) * TM, :],
                                    [TM, C], "wtm2")
                        xft = []
                        for i in range(CT):
                            dlp = psB.tile([128, TTW], F32, tag="pw", name="pw")
                            nc.tensor.matmul(out=dlp,
                                             lhsT=w2[:, i * 128:(i + 1) * 128],
                                             rhs=t5[fidx], start=True, stop=True)
                            a = k2.tile([128, TTW], F32, tag="lnt1", name="lnt1")
                            nc.vector.scalar_tensor_tensor(
                                out=a, in0=dlp, scalar=vcol(vecs, i, maa_i),
                                in1=xx[i], op0=OP.add, op1=OP.mult)
                            xf = k2.tile([128, TTW], BF16, tag=f"xf{i}", name=f"xf{i}")
                            nc.vector.tensor_add(out=xf, in0=a, in1=xt[i])
                            xft.append(xf)
                        return xft

                    def wmm(col_off, xft):
                        accs = [psA.tile([128, TTW], F32, tag=f"acc{m}",
                                         name=f"acc{m}") for m in range(4)]
                        wt = load_wblk(wbig[l][:, col_off:col_off + 512],
                                       [128, CT, 512])
                        for i in range(CT):
                            for m in range(4):
                                nc.tensor.matmul(out=accs[m],
                                                 lhsT=wt[:, i, m * 128:(m + 1) * 128],
                                                 rhs=xft[i], start=(i == 0),
                                                 stop=(i == CT - 1))
                        return accs

                    def wmm_tm(col_off, xft):
                        accs = [psA.tile([128, TTW], F32, tag=f"acc{m}",
                                         name=f"acc{m}") for m in range(4)]
                        wt = load_wblk(wbig[l][:, col_off:col_off + 512],
                                       [128, CT, 512])
                        for i in range(CT):
                            for ci in range(NCPT):
                                nc.tensor.matmul(out=accs[ci],
                                                 lhsT=xft[i][:, ci * L:(ci + 1) * L],
                                                 rhs=wt[:, i, :], start=(i == 0),
                                                 stop=(i == CT - 1))
                        return accs

                    # k
                    xf = build_xf(1, V_MAAK)
                    accs = wmm(WB_K, xf)
                    k_sb = [kp.tile([128, TTW], F32, tag=f"ksb{i}", name=f"ksb{i}") for i in range(CT)]
                    for m in range(4):
                        nc.scalar.activation(out=k_sb[m], in_=accs[m], func=AF.Copy)
                    # v token-major
                    xf = build_xf(2, V_MAAV)
                    accs = wmm_tm(WB_V, xf)
                    v_tm = [kp.tile([128, C], F32, tag=f"vtm{ci}", name=f"vtm{ci}") for ci in range(NCPT)]
                    v_tmb = [kp.tile([128, C], BF16, tag=f"vtb{ci}", name=f"vtb{ci}") for ci in range(NCPT)]
                    for ci in range(NCPT):
                        nc.scalar.activation(out=v_tm[ci], in_=accs[ci], func=AF.Copy)
                        nc.scalar.activation(out=v_tmb[ci], in_=accs[ci], func=AF.Copy)
                    # r
                    xf = build_xf(3, V_MAAR)
                    accs = wmm(WB_R, xf)
                    r_sb = [kp.tile([128, TTW], F32, tag=f"rsb{i}", name=f"rsb{i}") for i in range(CT)]
                    for m in range(4):
                        nc.scalar.activation(out=r_sb[m], in_=accs[m], func=AF.Copy)
                    if dbg and l == 0:
                        for i in range(CT):
                            nc.sync.dma_start(out=dbg_o["r0"][i * 128:(i + 1) * 128, sl],
                                              in_=r_sb[i])
                    # g token-major, silu
                    xf = build_xf(4, V_MAAG)
                    accs = wmm_tm(WB_G, xf)
                    g_tm = [kp.tile([128, C], F32, tag=f"gtm{ci}", name=f"gtm{ci}") for ci in range(NCPT)]
                    for ci in range(NCPT):
                        nc.scalar.activation(out=g_tm[ci], in_=accs[ci], func=AF.Silu)
                    # w -> wacc -> lai
                    xf = build_xf(0, V_MAAW)
                    tdp = psC.tile([TD, TTW], F32, tag="stA", name="stA")
                    for i in range(CT):
                        nc.tensor.matmul(out=tdp, lhsT=wtmtd[:, i, 160:224], rhs=xf[i],
                                         start=(i == 0), stop=(i == CT - 1))
                    tdt = k2.tile([TD, TTW], BF16, tag="tdt", name="tdt", bufs=1)
                    tanh_route(tdp, tdt)
                    w2t = load_w(tdw2[l][:, :], [TD, C], "wtd2", bufs=1)
                    lai = [kp.tile([128, 1 + TTW], F32, tag=f"lai{i}", name=f"lai{i}") for i in range(CT)]
                    for i in range(CT):
                        wwp = psB.tile([128, TTW], F32, tag="pw", name="pw")
                        nc.tensor.matmul(out=wwp, lhsT=w2t[:, i * 128:(i + 1) * 128],
                                         rhs=tdt, start=True, stop=True)
                        wacc = k2.tile([128, TTW], F32, tag="lnt1", name="lnt1")
                        nc.scalar.activation(out=wacc, in_=wwp, func=AF.Exp,
                                             bias=vcol(vecs, i, V_TDCY))
                        nc.gpsimd.memset(lai[i][:, 0:1], 0.0)
                        nc.vector.tensor_tensor_scan(
                            out=lai[i][:, 1:1 + TTW], data0=wacc, data1=wacc,
                            initial=0.0, op0=OP.add, op1=OP.bypass)
                        if dbg and l == 0:
                            nc.sync.dma_start(
                                out=dbg_o["w0"][i * 128:(i + 1) * 128, sl], in_=wacc)
                            nc.sync.dma_start(
                                out=dbg_o["lai0"][i * 128:(i + 1) * 128, sl],
                                in_=lai[i][:, 1:1 + TTW])

                    # ---- wkv chunks ----
                    ztc = [kp.tile([128, TTW], BF16, tag=f"ztc{i}", name=f"ztc{i}") for i in range(CT)]
                    for ci in range(NCPT):
                        gc = tt * NCPT + ci
                        c0 = ci * L
                        fpc, fnc, rt_t, kt_t, kh_tm, m_t = [], [], [], [], [], []
                        for i in range(CT):
                            ngc = k2.tile([128, 1], F32, tag="ngc", name="ngc", bufs=4)
                            nc.vector.tensor_scalar_mul(out=ngc,
                                                        in0=lai[i][:, c0:c0 + 1],
                                                        scalar1=-1.0)
                            fp = k2.tile([128, 1 + L], F32, tag="fp", name="fp", bufs=4)
                            nc.scalar.activation(out=fp, in_=lai[i][:, c0:c0 + 1 + L],
                                                 func=AF.Exp, bias=ngc)
                            fn = k2.tile([128, 1 + L], F32, tag="fn", name="fn", bufs=4)
                            nc.scalar.activation(out=fn, in_=lai[i][:, c0:c0 + 1 + L],
                                                 func=AF.Exp, scale=-1.0,
                                                 bias=lai[i][:, c0:c0 + 1])
                            fpc.append(fp)
                            fnc.append(fn)
                            rt = k2.tile([128, L], BF16, tag="rt", name="rt", bufs=4)
                            nc.vector.tensor_mul(out=rt, in0=r_sb[i][:, c0:c0 + L],
                                                 in1=fn[:, 0:L])
                            kt = k2.tile([128, L], BF16, tag="kt", name="kt", bufs=4)
                            nc.vector.tensor_mul(out=kt, in0=k_sb[i][:, c0:c0 + L],
                                                 in1=fp[:, 1:1 + L])
                            kh = k2.tile([128, L], F32, tag="kh", name="kh", bufs=4)
                            nc.vector.tensor_scalar_mul(out=kh, in0=kt,
                                                        scalar1=fn[:, L:L + 1])
                            mt = k2.tile([128, L], BF16, tag="mt", name="mt", bufs=4)
                            nc.gpsimd.tensor_mul(out=mt, in0=r_sb[i][:, c0:c0 + L],
                                                 in1=k_sb[i][:, c0:c0 + L])
                            rt_t.append(rt)
                            kt_t.append(kt)
                            m_t.append(mt)
                            trp = psB.tile([128, L], F32, tag="pw", name="pw")
                            nc.tensor.transpose(out=trp, in_=kh, identity=ident)
                            kht = k2.tile([128, L], BF16, tag="khtm", name="khtm", bufs=4)
                            nc.scalar.activation(out=kht, in_=trp, func=AF.Copy)
                            kh_tm.append(kht)
                        dall = psC.tile([128, 8], F32, tag="stB", name="stB")
                        for i in range(CT):
                            nc.tensor.matmul(out=dall, lhsT=m_t[i], rhs=hmu[i],
                                             start=(i == 0), stop=(i == CT - 1))
                        yps = psA.tile([128, C], F32, tag="acc0", name="acc0")
                        S_new = [k2.tile([128, HN], BF16, tag=f"Sn{i}", name=f"Sn{i}") for i in range(CT)]
                        for i in range(CT):
                            sup = psC.tile([128, HN], F32, tag="stA", name="stA")
                            for hh in range(2):
                                h = 2 * i + hh
                                hb = hh * HN
                                pt = psB.tile([L, L], F32, tag="pw", name="pw")
                                nc.tensor.matmul(out=pt, lhsT=kt_t[i][hb:hb + HN, :],
                                                 rhs=rt_t[i][hb:hb + HN, :],
                                                 start=True, stop=True)
                                pts = k2.tile([L, L], BF16, tag="pts", name="pts")
                                nc.vector.tensor_mul(out=pts, in0=pt, in1=triu)
                                nc.tensor.matmul(
                                    out=yps[:, h * HN:(h + 1) * HN], lhsT=pts,
                                    rhs=v_tmb[ci][:, h * HN:(h + 1) * HN],
                                    start=True, stop=(gc == 0), skip_group_check=True)
                                if gc > 0:
                                    nc.tensor.matmul(
                                        out=yps[:, h * HN:(h + 1) * HN],
                                        lhsT=rt_t[i][hb:hb + HN, :],
                                        rhs=S_cur[i][hb:hb + HN, :],
                                        start=False, stop=True, skip_group_check=True)
                                nc.tensor.matmul(
                                    out=sup[hb:hb + HN, :],
                                    lhsT=kh_tm[i][:, hb:hb + HN],
                                    rhs=v_tmb[ci][:, h * HN:(h + 1) * HN],
                                    start=True, stop=True, skip_group_check=True)
                            t0 = k2.tile([128, HN], F32, tag="ssc", name="ssc", bufs=4)
                            nc.vector.tensor_scalar_mul(out=t0, in0=S_cur[i],
                                                        scalar1=fnc[i][:, L:L + 1])
                            nc.vector.tensor_add(out=S_new[i], in0=t0, in1=sup)
                        S_cur = S_new
                        ysb = k2.tile([128, C], F32, tag="ysb", name="ysb")
                        for h in range(H):
                            nc.vector.scalar_tensor_tensor(
                                out=ysb[:, h * HN:(h + 1) * HN],
                                in0=v_tm[ci][:, h * HN:(h + 1) * HN],
                                scalar=dall[:, h:h + 1],
                                in1=yps[:, h * HN:(h + 1) * HN],
                                op0=OP.mult, op1=OP.add)
                        if dbg and l == 0:
                            nc.sync.dma_start(
                                out=dbg_o["y0tm"][tt * TTW + c0:tt * TTW + c0 + L, :],
                                in_=ysb)
                            nc.sync.dma_start(
                                out=dbg_o["g0tm"][tt * TTW + c0:tt * TTW + c0 + L, :],
                                in_=g_tm[ci])
                        # groupnorm + affine + *g  (token-major)
                        mv = k2.tile([128, 16], F32, tag="gnmv", name="gnmv")
                        for h in range(H):
                            st = k2.tile([128, 6], F32, tag="gnst", name="gnst")
                            nc.vector.bn_stats(out=st, in_=ysb[:, h * HN:(h + 1) * HN])
                            nc.vector.bn_aggr(out=mv[:, 2 * h:2 * h + 2], in_=st)
                        lnv = k2.tile([128, 8], F32, tag="gnln", name="gnln")
                        var_view = bass.AP(tensor=mv.tensor, offset=mv.offset + 1,
                                           ap=[mv.ap[0], [2, 8]])
                        nc.scalar.activation(out=lnv, in_=var_view, func=AF.Ln,
                                             bias=eps_gn_t)
                        rsg = k2.tile([128, 8], F32, tag="gnrs", name="gnrs")
                        nc.scalar.activation(out=rsg, in_=lnv, func=AF.Exp, scale=-0.5)
                        for h in range(H):
                            nc.vector.tensor_scalar(
                                out=ysb[:, h * HN:(h + 1) * HN],
                                in0=ysb[:, h * HN:(h + 1) * HN],
                                scalar1=mv[:, 2 * h:2 * h + 1],
                                scalar2=rsg[:, h:h + 1],
                                op0=OP.subtract, op1=OP.mult)
                        nc.gpsimd.tensor_mul(out=ysb, in0=ysb, in1=lnxt[:, 0:512])
                        nc.gpsimd.tensor_add(out=ysb, in0=ysb, in1=lnxt[:, 512:1024])
                        nc.vector.tensor_mul(out=ysb, in0=ysb, in1=g_tm[ci])
                        for i in range(CT):
                            trp = psB.tile([128, L], F32, tag="pw", name="pw")
                            nc.tensor.transpose(out=trp,
                                                in_=ysb[:, i * 128:(i + 1) * 128],
                                                identity=ident)
                            nc.scalar.activation(out=ztc[i][:, c0:c0 + L], in_=trp,
                                                 func=AF.Copy)
                    # ---- Wo ----
                    accs = [psA.tile([128, TTW], F32, tag=f"acc{m}",
                                     name=f"acc{m}") for m in range(4)]
                    wt = load_wblk(wbig[l][:, WB_O:WB_O + 512], [128, CT, 512])
                    for i in range(CT):
                        for m in range(4):
                            nc.tensor.matmul(out=accs[m],
                                             lhsT=wt[:, i, m * 128:(m + 1) * 128],
                                             rhs=ztc[i], start=(i == 0),
                                             stop=(i == CT - 1))
                    for m in range(4):
                        nc.vector.tensor_add(out=xb[m][:, sl], in0=xb[m][:, sl],
                                             in1=accs[m])
                    if dbg and l == 0:
                        for i in range(CT):
                            nc.sync.dma_start(
                                out=dbg_o["xbtm0"][i * 128:(i + 1) * 128, sl],
                                in_=xb[i][:, sl].bitcast(F32))

                # ================= channel mix =================
                sendh = [dp.tile([C, TTW], F32, tag=f"send{h}", name=f"send{h}")
                         for h in range(NTT)]
                recvh = [dp.tile([2 * C, TTW], F32, tag=f"recv{h}", name=f"recv{h}")
                         for h in range(NTT)]
                for tt in range(NTT):
                    sl = slice(tt * TTW, (tt + 1) * TTW)
                    xbs = [xb[i][:, sl] for i in range(CT)]
                    bc0, bc1 = ln_stats(xbs)
                    xc = [kp.tile([128, TTW], F32, tag=f"xt{i}", name=f"xt{i}") for i in range(CT)]
                    ln_apply(xbs, bc0, bc1, vecs, V_LN2W, V_LN2B, xc)
                    xx2 = [kp.tile([128, TTW], F32, tag=f"xx{i}", name=f"xx{i}") for i in range(CT)]
                    for i in range(CT):
                        nc.vector.tensor_sub(out=xx2[i][:, 1:TTW],
                                             in0=xc[i][:, 0:TTW - 1],
                                             in1=xc[i][:, 1:TTW])
                        nc.vector.scalar_tensor_tensor(
                            out=xx2[i][:, 0:1], in0=carry2[i], scalar=1.0,
                            in1=xc[i][:, 0:1], op0=OP.mult, op1=OP.subtract)
                        nc.gpsimd.tensor_copy(out=carry2[i], in_=xc[i][:, TTW - 1:TTW])
                    xk2 = [kp.tile([128, TTW], BF16, tag=f"xk2{i}", name=f"xk2{i}") for i in range(CT)]
                    xr2 = [kp.tile([128, TTW], BF16, tag=f"xr2{i}", name=f"xr2{i}") for i in range(CT)]
                    for i in range(CT):
                        nc.vector.scalar_tensor_tensor(
                            out=xk2[i], in0=xx2[i], scalar=vcol(vecs, i, V_CMK),
                            in1=xc[i], op0=OP.mult, op1=OP.add)
                        nc.vector.scalar_tensor_tensor(
                            out=xr2[i], in0=xx2[i], scalar=vcol(vecs, i, V_CMR),
                            in1=xc[i], op0=OP.mult, op1=OP.add)
                    # cm_Wr -> sigmoid
                    accs = [psA.tile([128, TTW], F32, tag=f"acc{m}", name=f"acc{m}") for m in range(4)]
                    wt = load_wblk(cmrg[l][:, 0:512], [128, CT, 512])
                    for i in range(CT):
                        for m in range(4):
                            nc.tensor.matmul(out=accs[m],
                                             lhsT=wt[:, i, m * 128:(m + 1) * 128],
                                             rhs=xr2[i], start=(i == 0),
                                             stop=(i == CT - 1))
                    sig = [kp.tile([128, TTW], F32, tag=f"gtm{m}", name=f"sig{m}") for m in range(4)]
                    for m in range(4):
                        nc.scalar.activation(out=sig[m], in_=accs[m], func=AF.Sigmoid)
                    # kk loop with cm_Wv accumulation
                    accs = [psA.tile([128, TTW], F32, tag=f"acc{m}", name=f"acc{m}") for m in range(4)]
                    wfq = wvq = None
                    for f in range(NFF):
                        fq, fr2 = f // 4, f % 4
                        nq = min(4, NFF - 4 * fq)
                        if fr2 == 0:
                            wfq = wp.tile([128, nq, CT, 128], BF16, tag="wblk",
                                          name="wfq", bufs=2)
                            nc.sync.dma_start(
                                out=wfq,
                                in_=cmkp[l][4 * fq:4 * fq + nq]
                                .rearrange("f (k p) n -> p f k n", p=128))
                            wvq = wp.tile([128, nq, C], BF16, tag="wblk",
                                          name="wvq", bufs=2)
                            nc.sync.dma_start(
                                out=wvq,
                                in_=cmvp[l][4 * fq * 128:(4 * fq + nq) * 128, :]
                                .rearrange("(f p) n -> p f n", p=128))
                        kkp = psB.tile([128, TTW], F32, tag="pw", name="pw")
                        for i in range(CT):
                            nc.tensor.matmul(out=kkp, lhsT=wfq[:, fr2, i, :],
                                             rhs=xk2[i],
                                             start=(i == 0), stop=(i == CT - 1))
                        rl = k2.tile([128, TTW], F32, tag="lnt1", name="lnt1")
                        nc.scalar.activation(out=rl, in_=kkp, func=AF.Relu)
                        kkf = k2.tile([128, TTW], BF16, tag="lnt2k", name="lnt2k")
                        nc.scalar.activation(out=kkf, in_=rl, func=AF.Square)
                        for m in range(4):
                            nc.tensor.matmul(out=accs[m],
                                             lhsT=wvq[:, fr2, m * 128:(m + 1) * 128],
                                             rhs=kkf, start=(f == 0),
                                             stop=(f == NFF - 1))
                    for m in range(4):
                        nc.vector.tensor_mul(out=sig[m], in0=sig[m], in1=accs[m])
                        nc.vector.tensor_add(out=xb[m][:, sl], in0=xb[m][:, sl],
                                             in1=sig[m])
                    # fire this half's exchange as soon as it is final: own
                    # slice tt lands (time-reversed) in the partner's half
                    # NTT-1-tt, so its AllGather overlaps the next tt's
                    # compute instead of serializing at the layer boundary.
                    oh = NTT - 1 - tt
                    for i in range(CT):
                        rv = k2.tile([128, TTW], F32, tag="revst", name="revst")
                        nc.vector.tensor_copy(
                            out=rv, in_=_revap(xb[i][:, sl].bitcast(F32)))
                        nc.sync.dma_start(out=sendh[oh][i * 128:(i + 1) * 128, :],
                                          in_=rv)
                    if solo:
                        for i in range(CT):
                            nc.sync.dma_start(
                                out=recvh[tt][i * 128:(i + 1) * 128, :],
                                in_=xb[i][:, sl].bitcast(F32))
                            nc.sync.dma_start(
                                out=recvh[tt][C + i * 128:C + (i + 1) * 128, :],
                                in_=xb[i][:, sl].bitcast(F32))
                    else:
                        nc.gpsimd.collective_compute(
                            "AllGather", OP.bypass, replica_groups=groups,
                            ins=[sendh[oh].opt()], outs=[recvh[oh].opt()])
                if dbg and l == 0:
                    for i in range(CT):
                        nc.sync.dma_start(out=dbg_o["p0"][i * 128:(i + 1) * 128, :],
                                          in_=xb[i].bitcast(F32))

                # ================= join =================
                # tt=NTT-1 first: its recv half was AllGathered right after
                # the first chanmix slice, so it never waits on the wire.
                for tt in reversed(range(NTT)):
                    sl = slice(tt * TTW, (tt + 1) * TTW)
                    # conv residue, all ct (fp32 accumulation, bf16 final for matmul)
                    cv32 = [kp.tile([128, TTW], F32, tag=f"ksb{i}", name=f"ksb{i}") for i in range(CT)]
                    cv = [k2.tile([128, TTW], BF16, tag=f"cvb{i}", name=f"cvb{i}") for i in range(CT)]
                    a = tt * TTW
                    for i in range(CT):
                        nc.vector.tensor_scalar_mul(out=cv32[i], in0=xres[i][:, sl],
                                                    scalar1=vcol(vecs, i, V_CW1))
                        lo = 1 if tt == 0 else 0
                        nc.vector.scalar_tensor_tensor(
                            out=cv32[i][:, lo:TTW],
                            in0=xres[i][:, a + lo - 1:a + TTW - 1],
                            scalar=vcol(vecs, i, V_CW0),
                            in1=cv32[i][:, lo:TTW], op0=OP.mult, op1=OP.add)
                        hi = TTW - 1 if tt == NTT - 1 else TTW
                        nc.vector.scalar_tensor_tensor(
                            out=cv[i][:, 0:hi],
                            in0=xres[i][:, a + 1:a + hi + 1],
                            scalar=vcol(vecs, i, V_CW2),
                            in1=cv32[i][:, 0:hi], op0=OP.mult, op1=OP.add)
                        if hi < TTW:
                            nc.scalar.activation(out=cv[i][:, hi:TTW],
                                                 in_=cv32[i][:, hi:TTW], func=AF.Copy)
                    accs = [psA.tile([128, TTW], F32, tag=f"acc{m}", name=f"acc{m}") for m in range(4)]
                    wt = load_wblk(cmrg[l][:, 512:1024], [128, CT, 512])
                    for i in range(CT):
                        for m in range(4):
                            nc.tensor.matmul(out=accs[m],
                                             lhsT=wt[:, i, m * 128:(m + 1) * 128],
                                             rhs=cv[i], start=(i == 0),
                                             stop=(i == CT - 1))
                    for m in range(4):
                        jr0 = kp.tile([128, TTW], F32, tag="vtm0", name="vtm0")
                        jr1 = kp.tile([128, TTW], F32, tag="vtm1", name="vtm1")
                        nc.sync.dma_start(out=jr0,
                                          in_=recvh[tt][m * 128:(m + 1) * 128, :])
                        nc.sync.dma_start(out=jr1,
                                          in_=recvh[tt][C + m * 128:C + (m + 1) * 128, :])
                        nc.vector.tensor_scalar_mul(
                            out=jr0, in0=jr0, scalar1=selt[:, S_ALPHA:S_ALPHA + 1])
                        nc.vector.scalar_tensor_tensor(
                            out=jr1, in0=jr1, scalar=selt[:, S_BETA:S_BETA + 1],
                            in1=jr0, op0=OP.mult, op1=OP.add)
                        sg = k2.tile([128, TTW], F32, tag="er0", name="er0", bufs=2)
                        nc.scalar.activation(out=sg, in_=accs[m], func=AF.Sigmoid,
                                             scale=selt[:, S_NEGS:S_NEGS + 1],
                                             bias=vcol(vecs, m, V_GBM))  # w_recv
                        wown = kp.tile([128, TTW], F32, tag="vtm3", name="vtm3")
                        nc.vector.tensor_scalar(out=wown, in0=sg, scalar1=-1.0,
                                                scalar2=1.0, op0=OP.mult, op1=OP.add)
                        jsum = kp.tile([128, TTW], F32, tag="vtm2", name="vtm2")
                        nc.vector.tensor_mul(out=jsum, in0=wown, in1=xb[m][:, sl])
                        nc.vector.tensor_mul(out=jr1, in0=sg, in1=jr1)
                        nc.vector.tensor_add(out=jsum, in0=jsum, in1=jr1)
                        # note: host mask05 already includes the 2x factor fold:
                        # mask05 = mask (not 0.5*mask) since sigmoid form used.
                        nc.vector.tensor_mul(out=xres[m][:, sl], in0=jsum,
                                             in1=maskt[:, sl])
            # ---- output ----
            for i in range(CT):
                nc.sync.dma_start(out=xout[i * 128:(i + 1) * 128, :],
                                  in_=xres[i].bitcast(F32))
    nc.compile()
    return nc


def _host_inputs(inputs):
    x = np.asarray(inputs["x"], np.float32)
    lengths = np.asarray(inputs["lengths"]).astype(np.int64)
    pos = np.arange(T, dtype=np.float32)[:, None]
    div = np.exp(np.arange(0, C, 2, dtype=np.float32) * (-np.log(10000.0) / C))
    pe = np.zeros((T, C), np.float32)
    pe[:, 0::2] = np.sin(pos * div)
    pe[:, 1::2] = np.cos(pos * div)
    mask = (np.arange(T)[None, :] < lengths[:, None]).astype(np.float32)

    consts = np.zeros((128, 384), np.float32)
    consts[:, 0:128] = np.eye(128, dtype=np.float32)
    consts[:, 128:256] = np.triu(np.ones((128, 128), np.float32), 1)
    consts[:, 256:384] = 1.0

    gw = np.asarray(inputs["gate_w"], np.float32)
    gb = np.asarray(inputs["gate_b"], np.float32)
    cw = np.asarray(inputs["conv_w"], np.float32)
    cb = np.asarray(inputs["conv_b"], np.float32)

    in_maps = []
    for c in range(8):
        b, d = c % 4, c // 4
        rev = d == 1
        s = -1.0 if rev else 1.0
        xin = (x[b] + pe)
        mrow = mask[b]
        if rev:
            xin = xin[::-1]
            mrow = mrow[::-1]
        m = {
            "x0": np.ascontiguousarray(xin.T),
            "mask05": np.ascontiguousarray(np.broadcast_to(mrow, (128, T))),
            "consts": consts,
        }
        sel = np.zeros((128, 8), np.float32)
        sel[:, S_LN0] = 0.0 if rev else 1.0
        sel[:, S_NEGS] = -s
        sel[:, S_ALPHA] = 1.0 if rev else 0.0
        sel[:, S_BETA] = 0.0 if rev else 1.0
        m["sel"] = sel
        for l in range(NL):
            W = {k: np.asarray(inputs[k], np.float32)[d, l]
                 for k in ["ln1_w", "ln1_b", "ln2_w", "ln2_b", "maa_x", "maa_w",
                           "maa_k", "maa_v", "maa_r", "maa_g", "tm_w1", "tm_w2",
                           "td_w1", "td_w2", "time_decay", "Wr", "Wk", "Wv",
                           "Wg", "Wo", "lnx_w", "lnx_b", "cm_maa_k", "cm_maa_r",
                           "cm_Wk", "cm_Wv", "cm_Wr", "time_faaaa"]}
            import ml_dtypes
            bf16 = ml_dtypes.bfloat16
            m[f"wbig{l}"] = np.ascontiguousarray(np.concatenate(
                [W["Wr"], W["Wk"], W["Wv"], W["Wg"], W["tm_w1"], W["td_w1"],
                 W["Wo"]], axis=1).astype(bf16))
            m[f"tmw2{l}"] = np.ascontiguousarray(
                W["tm_w2"].reshape(5 * TM, C).astype(bf16))
            m[f"tdw2{l}"] = np.ascontiguousarray(W["td_w2"].astype(bf16))
            m[f"cmk{l}"] = np.ascontiguousarray(
                W["cm_Wk"].reshape(C, NFF, 128).transpose(1, 0, 2).astype(bf16))
            m[f"cmv{l}"] = np.ascontiguousarray(W["cm_Wv"].astype(bf16))
            m[f"cmrg{l}"] = np.ascontiguousarray(
                np.concatenate([W["cm_Wr"], gw[l]], axis=1).astype(bf16))
            cwe = cw[l] if not rev else cw[l][:, ::-1]
            gbe = cb[l] @ gw[l] + gb[l]
            vec = np.zeros((C, NV), np.float32)
            vec[:, V_LN1W] = W["ln1_w"]; vec[:, V_LN1B] = W["ln1_b"]
            vec[:, V_LN2W] = W["ln2_w"]; vec[:, V_LN2B] = W["ln2_b"]
            vec[:, V_MAAX] = W["maa_x"]; vec[:, V_MAAW] = W["maa_w"]
            vec[:, V_MAAK] = W["maa_k"]; vec[:, V_MAAV] = W["maa_v"]
            vec[:, V_MAAR] = W["maa_r"]; vec[:, V_MAAG] = W["maa_g"]
            vec[:, V_TDCY] = W["time_decay"]
            vec[:, V_CMK] = W["cm_maa_k"]; vec[:, V_CMR] = W["cm_maa_r"]
            vec[:, V_GBM] = -s * gbe
            vec[:, V_CW0] = cwe[:, 0]
            vec[:, V_CW1] = cwe[:, 1] - 1.0
            vec[:, V_CW2] = cwe[:, 2]
            vec[:, V_LN0W] = np.asarray(inputs["ln0_w"], np.float32)
            vec[:, V_LN0B] = np.asarray(inputs["ln0_b"], np.float32)
            m[f"vecs{l}"] = vec
            lnx = np.zeros((128, 1024), np.float32)
            lnx[:, 0:512] = W["lnx_w"][None, :]
            lnx[:, 512:1024] = W["lnx_b"][None, :]
            m[f"lnx{l}"] = lnx
            u = W["time_faaaa"].reshape(C)
            hmu = np.zeros((C, 8), np.float32)
            for h in range(H):
                hmu[h * HN:(h + 1) * HN, h] = u[h * HN:(h + 1) * HN]
            m[f"hmu{l}"] = hmu.astype(bf16)
        in_maps.append(m)
    return in_maps


def kernel(**inputs):
    if "nc" not in _CACHE:
        _CACHE["nc"] = _build(dbg=False)
    nc = _CACHE["nc"]
    in_maps = _host_inputs(inputs)
    res = run_bass_kernel_spmd(nc, in_maps, list(range(8)))
    out = np.empty((B, T, C), np.float32)
    for b in range(B):
        out[b] = res.results[b]["xout"].T
    return out


if __name__ == "__main__":
    rng = np.random.default_rng(0)
    demo = None



# revision 40
# speedup vs baseline: 11.6388x; 1.0048x over previous
"""BiWKV6 encoder kernel for 8 Trainium2 NeuronCores.

Sharding: (batch, direction) -> 8 units, one per core; core c handles
batch c % 4, direction c // 4. Backward cores run the identical SPMD
program on time-reversed inputs; the only cross-core communication is a
pairwise AllGather of each layer's block output, written time-reversed
into the partner's domain. Within a core activations are channel-major
[C, T]; the WKV scan uses the chunked linear-attention formulation
(chunk 128) with log-space cumulative decay from the DVE prefix scan.
"""
import numpy as np

import concourse.bass as bass
import concourse.tile as tile
from concourse import bacc, mybir
from concourse.bass_utils import run_bass_kernel_spmd

B, T, C = 4, 1024, 512
H, HN = 8, 64
L = 128
TTW = 512
NTT = T // TTW
NCPT = TTW // L
CT = C // 128
TM, TD, FFN, NL = 32, 64, 1792, 2
NFF = FFN // 128
EPS_LN, EPS_GN = 1e-5, 64e-5
NV = 19

F32 = mybir.dt.float32
F32R = mybir.dt.float32r
BF16 = mybir.dt.bfloat16
AF = mybir.ActivationFunctionType
OP = mybir.AluOpType

WB_R, WB_K, WB_V, WB_G, WB_TM1, WB_TD1, WB_O = 0, 512, 1024, 1536, 2048, 2208, 2272
WB_COLS = 2784
(V_LN1W, V_LN1B, V_LN2W, V_LN2B, V_MAAX, V_MAAW, V_MAAK, V_MAAV, V_MAAR,
 V_MAAG, V_TDCY, V_CMK, V_CMR, V_GBM, V_CW0, V_CW1, V_CW2, V_LN0W,
 V_LN0B) = range(NV)
S_LN0, S_NEGS, S_ALPHA, S_BETA = 0, 1, 2, 3

_CACHE = {}


def _revap(ap):
    n = ap.ap[-1][1]
    return bass.AP(tensor=ap.tensor, offset=ap.offset + (n - 1) * ap.ap[-1][0],
                   ap=[ap.ap[0], [-ap.ap[-1][0], n]])


def _build(dbg=False, solo=False):
    nc = bacc.Bacc("TRN2", target_bir_lowering=False, debug=False, num_devices=8)

    x0 = nc.declare_dram_parameter("x0", [C, T], F32, isOutput=False)
    mask05 = nc.declare_dram_parameter("mask05", [128, T], F32, isOutput=False)
    sel_in = nc.declare_dram_parameter("sel", [128, 8], F32, isOutput=False)
    consts = nc.declare_dram_parameter("consts", [128, 384], F32, isOutput=False)
    wbig, tmw2, tdw2, cmkp, cmvp, cmrg, vecs_in, lnx_in, hmu_in = \
        [], [], [], [], [], [], [], [], []
    for l in range(NL):
        wbig.append(nc.declare_dram_parameter(f"wbig{l}", [C, WB_COLS], BF16, isOutput=False))
        tmw2.append(nc.declare_dram_parameter(f"tmw2{l}", [5 * TM, C], BF16, isOutput=False))
        tdw2.append(nc.declare_dram_parameter(f"tdw2{l}", [TD, C], BF16, isOutput=False))
        cmkp.append(nc.declare_dram_parameter(f"cmk{l}", [NFF, C, 128], BF16, isOutput=False))
        cmvp.append(nc.declare_dram_parameter(f"cmv{l}", [FFN, C], BF16, isOutput=False))
        cmrg.append(nc.declare_dram_parameter(f"cmrg{l}", [C, 1024], BF16, isOutput=False))
        vecs_in.append(nc.declare_dram_parameter(f"vecs{l}", [C, NV], F32, isOutput=False))
        lnx_in.append(nc.declare_dram_parameter(f"lnx{l}", [128, 1024], F32, isOutput=False))
        hmu_in.append(nc.declare_dram_parameter(f"hmu{l}", [C, 8], BF16, isOutput=False))
    xout = nc.declare_dram_parameter("xout", [C, T], F32, isOutput=True)
    dbg_o = {}
    if dbg:
        for nm in ["xt0", "r0", "lai0", "xbtm0", "p0", "w0"]:
            dbg_o[nm] = nc.declare_dram_parameter(nm, [C, T], F32, isOutput=True)
        for nm in ["y0tm", "g0tm"]:
            dbg_o[nm] = nc.declare_dram_parameter(nm, [T, C], F32, isOutput=True)

    groups = [[0, 4], [1, 5], [2, 6], [3, 7]]

    with tile.TileContext(nc) as tc:
        with (
            tc.tile_pool(name="pp", bufs=1) as pp,
            tc.tile_pool(name="wp", bufs=2) as wp,
            tc.tile_pool(name="kp", bufs=1) as kp,
            tc.tile_pool(name="k2", bufs=2) as k2,
            tc.tile_pool(name="psA", bufs=1, space="PSUM") as psA,
            tc.tile_pool(name="psB", bufs=2, space="PSUM") as psB,
            tc.tile_pool(name="psC", bufs=1, space="PSUM") as psC,
            tc.tile_pool(name="dp", bufs=2, space="DRAM") as dp,
        ):
            # ------------- persistent loads -------------
            xres = [pp.tile([128, T], F32R, tag=f"xres{i}", name=f"xres{i}") for i in range(CT)]
            xb = [pp.tile([128, T], F32R, tag=f"xb{i}", name=f"xb{i}") for i in range(CT)]
            for i in range(CT):
                nc.sync.dma_start(out=xres[i],
                                  in_=x0[i * 128:(i + 1) * 128, :].bitcast(F32R))
            maskt = pp.tile([128, T], F32, tag="mask", name="mask")
            nc.sync.dma_start(out=maskt, in_=mask05[:, :])
            selt = pp.tile([128, 8], F32, tag="sel", name="sel")
            nc.sync.dma_start(out=selt, in_=sel_in[:, :])
            cst = pp.tile([128, 384], F32, tag="consts", name="consts")
            nc.sync.dma_start(out=cst, in_=consts[:, :])
            eps_ln_t = pp.tile([128, 1], F32, tag="epsln", name="epsln")
            nc.vector.memset(eps_ln_t, EPS_LN)
            eps_gn_t = pp.tile([128, 1], F32, tag="epsgn", name="epsgn")
            nc.vector.memset(eps_gn_t, EPS_GN)
            ident = cst[:, 0:128]
            triu = cst[:, 128:256]
            identb = pp.tile([128, 128], BF16, tag="identb", name="identb")
            nc.scalar.activation(out=identb, in_=ident, func=AF.Copy)
            onesr_t = pp.tile([128, 128], F32R, tag="onesr", name="onesr")
            nc.sync.dma_start(out=onesr_t, in_=consts[:, 256:384].bitcast(F32R))
            ones_r = onesr_t

            def vcol(vt, i, j):
                return vt[i][:, j:j + 1]

            def load_w(dram_ap, shape, tag, bufs=2):
                t = wp.tile(shape, BF16, tag=tag, name="wld", bufs=bufs)
                nc.sync.dma_start(out=t, in_=dram_ap)
                return t

            def load_wblk(dram_2d, shape):
                # one DMA for a [C, w] weight block -> SBUF [128, CT, w]
                t = wp.tile(shape, BF16, tag="wblk", name="wblk", bufs=2)
                nc.sync.dma_start(
                    out=t, in_=dram_2d.rearrange("(k p) n -> p k n", p=128))
                return t

            def ln_stats(src_sl):
                ssum = psC.tile([1, TTW], F32, tag="stA", name="stA")
                ssq = psC.tile([1, TTW], F32, tag="stB", name="stB")
                for i in range(CT):
                    sq = k2.tile([128, TTW], F32R, tag="lnt1", name="lnt1")
                    nc.vector.tensor_mul(out=sq, in0=src_sl[i], in1=src_sl[i])
                    nc.tensor.matmul(out=ssum, lhsT=ones_r[:, 0:1], rhs=src_sl[i],
                                     start=(i == 0), stop=(i == CT - 1))
                    nc.tensor.matmul(out=ssq, lhsT=ones_r[:, 0:1], rhs=sq,
                                     start=(i == 0), stop=(i == CT - 1))
                rows = k2.tile([128, TTW], F32, tag="lnrows", name="lnrows", bufs=1)
                srow, s2, varu, lnv = (rows[j:j + 1, :] for j in (0, 32, 64, 96))
                nc.scalar.activation(out=srow, in_=ssum, func=AF.Copy)
                nc.vector.tensor_mul(out=s2, in0=srow, in1=srow)
                nc.vector.scalar_tensor_tensor(out=varu, in0=s2, scalar=-1.0 / C,
                                               in1=ssq, op0=OP.mult, op1=OP.add)
                nc.scalar.activation(out=lnv, in_=varu, func=AF.Ln, scale=1.0 / C,
                                     bias=eps_ln_t[0:1, :])
                rs = k2.tile([1, TTW], F32R, tag="lnrs", name="lnrs", bufs=1)
                nc.scalar.activation(out=rs, in_=lnv, func=AF.Exp, scale=-0.5)
                murs = k2.tile([1, TTW], F32R, tag="lnmu", name="lnmu", bufs=1)
                nc.vector.scalar_tensor_tensor(out=murs, in0=srow, scalar=1.0 / C,
                                               in1=rs, op0=OP.mult, op1=OP.mult)
                bc0 = psB.tile([128, TTW], F32, tag="pw", name="pw")
                nc.tensor.matmul(out=bc0, lhsT=ones_r[0:1, 0:128], rhs=rs,
                                 start=True, stop=True)
                bc1 = psB.tile([128, TTW], F32, tag="pw", name="pw")
                nc.tensor.matmul(out=bc1, lhsT=ones_r[0:1, 0:128], rhs=murs,
                                 start=True, stop=True)
                return bc0, bc1

            def ln_apply(src_sl, bc0, bc1, vt, wi, bi, out_tiles):
                for i in range(CT):
                    t1 = k2.tile([128, TTW], F32, tag="lnt1", name="lnt1")
                    nc.vector.tensor_mul(out=t1, in0=src_sl[i], in1=bc0)
                    t2 = k2.tile([128, TTW], F32, tag="lnt2", name="lnt2")
                    nc.vector.tensor_sub(out=t2, in0=t1, in1=bc1)
                    nc.vector.tensor_scalar(out=out_tiles[i], in0=t2,
                                            scalar1=vcol(vt, i, wi),
                                            scalar2=vcol(vt, i, bi),
                                            op0=OP.mult, op1=OP.add)

            def tanh_route(psum_ap, out_tile):
                nc.scalar.activation(out=out_tile, in_=psum_ap, func=AF.Tanh)

            # ================= layers =================
            for l in range(NL):
                vecs = []
                for i in range(CT):
                    vt = pp.tile([128, NV], F32, tag=f"vecs{i}", name=f"vecs{i}")
                    nc.sync.dma_start(out=vt, in_=vecs_in[l][i * 128:(i + 1) * 128, :])
                    vecs.append(vt)
                lnxt = pp.tile([128, 1024], F32, tag="lnx", name="lnx")
                nc.sync.dma_start(out=lnxt, in_=lnx_in[l][:, :])
                hmu = []
                for i in range(CT):
                    ht = pp.tile([128, 8], BF16, tag=f"hmu{i}", name=f"hmu{i}")
                    nc.sync.dma_start(out=ht, in_=hmu_in[l][i * 128:(i + 1) * 128, :])
                    hmu.append(ht)

                # ---- xb init ----
                if l == 0:
                    for tt in range(NTT):
                        sl = slice(tt * TTW, (tt + 1) * TTW)
                        src = [xres[i][:, sl] for i in range(CT)]
                        bc0, bc1 = ln_stats(src)
                        xbs = [xb[i][:, sl] for i in range(CT)]
                        ln_apply(src, bc0, bc1, vecs, V_LN0W, V_LN0B, xbs)
                        for i in range(CT):
                            d = k2.tile([128, TTW], F32, tag="lnt2", name="lnt2")
                            nc.vector.tensor_sub(out=d, in0=xb[i][:, sl],
                                                 in1=xres[i][:, sl])
                            nc.vector.scalar_tensor_tensor(
                                out=xb[i][:, sl], in0=d,
                                scalar=selt[:, S_LN0:S_LN0 + 1],
                                in1=xres[i][:, sl], op0=OP.mult, op1=OP.add)
                else:
                    for i in range(CT):
                        nc.gpsimd.tensor_copy(out=xb[i], in_=xres[i])

                S_cur = [pp.tile([128, HN], BF16, tag=f"S{i}", name=f"S{i}") for i in range(CT)]
                for i in range(CT):
                    nc.vector.memset(S_cur[i], 0.0)
                carry = [pp.tile([128, 1], F32, tag=f"ca{i}", name=f"ca{i}") for i in range(CT)]
                carry2 = [pp.tile([128, 1], F32, tag=f"cb{i}", name=f"cb{i}") for i in range(CT)]
                for i in range(CT):
                    nc.gpsimd.memset(carry[i], 0.0)
                    nc.gpsimd.memset(carry2[i], 0.0)

                # ================= time mix =================
                for tt in range(NTT):
                    sl = slice(tt * TTW, (tt + 1) * TTW)
                    # prefetch the first projection's weights: the DMA lands
                    # while ln/t5 compute runs, instead of stalling wmm(K).
                    wtk = wp.tile([128, CT, 512], BF16, tag="wblkK", name="wblkK",
                                  bufs=2)
                    nc.sync.dma_start(
                        out=wtk, in_=wbig[l][:, WB_K:WB_K + 512]
                        .rearrange("(k p) n -> p k n", p=128))
                    xbs = [xb[i][:, sl] for i in range(CT)]
                    bc0, bc1 = ln_stats(xbs)
                    xt = [kp.tile([128, TTW], F32, tag=f"xt{i}", name=f"xt{i}") for i in range(CT)]
                    ln_apply(xbs, bc0, bc1, vecs, V_LN1W, V_LN1B, xt)
                    xx = [kp.tile([128, TTW], F32, tag=f"xx{i}", name=f"xx{i}") for i in range(CT)]
                    for i in range(CT):
                        nc.vector.tensor_sub(out=xx[i][:, 1:TTW],
                                             in0=xt[i][:, 0:TTW - 1],
                                             in1=xt[i][:, 1:TTW])
                        nc.vector.scalar_tensor_tensor(
                            out=xx[i][:, 0:1], in0=carry[i], scalar=1.0,
                            in1=xt[i][:, 0:1], op0=OP.mult, op1=OP.subtract)
                        nc.gpsimd.tensor_copy(out=carry[i], in_=xt[i][:, TTW - 1:TTW])
                    if dbg and l == 0:
                        for i in range(CT):
                            nc.sync.dma_start(out=dbg_o["xt0"][i * 128:(i + 1) * 128, sl],
                                              in_=xt[i])

                    # ---- t5 ----
                    mx = [k2.tile([128, TTW], BF16, tag=f"xf{i}", name=f"xf{i}") for i in range(CT)]
                    for i in range(CT):
                        nc.vector.scalar_tensor_tensor(
                            out=mx[i], in0=xx[i], scalar=vcol(vecs, i, V_MAAX),
                            in1=xt[i], op0=OP.mult, op1=OP.add)
                    p160a = psC.tile([128, TTW], F32, tag="stA", name="stA")
                    p160b = psC.tile([32, TTW], F32, tag="stB", name="stB")
                    wtmtd = wp.tile([128, CT, 224], BF16, tag="wtmtd", name="wtmtd",
                                    bufs=1)
                    nc.sync.dma_start(
                        out=wtmtd,
                        in_=wbig[l][:, WB_TM1:WB_TM1 + 224]
                        .rearrange("(k p) n -> p k n", p=128))
                    for i in range(CT):
                        nc.tensor.matmul(out=p160a, lhsT=wtmtd[:, i, 0:128], rhs=mx[i],
                                         start=(i == 0), stop=(i == CT - 1))
                        nc.tensor.matmul(out=p160b, lhsT=wtmtd[:, i, 128:160], rhs=mx[i],
                                         start=(i == 0), stop=(i == CT - 1))
                    t5 = [k2.tile([32, TTW], BF16, tag=f"t5{f}", name=f"t5{f}", bufs=1) for f in range(5)]
                    for f in range(4):
                        tanh_route(p160a[f * 32:(f + 1) * 32, :], t5[f])
                    tanh_route(p160b, t5[4])

                    def build_xf(fidx, maa_i):
                        w2 = load_w(tmw2[l][fidx * TM:(fidx + 1) * TM, :],
                                    [TM, C], "wtm2")
                        xft = []
                        for i in range(CT):
                            dlp = psB.tile([128, TTW], F32, tag="pw", name="pw")
                            nc.tensor.matmul(out=dlp,
                                             lhsT=w2[:, i * 128:(i + 1) * 128],
                                             rhs=t5[fidx], start=True, stop=True)
                            a = k2.tile([128, TTW], F32, tag="lnt1", name="lnt1")
                            nc.vector.scalar_tensor_tensor(
                                out=a, in0=dlp, scalar=vcol(vecs, i, maa_i),
                                in1=xx[i], op0=OP.add, op1=OP.mult)
                            xf = k2.tile([128, TTW], BF16, tag=f"xf{i}", name=f"xf{i}")
                            nc.vector.tensor_add(out=xf, in0=a, in1=xt[i])
                            xft.append(xf)
                        return xft

                    def wmm(col_off, xft, wt=None):
                        accs = [psA.tile([128, TTW], F32, tag=f"acc{m}",
                                         name=f"acc{m}") for m in range(4)]
                        if wt is None:
                            wt = load_wblk(wbig[l][:, col_off:col_off + 512],
                                           [128, CT, 512])
                        for i in range(CT):
                            for m in range(4):
                                nc.tensor.matmul(out=accs[m],
                                                 lhsT=wt[:, i, m * 128:(m + 1) * 128],
                                                 rhs=xft[i], start=(i == 0),
                                                 stop=(i == CT - 1))
                        return accs

                    def wmm_tm(col_off, xft):
                        accs = [psA.tile([128, TTW], F32, tag=f"acc{m}",
                                         name=f"acc{m}") for m in range(4)]
                        wt = load_wblk(wbig[l][:, col_off:col_off + 512],
                                       [128, CT, 512])
                        for i in range(CT):
                            for ci in range(NCPT):
                                nc.tensor.matmul(out=accs[ci],
                                                 lhsT=xft[i][:, ci * L:(ci + 1) * L],
                                                 rhs=wt[:, i, :], start=(i == 0),
                                                 stop=(i == CT - 1))
                        return accs

                    # w -> wacc -> lai FIRST: the four serial prefix scans run
                    # on the vector engine while the tensor engine chews
                    # through the k/v/r/g projections below.
                    xf = build_xf(0, V_MAAW)
                    tdp = psC.tile([TD, TTW], F32, tag="stA", name="stA")
                    for i in range(CT):
                        nc.tensor.matmul(out=tdp, lhsT=wtmtd[:, i, 160:224], rhs=xf[i],
                                         start=(i == 0), stop=(i == CT - 1))
                    tdt = k2.tile([TD, TTW], BF16, tag="tdt", name="tdt", bufs=1)
                    tanh_route(tdp, tdt)
                    w2t = load_w(tdw2[l][:, :], [TD, C], "wtd2", bufs=1)
                    lai = [kp.tile([128, 1 + TTW], F32, tag=f"lai{i}", name=f"lai{i}") for i in range(CT)]
                    for i in range(CT):
                        wwp = psB.tile([128, TTW], F32, tag="pw", name="pw")
                        nc.tensor.matmul(out=wwp, lhsT=w2t[:, i * 128:(i + 1) * 128],
                                         rhs=tdt, start=True, stop=True)
                        wacc = k2.tile([128, TTW], F32, tag="lnt1", name="lnt1")
                        nc.scalar.activation(out=wacc, in_=wwp, func=AF.Exp,
                                             bias=vcol(vecs, i, V_TDCY))
                        nc.gpsimd.memset(lai[i][:, 0:1], 0.0)
                        nc.vector.tensor_tensor_scan(
                            out=lai[i][:, 1:1 + TTW], data0=wacc, data1=wacc,
                            initial=0.0, op0=OP.add, op1=OP.bypass)
                        if dbg and l == 0:
                            nc.sync.dma_start(
                                out=dbg_o["w0"][i * 128:(i + 1) * 128, sl], in_=wacc)
                            nc.sync.dma_start(
                                out=dbg_o["lai0"][i * 128:(i + 1) * 128, sl],
                                in_=lai[i][:, 1:1 + TTW])
                    # k
                    xf = build_xf(1, V_MAAK)
                    accs = wmm(WB_K, xf, wt=wtk)
                    k_sb = [kp.tile([128, TTW], F32, tag=f"ksb{i}", name=f"ksb{i}") for i in range(CT)]
                    for m in range(4):
                        nc.scalar.activation(out=k_sb[m], in_=accs[m], func=AF.Copy)
                    # v token-major
                    xf = build_xf(2, V_MAAV)
                    accs = wmm_tm(WB_V, xf)
                    v_tm = [kp.tile([128, C], F32, tag=f"vtm{ci}", name=f"vtm{ci}") for ci in range(NCPT)]
                    v_tmb = [kp.tile([128, C], BF16, tag=f"vtb{ci}", name=f"vtb{ci}") for ci in range(NCPT)]
                    for ci in range(NCPT):
                        nc.scalar.activation(out=v_tm[ci], in_=accs[ci], func=AF.Copy)
                        nc.scalar.activation(out=v_tmb[ci], in_=accs[ci], func=AF.Copy)
                    # r
                    xf = build_xf(3, V_MAAR)
                    accs = wmm(WB_R, xf)
                    r_sb = [kp.tile([128, TTW], F32, tag=f"rsb{i}", name=f"rsb{i}") for i in range(CT)]
                    for m in range(4):
                        nc.scalar.activation(out=r_sb[m], in_=accs[m], func=AF.Copy)
                    if dbg and l == 0:
                        for i in range(CT):
                            nc.sync.dma_start(out=dbg_o["r0"][i * 128:(i + 1) * 128, sl],
                                              in_=r_sb[i])
                    # g token-major, silu
                    xf = build_xf(4, V_MAAG)
                    accs = wmm_tm(WB_G, xf)
                    g_tm = [kp.tile([128, C], F32, tag=f"gtm{ci}", name=f"gtm{ci}") for ci in range(NCPT)]
                    for ci in range(NCPT):
                        nc.scalar.activation(out=g_tm[ci], in_=accs[ci], func=AF.Silu)

                    # ---- wkv chunks ----
                    ztc = [kp.tile([128, TTW], BF16, tag=f"ztc{i}", name=f"ztc{i}") for i in range(CT)]
                    for ci in range(NCPT):
                        gc = tt * NCPT + ci
                        c0 = ci * L
                        fpc, fnc, rt_t, kt_t, kh_tm, m_t = [], [], [], [], [], []
                        for i in range(CT):
                            ngc = k2.tile([128, 1], F32, tag="ngc", name="ngc", bufs=4)
                            nc.vector.tensor_scalar_mul(out=ngc,
                                                        in0=lai[i][:, c0:c0 + 1],
                                                        scalar1=-1.0)
                            fp = k2.tile([128, 1 + L], F32, tag="fp", name="fp", bufs=4)
                            nc.scalar.activation(out=fp, in_=lai[i][:, c0:c0 + 1 + L],
                                                 func=AF.Exp, bias=ngc)
                            fn = k2.tile([128, 1 + L], F32, tag="fn", name="fn", bufs=4)
                            nc.scalar.activation(out=fn, in_=lai[i][:, c0:c0 + 1 + L],
                                                 func=AF.Exp, scale=-1.0,
                                                 bias=lai[i][:, c0:c0 + 1])
                            fpc.append(fp)
                            fnc.append(fn)
                            rt = k2.tile([128, L], BF16, tag="rt", name="rt", bufs=4)
                            nc.vector.tensor_mul(out=rt, in0=r_sb[i][:, c0:c0 + L],
                                                 in1=fn[:, 0:L])
                            kt = k2.tile([128, L], BF16, tag="kt", name="kt", bufs=4)
                            nc.vector.tensor_mul(out=kt, in0=k_sb[i][:, c0:c0 + L],
                                                 in1=fp[:, 1:1 + L])
                            kh = k2.tile([128, L], F32, tag="kh", name="kh", bufs=4)
                            nc.vector.tensor_scalar_mul(out=kh, in0=kt,
                                                        scalar1=fn[:, L:L + 1])
                            mt = k2.tile([128, L], BF16, tag="mt", name="mt", bufs=4)
                            nc.gpsimd.tensor_mul(out=mt, in0=r_sb[i][:, c0:c0 + L],
                                                 in1=k_sb[i][:, c0:c0 + L])
                            rt_t.append(rt)
                            kt_t.append(kt)
                            m_t.append(mt)
                            trp = psB.tile([128, L], F32, tag="pw", name="pw")
                            nc.tensor.transpose(out=trp, in_=kh, identity=ident)
                            kht = k2.tile([128, L], BF16, tag="khtm", name="khtm", bufs=4)
                            nc.scalar.activation(out=kht, in_=trp, func=AF.Copy)
                            kh_tm.append(kht)
                        dall = psC.tile([128, 8], F32, tag="stB", name="stB")
                        for i in range(CT):
                            nc.tensor.matmul(out=dall, lhsT=m_t[i], rhs=hmu[i],
                                             start=(i == 0), stop=(i == CT - 1))
                        yps = psA.tile([128, C], F32, tag="acc0", name="acc0")
                        S_new = [k2.tile([128, HN], BF16, tag=f"Sn{i}", name=f"Sn{i}") for i in range(CT)]
                        for i in range(CT):
                            sup = psC.tile([128, HN], F32, tag="stA", name="stA")
                            for hh in range(2):
                                h = 2 * i + hh
                                hb = hh * HN
                                pt = psB.tile([L, L], F32, tag="pw", name="pw")
                                nc.tensor.matmul(out=pt, lhsT=kt_t[i][hb:hb + HN, :],
                                                 rhs=rt_t[i][hb:hb + HN, :],
                                                 start=True, stop=True)
                                pts = k2.tile([L, L], BF16, tag="pts", name="pts")
                                nc.vector.tensor_mul(out=pts, in0=pt, in1=triu)
                                nc.tensor.matmul(
                                    out=yps[:, h * HN:(h + 1) * HN], lhsT=pts,
                                    rhs=v_tmb[ci][:, h * HN:(h + 1) * HN],
                                    start=True, stop=(gc == 0), skip_group_check=True)
                                if gc > 0:
                                    nc.tensor.matmul(
                                        out=yps[:, h * HN:(h + 1) * HN],
                                        lhsT=rt_t[i][hb:hb + HN, :],
                                        rhs=S_cur[i][hb:hb + HN, :],
                                        start=False, stop=True, skip_group_check=True)
                                nc.tensor.matmul(
                                    out=sup[hb:hb + HN, :],
                                    lhsT=kh_tm[i][:, hb:hb + HN],
                                    rhs=v_tmb[ci][:, h * HN:(h + 1) * HN],
                                    start=True, stop=True, skip_group_check=True)
                            t0 = k2.tile([128, HN], F32, tag="ssc", name="ssc", bufs=4)
                            nc.vector.tensor_scalar_mul(out=t0, in0=S_cur[i],
                                                        scalar1=fnc[i][:, L:L + 1])
                            nc.vector.tensor_add(out=S_new[i], in0=t0, in1=sup)
                        S_cur = S_new
                        ysb = k2.tile([128, C], F32, tag="ysb", name="ysb")
                        for h in range(H):
                            nc.vector.scalar_tensor_tensor(
                                out=ysb[:, h * HN:(h + 1) * HN],
                                in0=v_tm[ci][:, h * HN:(h + 1) * HN],
                                scalar=dall[:, h:h + 1],
                                in1=yps[:, h * HN:(h + 1) * HN],
                                op0=OP.mult, op1=OP.add)
                        if dbg and l == 0:
                            nc.sync.dma_start(
                                out=dbg_o["y0tm"][tt * TTW + c0:tt * TTW + c0 + L, :],
                                in_=ysb)
                            nc.sync.dma_start(
                                out=dbg_o["g0tm"][tt * TTW + c0:tt * TTW + c0 + L, :],
                                in_=g_tm[ci])
                        # groupnorm + affine + *g  (token-major)
                        mv = k2.tile([128, 16], F32, tag="gnmv", name="gnmv")
                        for h in range(H):
                            st = k2.tile([128, 6], F32, tag="gnst", name="gnst")
                            nc.vector.bn_stats(out=st, in_=ysb[:, h * HN:(h + 1) * HN])
                            nc.vector.bn_aggr(out=mv[:, 2 * h:2 * h + 2], in_=st)
                        lnv = k2.tile([128, 8], F32, tag="gnln", name="gnln")
                        var_view = bass.AP(tensor=mv.tensor, offset=mv.offset + 1,
                                           ap=[mv.ap[0], [2, 8]])
                        nc.scalar.activation(out=lnv, in_=var_view, func=AF.Ln,
                                             bias=eps_gn_t)
                        rsg = k2.tile([128, 8], F32, tag="gnrs", name="gnrs")
                        nc.scalar.activation(out=rsg, in_=lnv, func=AF.Exp, scale=-0.5)
                        for h in range(H):
                            nc.vector.tensor_scalar(
                                out=ysb[:, h * HN:(h + 1) * HN],
                                in0=ysb[:, h * HN:(h + 1) * HN],
                                scalar1=mv[:, 2 * h:2 * h + 1],
                                scalar2=rsg[:, h:h + 1],
                                op0=OP.subtract, op1=OP.mult)
                        nc.gpsimd.tensor_mul(out=ysb, in0=ysb, in1=lnxt[:, 0:512])
                        nc.gpsimd.tensor_add(out=ysb, in0=ysb, in1=lnxt[:, 512:1024])
                        nc.vector.tensor_mul(out=ysb, in0=ysb, in1=g_tm[ci])
                        for i in range(CT):
                            trp = psB.tile([128, L], F32, tag="pw", name="pw")
                            nc.tensor.transpose(out=trp,
                                                in_=ysb[:, i * 128:(i + 1) * 128],
                                                identity=ident)
                            nc.scalar.activation(out=ztc[i][:, c0:c0 + L], in_=trp,
                                                 func=AF.Copy)
                    # ---- Wo ----
                    accs = [psA.tile([128, TTW], F32, tag=f"acc{m}",
                                     name=f"acc{m}") for m in range(4)]
                    wt = load_wblk(wbig[l][:, WB_O:WB_O + 512], [128, CT, 512])
                    for i in range(CT):
                        for m in range(4):
                            nc.tensor.matmul(out=accs[m],
                                             lhsT=wt[:, i, m * 128:(m + 1) * 128],
                                             rhs=ztc[i], start=(i == 0),
                                             stop=(i == CT - 1))
                    for m in range(4):
                        nc.vector.tensor_add(out=xb[m][:, sl], in0=xb[m][:, sl],
                                             in1=accs[m])
                    if dbg and l == 0:
                        for i in range(CT):
                            nc.sync.dma_start(
                                out=dbg_o["xbtm0"][i * 128:(i + 1) * 128, sl],
                                in_=xb[i][:, sl].bitcast(F32))

                # ================= channel mix =================
                sendh = [dp.tile([C, TTW], F32, tag=f"send{h}", name=f"send{h}")
                         for h in range(NTT)]
                recvh = [dp.tile([2 * C, TTW], F32, tag=f"recv{h}", name=f"recv{h}")
                         for h in range(NTT)]
                for tt in range(NTT):
                    sl = slice(tt * TTW, (tt + 1) * TTW)
                    xbs = [xb[i][:, sl] for i in range(CT)]
                    bc0, bc1 = ln_stats(xbs)
                    xc = [kp.tile([128, TTW], F32, tag=f"xt{i}", name=f"xt{i}") for i in range(CT)]
                    ln_apply(xbs, bc0, bc1, vecs, V_LN2W, V_LN2B, xc)
                    xx2 = [kp.tile([128, TTW], F32, tag=f"xx{i}", name=f"xx{i}") for i in range(CT)]
                    for i in range(CT):
                        nc.vector.tensor_sub(out=xx2[i][:, 1:TTW],
                                             in0=xc[i][:, 0:TTW - 1],
                                             in1=xc[i][:, 1:TTW])
                        nc.vector.scalar_tensor_tensor(
                            out=xx2[i][:, 0:1], in0=carry2[i], scalar=1.0,
                            in1=xc[i][:, 0:1], op0=OP.mult, op1=OP.subtract)
                        nc.gpsimd.tensor_copy(out=carry2[i], in_=xc[i][:, TTW - 1:TTW])
                    xk2 = [kp.tile([128, TTW], BF16, tag=f"xk2{i}", name=f"xk2{i}") for i in range(CT)]
                    xr2 = [kp.tile([128, TTW], BF16, tag=f"xr2{i}", name=f"xr2{i}") for i in range(CT)]
                    for i in range(CT):
                        nc.vector.scalar_tensor_tensor(
                            out=xk2[i], in0=xx2[i], scalar=vcol(vecs, i, V_CMK),
                            in1=xc[i], op0=OP.mult, op1=OP.add)
                        nc.vector.scalar_tensor_tensor(
                            out=xr2[i], in0=xx2[i], scalar=vcol(vecs, i, V_CMR),
                            in1=xc[i], op0=OP.mult, op1=OP.add)
                    # cm_Wr -> sigmoid
                    accs = [psA.tile([128, TTW], F32, tag=f"acc{m}", name=f"acc{m}") for m in range(4)]
                    wt = load_wblk(cmrg[l][:, 0:512], [128, CT, 512])
                    for i in range(CT):
                        for m in range(4):
                            nc.tensor.matmul(out=accs[m],
                                             lhsT=wt[:, i, m * 128:(m + 1) * 128],
                                             rhs=xr2[i], start=(i == 0),
                                             stop=(i == CT - 1))
                    sig = [kp.tile([128, TTW], F32, tag=f"gtm{m}", name=f"sig{m}") for m in range(4)]
                    for m in range(4):
                        nc.scalar.activation(out=sig[m], in_=accs[m], func=AF.Sigmoid)
                    # kk loop with cm_Wv accumulation
                    accs = [psA.tile([128, TTW], F32, tag=f"acc{m}", name=f"acc{m}") for m in range(4)]
                    wfq = wvq = None
                    for f in range(NFF):
                        fq, fr2 = f // 4, f % 4
                        nq = min(4, NFF - 4 * fq)
                        if fr2 == 0:
                            wfq = wp.tile([128, nq, CT, 128], BF16, tag="wblk",
                                          name="wfq", bufs=2)
                            nc.sync.dma_start(
                                out=wfq,
                                in_=cmkp[l][4 * fq:4 * fq + nq]
                                .rearrange("f (k p) n -> p f k n", p=128))
                            wvq = wp.tile([128, nq, C], BF16, tag="wblk",
                                          name="wvq", bufs=2)
                            nc.sync.dma_start(
                                out=wvq,
                                in_=cmvp[l][4 * fq * 128:(4 * fq + nq) * 128, :]
                                .rearrange("(f p) n -> p f n", p=128))
                        kkp = psB.tile([128, TTW], F32, tag="pw", name="pw")
                        for i in range(CT):
                            nc.tensor.matmul(out=kkp, lhsT=wfq[:, fr2, i, :],
                                             rhs=xk2[i],
                                             start=(i == 0), stop=(i == CT - 1))
                        rl = k2.tile([128, TTW], F32, tag="lnt1", name="lnt1")
                        nc.scalar.activation(out=rl, in_=kkp, func=AF.Relu)
                        kkf = k2.tile([128, TTW], BF16, tag="lnt2k", name="lnt2k")
                        nc.scalar.activation(out=kkf, in_=rl, func=AF.Square)
                        for m in range(4):
                            nc.tensor.matmul(out=accs[m],
                                             lhsT=wvq[:, fr2, m * 128:(m + 1) * 128],
                                             rhs=kkf, start=(f == 0),
                                             stop=(f == NFF - 1))
                    for m in range(4):
                        nc.vector.tensor_mul(out=sig[m], in0=sig[m], in1=accs[m])
                        nc.vector.tensor_add(out=xb[m][:, sl], in0=xb[m][:, sl],
                                             in1=sig[m])
                    # fire this half's exchange as soon as it is final: own
                    # slice tt lands (time-reversed) in the partner's half
                    # NTT-1-tt, so its AllGather overlaps the next tt's
                    # compute instead of serializing at the layer boundary.
                    oh = NTT - 1 - tt
                    for i in range(CT):
                        rv = k2.tile([128, TTW], F32, tag="revst", name="revst")
                        nc.vector.tensor_copy(
                            out=rv, in_=_revap(xb[i][:, sl].bitcast(F32)))
                        nc.sync.dma_start(out=sendh[oh][i * 128:(i + 1) * 128, :],
                                          in_=rv)
                    if solo:
                        for i in range(CT):
                            nc.sync.dma_start(
                                out=recvh[tt][i * 128:(i + 1) * 128, :],
                                in_=xb[i][:, sl].bitcast(F32))
                            nc.sync.dma_start(
                                out=recvh[tt][C + i * 128:C + (i + 1) * 128, :],
                                in_=xb[i][:, sl].bitcast(F32))
                    else:
                        nc.gpsimd.collective_compute(
                            "AllGather", OP.bypass, replica_groups=groups,
                            ins=[sendh[oh].opt()], outs=[recvh[oh].opt()])
                if dbg and l == 0:
                    for i in range(CT):
                        nc.sync.dma_start(out=dbg_o["p0"][i * 128:(i + 1) * 128, :],
                                          in_=xb[i].bitcast(F32))

                # ================= join =================
                # tt=NTT-1 first: its recv half was AllGathered right after
                # the first chanmix slice, so it never waits on the wire.
                for tt in reversed(range(NTT)):
                    sl = slice(tt * TTW, (tt + 1) * TTW)
                    # conv residue, all ct (fp32 accumulation, bf16 final for matmul)
                    cv32 = [kp.tile([128, TTW], F32, tag=f"ksb{i}", name=f"ksb{i}") for i in range(CT)]
                    cv = [k2.tile([128, TTW], BF16, tag=f"cvb{i}", name=f"cvb{i}") for i in range(CT)]
                    a = tt * TTW
                    for i in range(CT):
                        nc.vector.tensor_scalar_mul(out=cv32[i], in0=xres[i][:, sl],
                                                    scalar1=vcol(vecs, i, V_CW1))
                        lo = 1 if tt == 0 else 0
                        nc.vector.scalar_tensor_tensor(
                            out=cv32[i][:, lo:TTW],
                            in0=xres[i][:, a + lo - 1:a + TTW - 1],
                            scalar=vcol(vecs, i, V_CW0),
                            in1=cv32[i][:, lo:TTW], op0=OP.mult, op1=OP.add)
                        hi = TTW - 1 if tt == NTT - 1 else TTW
                        nc.vector.scalar_tensor_tensor(
                            out=cv[i][:, 0:hi],
                            in0=xres[i][:, a + 1:a + hi + 1],
                            scalar=vcol(vecs, i, V_CW2),
                            in1=cv32[i][:, 0:hi], op0=OP.mult, op1=OP.add)
                        if hi < TTW:
                            nc.scalar.activation(out=cv[i][:, hi:TTW],
                                                 in_=cv32[i][:, hi:TTW], func=AF.Copy)
                    accs = [psA.tile([128, TTW], F32, tag=f"acc{m}", name=f"acc{m}") for m in range(4)]
                    wt = load_wblk(cmrg[l][:, 512:1024], [128, CT, 512])
                    for i in range(CT):
                        for m in range(4):
                            nc.tensor.matmul(out=accs[m],
                                             lhsT=wt[:, i, m * 128:(m + 1) * 128],
                                             rhs=cv[i], start=(i == 0),
                                             stop=(i == CT - 1))
                    for m in range(4):
                        jr0 = kp.tile([128, TTW], F32, tag="vtm0", name="vtm0")
                        jr1 = kp.tile([128, TTW], F32, tag="vtm1", name="vtm1")
                        nc.sync.dma_start(out=jr0,
                                          in_=recvh[tt][m * 128:(m + 1) * 128, :])
                        nc.sync.dma_start(out=jr1,
                                          in_=recvh[tt][C + m * 128:C + (m + 1) * 128, :])
                        nc.vector.tensor_scalar_mul(
                            out=jr0, in0=jr0, scalar1=selt[:, S_ALPHA:S_ALPHA + 1])
                        nc.vector.scalar_tensor_tensor(
                            out=jr1, in0=jr1, scalar=selt[:, S_BETA:S_BETA + 1],
                            in1=jr0, op0=OP.mult, op1=OP.add)
                        sg = k2.tile([128, TTW], F32, tag="er0", name="er0", bufs=2)
                        nc.scalar.activation(out=sg, in_=accs[m], func=AF.Sigmoid,
                                             scale=selt[:, S_NEGS:S_NEGS + 1],
                                             bias=vcol(vecs, m, V_GBM))  # w_recv
                        wown = kp.tile([128, TTW], F32, tag="vtm3", name="vtm3")
                        nc.vector.tensor_scalar(out=wown, in0=sg, scalar1=-1.0,
                                                scalar2=1.0, op0=OP.mult, op1=OP.add)
                        jsum = kp.tile([128, TTW], F32, tag="vtm2", name="vtm2")
                        nc.vector.tensor_mul(out=jsum, in0=wown, in1=xb[m][:, sl])
                        nc.vector.tensor_mul(out=jr1, in0=sg, in1=jr1)
                        nc.vector.tensor_add(out=jsum, in0=jsum, in1=jr1)
                        # note: host mask05 already includes the 2x factor fold:
                        # mask05 = mask (not 0.5*mask) since sigmoid form used.
                        nc.vector.tensor_mul(out=xres[m][:, sl], in0=jsum,
                                             in1=maskt[:, sl])
            # ---- output ----
            for i in range(CT):
                nc.sync.dma_start(out=xout[i * 128:(i + 1) * 128, :],
                                  in_=xres[i].bitcast(F32))
    nc.compile()
    return nc


def _host_inputs(inputs):
    x = np.asarray(inputs["x"], np.float32)
    lengths = np.asarray(inputs["lengths"]).astype(np.int64)
    pos = np.arange(T, dtype=np.float32)[:, None]
    div = np.exp(np.arange(0, C, 2, dtype=np.float32) * (-np.log(10000.0) / C))
    pe = np.zeros((T, C), np.float32)
    pe[:, 0::2] = np.sin(pos * div)
    pe[:, 1::2] = np.cos(pos * div)
    mask = (np.arange(T)[None, :] < lengths[:, None]).astype(np.float32)

    consts = np.zeros((128, 384), np.float32)
    consts[:, 0:128] = np.eye(128, dtype=np.float32)
    consts[:, 128:256] = np.triu(np.ones((128, 128), np.float32), 1)
    consts[:, 256:384] = 1.0

    gw = np.asarray(inputs["gate_w"], np.float32)
    gb = np.asarray(inputs["gate_b"], np.float32)
    cw = np.asarray(inputs["conv_w"], np.float32)
    cb = np.asarray(inputs["conv_b"], np.float32)

    in_maps = []
    for c in range(8):
        b, d = c % 4, c // 4
        rev = d == 1
        s = -1.0 if rev else 1.0
        xin = (x[b] + pe)
        mrow = mask[b]
        if rev:
            xin = xin[::-1]
            mrow = mrow[::-1]
        m = {
            "x0": np.ascontiguousarray(xin.T),
            "mask05": np.ascontiguousarray(np.broadcast_to(mrow, (128, T))),
            "consts": consts,
        }
        sel = np.zeros((128, 8), np.float32)
        sel[:, S_LN0] = 0.0 if rev else 1.0
        sel[:, S_NEGS] = -s
        sel[:, S_ALPHA] = 1.0 if rev else 0.0
        sel[:, S_BETA] = 0.0 if rev else 1.0
        m["sel"] = sel
        for l in range(NL):
            W = {k: np.asarray(inputs[k], np.float32)[d, l]
                 for k in ["ln1_w", "ln1_b", "ln2_w", "ln2_b", "maa_x", "maa_w",
                           "maa_k", "maa_v", "maa_r", "maa_g", "tm_w1", "tm_w2",
                           "td_w1", "td_w2", "time_decay", "Wr", "Wk", "Wv",
                           "Wg", "Wo", "lnx_w", "lnx_b", "cm_maa_k", "cm_maa_r",
                           "cm_Wk", "cm_Wv", "cm_Wr", "time_faaaa"]}
            import ml_dtypes
            bf16 = ml_dtypes.bfloat16
            m[f"wbig{l}"] = np.ascontiguousarray(np.concatenate(
                [W["Wr"], W["Wk"], W["Wv"], W["Wg"], W["tm_w1"], W["td_w1"],
                 W["Wo"]], axis=1).astype(bf16))
            m[f"tmw2{l}"] = np.ascontiguousarray(
                W["tm_w2"].reshape(5 * TM, C).astype(bf16))
            m[f"tdw2{l}"] = np.ascontiguousarray(W["td_w2"].astype(bf16))
            m[f"cmk{l}"] = np.ascontiguousarray(
                W["cm_Wk"].reshape(C, NFF, 128).transpose(1, 0, 2).astype(bf16))
            m[f"cmv{l}"] = np.ascontiguousarray(W["cm_Wv"].astype(bf16))
            m[f"cmrg{l}"] = np.ascontiguousarray(
                np.concatenate([W["cm_Wr"], gw[l]], axis=1).astype(bf16))
            cwe = cw[l] if not rev else cw[l][:, ::-1]
            gbe = cb[l] @ gw[l] + gb[l]
            vec = np.zeros((C, NV), np.float32)
            vec[:, V_LN1W] = W["ln1_w"]; vec[:, V_LN1B] = W["ln1_b"]
            vec[:, V_LN2W] = W["ln2_w"]; vec[:, V_LN2B] = W["ln2_b"]
            vec[:, V_MAAX] = W["maa_x"]; vec[:, V_MAAW] = W["maa_w"]
            vec[:, V_MAAK] = W["maa_k"]; vec[:, V_MAAV] = W["maa_v"]
            vec[:, V_MAAR] = W["maa_r"]; vec[:, V_MAAG] = W["maa_g"]
            vec[:, V_TDCY] = W["time_decay"]
            vec[:, V_CMK] = W["cm_maa_k"]; vec[:, V_CMR] = W["cm_maa_r"]
            vec[:, V_GBM] = -s * gbe
            vec[:, V_CW0] = cwe[:, 0]
            vec[:, V_CW1] = cwe[:, 1] - 1.0
            vec[:, V_CW2] = cwe[:, 2]
            vec[:, V_LN0W] = np.asarray(inputs["ln0_w"], np.float32)
            vec[:, V_LN0B] = np.asarray(inputs["ln0_b"], np.float32)
            m[f"vecs{l}"] = vec
            lnx = np.zeros((128, 1024), np.float32)
            lnx[:, 0:512] = W["lnx_w"][None, :]
            lnx[:, 512:1024] = W["lnx_b"][None, :]
            m[f"lnx{l}"] = lnx
            u = W["time_faaaa"].reshape(C)
            hmu = np.zeros((C, 8), np.float32)
            for h in range(H):
                hmu[h * HN:(h + 1) * HN, h] = u[h * HN:(h + 1) * HN]
            m[f"hmu{l}"] = hmu.astype(bf16)
        in_maps.append(m)
    return in_maps


def kernel(**inputs):
    if "nc" not in _CACHE:
        _CACHE["nc"] = _build(dbg=False)
    nc = _CACHE["nc"]
    in_maps = _host_inputs(inputs)
    res = run_bass_kernel_spmd(nc, in_maps, list(range(8)))
    out = np.empty((B, T, C), np.float32)
    for b in range(B):
        out[b] = res.results[b]["xout"].T
    return out


if __name__ == "__main__":
    rng = np.random.default_rng(0)
    demo = None



# revision 50
# speedup vs baseline: 11.7649x; 1.0108x over previous
"""BiWKV6 encoder kernel for 8 Trainium2 NeuronCores.

Sharding: (batch, direction) -> 8 units, one per core; core c handles
batch c % 4, direction c // 4. Backward cores run the identical SPMD
program on time-reversed inputs; the only cross-core communication is a
pairwise AllGather of each layer's block output, written time-reversed
into the partner's domain. Within a core activations are channel-major
[C, T]; the WKV scan uses the chunked linear-attention formulation
(chunk 128) with log-space cumulative decay from the DVE prefix scan.
"""
import numpy as np

import concourse.bass as bass
import concourse.tile as tile
from concourse import bacc, mybir
from concourse.bass_utils import run_bass_kernel_spmd

B, T, C = 4, 1024, 512
H, HN = 8, 64
L = 128
TTW = 512
NTT = T // TTW
NCPT = TTW // L
CT = C // 128
TM, TD, FFN, NL = 32, 64, 1792, 2
NFF = FFN // 128
EPS_LN, EPS_GN = 1e-5, 64e-5
NV = 19

F32 = mybir.dt.float32
F32R = mybir.dt.float32r
BF16 = mybir.dt.bfloat16
AF = mybir.ActivationFunctionType
OP = mybir.AluOpType

WB_R, WB_K, WB_V, WB_G, WB_TM1, WB_TD1, WB_O = 0, 512, 1024, 1536, 2048, 2208, 2272
WB_COLS = 2784
(V_LN1W, V_LN1B, V_LN2W, V_LN2B, V_MAAX, V_MAAW, V_MAAK, V_MAAV, V_MAAR,
 V_MAAG, V_TDCY, V_CMK, V_CMR, V_GBM, V_CW0, V_CW1, V_CW2, V_LN0W,
 V_LN0B) = range(NV)
S_LN0, S_NEGS, S_ALPHA, S_BETA = 0, 1, 2, 3

_CACHE = {}


def _revap(ap):
    n = ap.ap[-1][1]
    return bass.AP(tensor=ap.tensor, offset=ap.offset + (n - 1) * ap.ap[-1][0],
                   ap=[ap.ap[0], [-ap.ap[-1][0], n]])


def _build(dbg=False, solo=False):
    nc = bacc.Bacc("TRN2", target_bir_lowering=False, debug=False, num_devices=8)

    x0 = nc.declare_dram_parameter("x0", [C, T], F32, isOutput=False)
    mask05 = nc.declare_dram_parameter("mask05", [128, T], F32, isOutput=False)
    sel_in = nc.declare_dram_parameter("sel", [128, 8], F32, isOutput=False)
    consts = nc.declare_dram_parameter("consts", [128, 384], F32, isOutput=False)
    wbig, tmw2, tdw2, cmkp, cmvp, cmrg, vecs_in, lnx_in, hmu_in = \
        [], [], [], [], [], [], [], [], []
    for l in range(NL):
        wbig.append(nc.declare_dram_parameter(f"wbig{l}", [C, WB_COLS], BF16, isOutput=False))
        tmw2.append(nc.declare_dram_parameter(f"tmw2{l}", [5 * TM, C], BF16, isOutput=False))
        tdw2.append(nc.declare_dram_parameter(f"tdw2{l}", [TD, C], BF16, isOutput=False))
        cmkp.append(nc.declare_dram_parameter(f"cmk{l}", [NFF, C, 128], BF16, isOutput=False))
        cmvp.append(nc.declare_dram_parameter(f"cmv{l}", [FFN, C], BF16, isOutput=False))
        cmrg.append(nc.declare_dram_parameter(f"cmrg{l}", [C, 1024], BF16, isOutput=False))
        vecs_in.append(nc.declare_dram_parameter(f"vecs{l}", [C, NV], F32, isOutput=False))
        lnx_in.append(nc.declare_dram_parameter(f"lnx{l}", [128, 1024], F32, isOutput=False))
        hmu_in.append(nc.declare_dram_parameter(f"hmu{l}", [C, 8], BF16, isOutput=False))
    xout = nc.declare_dram_parameter("xout", [C, T], F32, isOutput=True)
    dbg_o = {}
    if dbg:
        for nm in ["xt0", "r0", "lai0", "xbtm0", "p0", "w0"]:
            dbg_o[nm] = nc.declare_dram_parameter(nm, [C, T], F32, isOutput=True)
        for nm in ["y0tm", "g0tm"]:
            dbg_o[nm] = nc.declare_dram_parameter(nm, [T, C], F32, isOutput=True)

    groups = [[0, 4], [1, 5], [2, 6], [3, 7]]

    with tile.TileContext(nc) as tc:
        with (
            tc.tile_pool(name="pp", bufs=1) as pp,
            tc.tile_pool(name="wp", bufs=2) as wp,
            tc.tile_pool(name="kp", bufs=1) as kp,
            tc.tile_pool(name="k2", bufs=2) as k2,
            tc.tile_pool(name="psA", bufs=1, space="PSUM") as psA,
            tc.tile_pool(name="psB", bufs=2, space="PSUM") as psB,
            tc.tile_pool(name="psC", bufs=1, space="PSUM") as psC,
            tc.tile_pool(name="dp", bufs=2, space="DRAM") as dp,
        ):
            # ------------- persistent loads -------------
            xres = [pp.tile([128, T], F32R, tag=f"xres{i}", name=f"xres{i}") for i in range(CT)]
            xb = [pp.tile([128, T], F32R, tag=f"xb{i}", name=f"xb{i}") for i in range(CT)]
            for i in range(CT):
                nc.sync.dma_start(out=xres[i],
                                  in_=x0[i * 128:(i + 1) * 128, :].bitcast(F32R))
            maskt = pp.tile([128, T], F32, tag="mask", name="mask")
            nc.sync.dma_start(out=maskt, in_=mask05[:, :])
            selt = pp.tile([128, 8], F32, tag="sel", name="sel")
            nc.sync.dma_start(out=selt, in_=sel_in[:, :])
            cst = pp.tile([128, 384], F32, tag="consts", name="consts")
            nc.sync.dma_start(out=cst, in_=consts[:, :])
            eps_ln_t = pp.tile([128, 1], F32, tag="epsln", name="epsln")
            nc.vector.memset(eps_ln_t, EPS_LN)
            eps_gn_t = pp.tile([128, 1], F32, tag="epsgn", name="epsgn")
            nc.vector.memset(eps_gn_t, EPS_GN)
            ident = cst[:, 0:128]
            triu = cst[:, 128:256]
            identb = pp.tile([128, 128], BF16, tag="identb", name="identb")
            nc.scalar.activation(out=identb, in_=ident, func=AF.Copy)
            onesr_t = pp.tile([128, 128], F32R, tag="onesr", name="onesr")
            nc.sync.dma_start(out=onesr_t, in_=consts[:, 256:384].bitcast(F32R))
            ones_r = onesr_t

            def vcol(vt, i, j):
                return vt[i][:, j:j + 1]

            def load_w(dram_ap, shape, tag, bufs=2):
                t = wp.tile(shape, BF16, tag=tag, name="wld", bufs=bufs)
                nc.sync.dma_start(out=t, in_=dram_ap)
                return t

            def load_wblk(dram_2d, shape):
                # one DMA for a [C, w] weight block -> SBUF [128, CT, w]
                t = wp.tile(shape, BF16, tag="wblk", name="wblk", bufs=2)
                nc.sync.dma_start(
                    out=t, in_=dram_2d.rearrange("(k p) n -> p k n", p=128))
                return t

            def ln_stats(src_sl):
                ssum = psC.tile([1, TTW], F32, tag="stA", name="stA")
                ssq = psC.tile([1, TTW], F32, tag="stB", name="stB")
                for i in range(CT):
                    sq = k2.tile([128, TTW], F32R, tag="lnt1", name="lnt1")
                    nc.vector.tensor_mul(out=sq, in0=src_sl[i], in1=src_sl[i])
                    nc.tensor.matmul(out=ssum, lhsT=ones_r[:, 0:1], rhs=src_sl[i],
                                     start=(i == 0), stop=(i == CT - 1))
                    nc.tensor.matmul(out=ssq, lhsT=ones_r[:, 0:1], rhs=sq,
                                     start=(i == 0), stop=(i == CT - 1))
                rows = k2.tile([128, TTW], F32, tag="lnrows", name="lnrows", bufs=1)
                srow, s2, varu, lnv = (rows[j:j + 1, :] for j in (0, 32, 64, 96))
                nc.scalar.activation(out=srow, in_=ssum, func=AF.Copy)
                nc.vector.tensor_mul(out=s2, in0=srow, in1=srow)
                nc.vector.scalar_tensor_tensor(out=varu, in0=s2, scalar=-1.0 / C,
                                               in1=ssq, op0=OP.mult, op1=OP.add)
                nc.scalar.activation(out=lnv, in_=varu, func=AF.Ln, scale=1.0 / C,
                                     bias=eps_ln_t[0:1, :])
                rs = k2.tile([1, TTW], F32R, tag="lnrs", name="lnrs", bufs=1)
                nc.scalar.activation(out=rs, in_=lnv, func=AF.Exp, scale=-0.5)
                murs = k2.tile([1, TTW], F32R, tag="lnmu", name="lnmu", bufs=1)
                nc.vector.scalar_tensor_tensor(out=murs, in0=srow, scalar=1.0 / C,
                                               in1=rs, op0=OP.mult, op1=OP.mult)
                bc0 = psB.tile([128, TTW], F32, tag="pw", name="pw")
                nc.tensor.matmul(out=bc0, lhsT=ones_r[0:1, 0:128], rhs=rs,
                                 start=True, stop=True)
                bc1 = psB.tile([128, TTW], F32, tag="pw", name="pw")
                nc.tensor.matmul(out=bc1, lhsT=ones_r[0:1, 0:128], rhs=murs,
                                 start=True, stop=True)
                return bc0, bc1

            def ln_apply(src_sl, bc0, bc1, vt, wi, bi, out_tiles):
                for i in range(CT):
                    t1 = k2.tile([128, TTW], F32, tag="lnt1", name="lnt1")
                    nc.vector.tensor_mul(out=t1, in0=src_sl[i], in1=bc0)
                    t2 = k2.tile([128, TTW], F32, tag="lnt2", name="lnt2")
                    nc.vector.tensor_sub(out=t2, in0=t1, in1=bc1)
                    nc.vector.tensor_scalar(out=out_tiles[i], in0=t2,
                                            scalar1=vcol(vt, i, wi),
                                            scalar2=vcol(vt, i, bi),
                                            op0=OP.mult, op1=OP.add)

            def tanh_route(psum_ap, out_tile):
                nc.scalar.activation(out=out_tile, in_=psum_ap, func=AF.Tanh)

            # ================= layers =================
            for l in range(NL):
                vecs = []
                for i in range(CT):
                    vt = pp.tile([128, NV], F32, tag=f"vecs{i}", name=f"vecs{i}")
                    nc.sync.dma_start(out=vt, in_=vecs_in[l][i * 128:(i + 1) * 128, :])
                    vecs.append(vt)
                lnxt = pp.tile([128, 1024], F32, tag="lnx", name="lnx")
                nc.sync.dma_start(out=lnxt, in_=lnx_in[l][:, :])
                hmu = []
                for i in range(CT):
                    ht = pp.tile([128, 8], BF16, tag=f"hmu{i}", name=f"hmu{i}")
                    nc.sync.dma_start(out=ht, in_=hmu_in[l][i * 128:(i + 1) * 128, :])
                    hmu.append(ht)

                # ---- xb init ----
                if l == 0:
                    for tt in range(NTT):
                        sl = slice(tt * TTW, (tt + 1) * TTW)
                        src = [xres[i][:, sl] for i in range(CT)]
                        bc0, bc1 = ln_stats(src)
                        xbs = [xb[i][:, sl] for i in range(CT)]
                        ln_apply(src, bc0, bc1, vecs, V_LN0W, V_LN0B, xbs)
                        for i in range(CT):
                            d = k2.tile([128, TTW], F32, tag="lnt2", name="lnt2")
                            nc.vector.tensor_sub(out=d, in0=xb[i][:, sl],
                                                 in1=xres[i][:, sl])
                            nc.vector.scalar_tensor_tensor(
                                out=xb[i][:, sl], in0=d,
                                scalar=selt[:, S_LN0:S_LN0 + 1],
                                in1=xres[i][:, sl], op0=OP.mult, op1=OP.add)
                else:
                    for i in range(CT):
                        nc.gpsimd.tensor_copy(out=xb[i], in_=xres[i])

                S_cur = [pp.tile([128, HN], BF16, tag=f"S{i}", name=f"S{i}") for i in range(CT)]
                for i in range(CT):
                    nc.vector.memset(S_cur[i], 0.0)
                carry = [pp.tile([128, 1], F32, tag=f"ca{i}", name=f"ca{i}") for i in range(CT)]
                carry2 = [pp.tile([128, 1], F32, tag=f"cb{i}", name=f"cb{i}") for i in range(CT)]
                for i in range(CT):
                    nc.gpsimd.memset(carry[i], 0.0)
                    nc.gpsimd.memset(carry2[i], 0.0)

                # ================= time mix =================
                for tt in range(NTT):
                    sl = slice(tt * TTW, (tt + 1) * TTW)
                    # prefetch ALL projection weights (R,K,V,G are contiguous
                    # columns) in one DMA: it lands while ln/t5 compute runs,
                    # so no wmm ever stalls on a weight load.
                    wtA = wp.tile([128, CT, 2048], BF16, tag="wblkA", name="wblkA",
                                  bufs=1)
                    nc.sync.dma_start(
                        out=wtA, in_=wbig[l][:, 0:2048]
                        .rearrange("(k p) n -> p k n", p=128))
                    xbs = [xb[i][:, sl] for i in range(CT)]
                    bc0, bc1 = ln_stats(xbs)
                    xt = [kp.tile([128, TTW], F32, tag=f"xt{i}", name=f"xt{i}") for i in range(CT)]
                    ln_apply(xbs, bc0, bc1, vecs, V_LN1W, V_LN1B, xt)
                    xx = [kp.tile([128, TTW], F32, tag=f"xx{i}", name=f"xx{i}") for i in range(CT)]
                    for i in range(CT):
                        nc.vector.tensor_sub(out=xx[i][:, 1:TTW],
                                             in0=xt[i][:, 0:TTW - 1],
                                             in1=xt[i][:, 1:TTW])
                        nc.vector.scalar_tensor_tensor(
                            out=xx[i][:, 0:1], in0=carry[i], scalar=1.0,
                            in1=xt[i][:, 0:1], op0=OP.mult, op1=OP.subtract)
                        nc.gpsimd.tensor_copy(out=carry[i], in_=xt[i][:, TTW - 1:TTW])
                    if dbg and l == 0:
                        for i in range(CT):
                            nc.sync.dma_start(out=dbg_o["xt0"][i * 128:(i + 1) * 128, sl],
                                              in_=xt[i])

                    # ---- t5 ----
                    mx = [k2.tile([128, TTW], BF16, tag=f"xf{i}", name=f"xf{i}") for i in range(CT)]
                    for i in range(CT):
                        nc.vector.scalar_tensor_tensor(
                            out=mx[i], in0=xx[i], scalar=vcol(vecs, i, V_MAAX),
                            in1=xt[i], op0=OP.mult, op1=OP.add)
                    p160a = psC.tile([128, TTW], F32, tag="stA", name="stA")
                    p160b = psC.tile([32, TTW], F32, tag="stB", name="stB")
                    wtmtd = wp.tile([128, CT, 224], BF16, tag="wtmtd", name="wtmtd",
                                    bufs=1)
                    nc.sync.dma_start(
                        out=wtmtd,
                        in_=wbig[l][:, WB_TM1:WB_TM1 + 224]
                        .rearrange("(k p) n -> p k n", p=128))
                    for i in range(CT):
                        nc.tensor.matmul(out=p160a, lhsT=wtmtd[:, i, 0:128], rhs=mx[i],
                                         start=(i == 0), stop=(i == CT - 1))
                        nc.tensor.matmul(out=p160b, lhsT=wtmtd[:, i, 128:160], rhs=mx[i],
                                         start=(i == 0), stop=(i == CT - 1))
                    t5 = [k2.tile([32, TTW], BF16, tag=f"t5{f}", name=f"t5{f}", bufs=1) for f in range(5)]
                    for f in range(4):
                        tanh_route(p160a[f * 32:(f + 1) * 32, :], t5[f])
                    tanh_route(p160b, t5[4])

                    def build_xf(fidx, maa_i):
                        w2 = load_w(tmw2[l][fidx * TM:(fidx + 1) * TM, :],
                                    [TM, C], "wtm2")
                        xft = []
                        for i in range(CT):
                            dlp = psB.tile([128, TTW], F32, tag="pw", name="pw")
                            nc.tensor.matmul(out=dlp,
                                             lhsT=w2[:, i * 128:(i + 1) * 128],
                                             rhs=t5[fidx], start=True, stop=True)
                            a = k2.tile([128, TTW], F32, tag="lnt1", name="lnt1")
                            nc.vector.scalar_tensor_tensor(
                                out=a, in0=dlp, scalar=vcol(vecs, i, maa_i),
                                in1=xx[i], op0=OP.add, op1=OP.mult)
                            xf = k2.tile([128, TTW], BF16, tag=f"xf{i}", name=f"xf{i}")
                            nc.vector.tensor_add(out=xf, in0=a, in1=xt[i])
                            xft.append(xf)
                        return xft

                    def wmm(col_off, xft):
                        accs = [psA.tile([128, TTW], F32, tag=f"acc{m}",
                                         name=f"acc{m}") for m in range(4)]
                        for i in range(CT):
                            for m in range(4):
                                nc.tensor.matmul(
                                    out=accs[m],
                                    lhsT=wtA[:, i, col_off + m * 128:
                                             col_off + (m + 1) * 128],
                                    rhs=xft[i], start=(i == 0),
                                    stop=(i == CT - 1))
                        return accs

                    def wmm_tm(col_off, xft):
                        accs = [psA.tile([128, TTW], F32, tag=f"acc{m}",
                                         name=f"acc{m}") for m in range(4)]
                        for i in range(CT):
                            for ci in range(NCPT):
                                nc.tensor.matmul(
                                    out=accs[ci],
                                    lhsT=xft[i][:, ci * L:(ci + 1) * L],
                                    rhs=wtA[:, i, col_off:col_off + 512],
                                    start=(i == 0), stop=(i == CT - 1))
                        return accs

                    # w -> wacc -> lai FIRST: the four serial prefix scans run
                    # on the vector engine while the tensor engine chews
                    # through the k/v/r/g projections below.
                    xf = build_xf(0, V_MAAW)
                    tdp = psC.tile([TD, TTW], F32, tag="stA", name="stA")
                    for i in range(CT):
                        nc.tensor.matmul(out=tdp, lhsT=wtmtd[:, i, 160:224], rhs=xf[i],
                                         start=(i == 0), stop=(i == CT - 1))
                    tdt = k2.tile([TD, TTW], BF16, tag="tdt", name="tdt", bufs=1)
                    tanh_route(tdp, tdt)
                    w2t = load_w(tdw2[l][:, :], [TD, C], "wtd2", bufs=1)
                    lai = [kp.tile([128, 1 + TTW], F32, tag=f"lai{i}", name=f"lai{i}") for i in range(CT)]
                    for i in range(CT):
                        wwp = psB.tile([128, TTW], F32, tag="pw", name="pw")
                        nc.tensor.matmul(out=wwp, lhsT=w2t[:, i * 128:(i + 1) * 128],
                                         rhs=tdt, start=True, stop=True)
                        wacc = k2.tile([128, TTW], F32, tag="lnt1", name="lnt1")
                        nc.scalar.activation(out=wacc, in_=wwp, func=AF.Exp,
                                             bias=vcol(vecs, i, V_TDCY))
                        nc.gpsimd.memset(lai[i][:, 0:1], 0.0)
                        nc.vector.tensor_tensor_scan(
                            out=lai[i][:, 1:1 + TTW], data0=wacc, data1=wacc,
                            initial=0.0, op0=OP.add, op1=OP.bypass)
                        if dbg and l == 0:
                            nc.sync.dma_start(
                                out=dbg_o["w0"][i * 128:(i + 1) * 128, sl], in_=wacc)
                            nc.sync.dma_start(
                                out=dbg_o["lai0"][i * 128:(i + 1) * 128, sl],
                                in_=lai[i][:, 1:1 + TTW])
                    # k
                    xf = build_xf(1, V_MAAK)
                    accs = wmm(WB_K, xf)
                    k_sb = [kp.tile([128, TTW], F32, tag=f"ksb{i}", name=f"ksb{i}") for i in range(CT)]
                    for m in range(4):
                        nc.scalar.activation(out=k_sb[m], in_=accs[m], func=AF.Copy)
                    # v token-major
                    xf = build_xf(2, V_MAAV)
                    accs = wmm_tm(WB_V, xf)
                    v_tm = [kp.tile([128, C], F32, tag=f"vtm{ci}", name=f"vtm{ci}") for ci in range(NCPT)]
                    v_tmb = [kp.tile([128, C], BF16, tag=f"vtb{ci}", name=f"vtb{ci}") for ci in range(NCPT)]
                    for ci in range(NCPT):
                        nc.scalar.activation(out=v_tm[ci], in_=accs[ci], func=AF.Copy)
                        nc.scalar.activation(out=v_tmb[ci], in_=accs[ci], func=AF.Copy)
                    # r
                    xf = build_xf(3, V_MAAR)
                    accs = wmm(WB_R, xf)
                    r_sb = [kp.tile([128, TTW], F32, tag=f"rsb{i}", name=f"rsb{i}") for i in range(CT)]
                    for m in range(4):
                        nc.scalar.activation(out=r_sb[m], in_=accs[m], func=AF.Copy)
                    if dbg and l == 0:
                        for i in range(CT):
                            nc.sync.dma_start(out=dbg_o["r0"][i * 128:(i + 1) * 128, sl],
                                              in_=r_sb[i])
                    # g token-major, silu
                    xf = build_xf(4, V_MAAG)
                    accs = wmm_tm(WB_G, xf)
                    g_tm = [kp.tile([128, C], F32, tag=f"gtm{ci}", name=f"gtm{ci}") for ci in range(NCPT)]
                    for ci in range(NCPT):
                        nc.scalar.activation(out=g_tm[ci], in_=accs[ci], func=AF.Silu)

                    # ---- wkv chunks ----
                    # prefetch Wo weights so the post-chunk matmuls never wait
                    wto = load_wblk(wbig[l][:, WB_O:WB_O + 512], [128, CT, 512])
                    ztc = [kp.tile([128, TTW], BF16, tag=f"ztc{i}", name=f"ztc{i}") for i in range(CT)]
                    for ci in range(NCPT):
                        gc = tt * NCPT + ci
                        c0 = ci * L
                        fpc, fnc, rt_t, kt_t, kh_tm, m_t = [], [], [], [], [], []
                        for i in range(CT):
                            ngc = k2.tile([128, 1], F32, tag="ngc", name="ngc", bufs=4)
                            nc.vector.tensor_scalar_mul(out=ngc,
                                                        in0=lai[i][:, c0:c0 + 1],
                                                        scalar1=-1.0)
                            fp = k2.tile([128, 1 + L], F32, tag="fp", name="fp", bufs=4)
                            nc.scalar.activation(out=fp, in_=lai[i][:, c0:c0 + 1 + L],
                                                 func=AF.Exp, bias=ngc)
                            fn = k2.tile([128, 1 + L], F32, tag="fn", name="fn", bufs=4)
                            nc.scalar.activation(out=fn, in_=lai[i][:, c0:c0 + 1 + L],
                                                 func=AF.Exp, scale=-1.0,
                                                 bias=lai[i][:, c0:c0 + 1])
                            fpc.append(fp)
                            fnc.append(fn)
                            rt = k2.tile([128, L], BF16, tag="rt", name="rt", bufs=4)
                            nc.vector.tensor_mul(out=rt, in0=r_sb[i][:, c0:c0 + L],
                                                 in1=fn[:, 0:L])
                            kt = k2.tile([128, L], BF16, tag="kt", name="kt", bufs=4)
                            nc.vector.tensor_mul(out=kt, in0=k_sb[i][:, c0:c0 + L],
                                                 in1=fp[:, 1:1 + L])
                            kh = k2.tile([128, L], F32, tag="kh", name="kh", bufs=4)
                            nc.vector.tensor_scalar_mul(out=kh, in0=kt,
                                                        scalar1=fn[:, L:L + 1])
                            mt = k2.tile([128, L], BF16, tag="mt", name="mt", bufs=4)
                            nc.vector.tensor_mul(out=mt, in0=r_sb[i][:, c0:c0 + L],
                                                 in1=k_sb[i][:, c0:c0 + L])
                            rt_t.append(rt)
                            kt_t.append(kt)
                            m_t.append(mt)
                            trp = psB.tile([128, L], F32, tag="pw", name="pw")
                            nc.tensor.transpose(out=trp, in_=kh, identity=ident)
                            kht = k2.tile([128, L], BF16, tag="khtm", name="khtm", bufs=4)
                            nc.scalar.activation(out=kht, in_=trp, func=AF.Copy)
                            kh_tm.append(kht)
                        dall = psC.tile([128, 8], F32, tag="stB", name="stB")
                        for i in range(CT):
                            nc.tensor.matmul(out=dall, lhsT=m_t[i], rhs=hmu[i],
                                             start=(i == 0), stop=(i == CT - 1))
                        yps = psA.tile([128, C], F32, tag="acc0", name="acc0")
                        S_new = [k2.tile([128, HN], BF16, tag=f"Sn{i}", name=f"Sn{i}") for i in range(CT)]
                        for i in range(CT):
                            sup = psC.tile([128, HN], F32, tag="stA", name="stA")
                            for hh in range(2):
                                h = 2 * i + hh
                                hb = hh * HN
                                pt = psB.tile([L, L], F32, tag="pw", name="pw")
                                nc.tensor.matmul(out=pt, lhsT=kt_t[i][hb:hb + HN, :],
                                                 rhs=rt_t[i][hb:hb + HN, :],
                                                 start=True, stop=True)
                                pts = k2.tile([L, L], BF16, tag="pts", name="pts")
                                nc.vector.tensor_mul(out=pts, in0=pt, in1=triu)
                                nc.tensor.matmul(
                                    out=yps[:, h * HN:(h + 1) * HN], lhsT=pts,
                                    rhs=v_tmb[ci][:, h * HN:(h + 1) * HN],
                                    start=True, stop=(gc == 0), skip_group_check=True)
                                if gc > 0:
                                    nc.tensor.matmul(
                                        out=yps[:, h * HN:(h + 1) * HN],
                                        lhsT=rt_t[i][hb:hb + HN, :],
                                        rhs=S_cur[i][hb:hb + HN, :],
                                        start=False, stop=True, skip_group_check=True)
                                nc.tensor.matmul(
                                    out=sup[hb:hb + HN, :],
                                    lhsT=kh_tm[i][:, hb:hb + HN],
                                    rhs=v_tmb[ci][:, h * HN:(h + 1) * HN],
                                    start=True, stop=True, skip_group_check=True)
                            t0 = k2.tile([128, HN], F32, tag="ssc", name="ssc", bufs=4)
                            nc.vector.tensor_scalar_mul(out=t0, in0=S_cur[i],
                                                        scalar1=fnc[i][:, L:L + 1])
                            nc.vector.tensor_add(out=S_new[i], in0=t0, in1=sup)
                        S_cur = S_new
                        ysb = k2.tile([128, C], F32, tag="ysb", name="ysb")
                        for h in range(H):
                            nc.vector.scalar_tensor_tensor(
                                out=ysb[:, h * HN:(h + 1) * HN],
                                in0=v_tm[ci][:, h * HN:(h + 1) * HN],
                                scalar=dall[:, h:h + 1],
                                in1=yps[:, h * HN:(h + 1) * HN],
                                op0=OP.mult, op1=OP.add)
                        if dbg and l == 0:
                            nc.sync.dma_start(
                                out=dbg_o["y0tm"][tt * TTW + c0:tt * TTW + c0 + L, :],
                                in_=ysb)
                            nc.sync.dma_start(
                                out=dbg_o["g0tm"][tt * TTW + c0:tt * TTW + c0 + L, :],
                                in_=g_tm[ci])
                        # groupnorm + affine + *g  (token-major)
                        mv = k2.tile([128, 16], F32, tag="gnmv", name="gnmv")
                        for h in range(H):
                            st = k2.tile([128, 6], F32, tag="gnst", name="gnst")
                            nc.vector.bn_stats(out=st, in_=ysb[:, h * HN:(h + 1) * HN])
                            nc.vector.bn_aggr(out=mv[:, 2 * h:2 * h + 2], in_=st)
                        lnv = k2.tile([128, 8], F32, tag="gnln", name="gnln")
                        var_view = bass.AP(tensor=mv.tensor, offset=mv.offset + 1,
                                           ap=[mv.ap[0], [2, 8]])
                        nc.scalar.activation(out=lnv, in_=var_view, func=AF.Ln,
                                             bias=eps_gn_t)
                        rsg = k2.tile([128, 8], F32, tag="gnrs", name="gnrs")
                        nc.scalar.activation(out=rsg, in_=lnv, func=AF.Exp, scale=-0.5)
                        for h in range(H):
                            nc.vector.tensor_scalar(
                                out=ysb[:, h * HN:(h + 1) * HN],
                                in0=ysb[:, h * HN:(h + 1) * HN],
                                scalar1=mv[:, 2 * h:2 * h + 1],
                                scalar2=rsg[:, h:h + 1],
                                op0=OP.subtract, op1=OP.mult)
                        nc.vector.tensor_mul(out=ysb, in0=ysb, in1=lnxt[:, 0:512])
                        nc.vector.tensor_add(out=ysb, in0=ysb, in1=lnxt[:, 512:1024])
                        nc.vector.tensor_mul(out=ysb, in0=ysb, in1=g_tm[ci])
                        for i in range(CT):
                            trp = psB.tile([128, L], F32, tag="pw", name="pw")
                            nc.tensor.transpose(out=trp,
                                                in_=ysb[:, i * 128:(i + 1) * 128],
                                                identity=ident)
                            nc.scalar.activation(out=ztc[i][:, c0:c0 + L], in_=trp,
                                                 func=AF.Copy)
                    # ---- Wo ----
                    accs = [psA.tile([128, TTW], F32, tag=f"acc{m}",
                                     name=f"acc{m}") for m in range(4)]
                    for i in range(CT):
                        for m in range(4):
                            nc.tensor.matmul(out=accs[m],
                                             lhsT=wto[:, i, m * 128:(m + 1) * 128],
                                             rhs=ztc[i], start=(i == 0),
                                             stop=(i == CT - 1))
                    for m in range(4):
                        nc.vector.tensor_add(out=xb[m][:, sl], in0=xb[m][:, sl],
                                             in1=accs[m])
                    if dbg and l == 0:
                        for i in range(CT):
                            nc.sync.dma_start(
                                out=dbg_o["xbtm0"][i * 128:(i + 1) * 128, sl],
                                in_=xb[i][:, sl].bitcast(F32))

                # ================= channel mix =================
                sendh = [dp.tile([C, TTW], F32, tag=f"send{h}", name=f"send{h}")
                         for h in range(NTT)]
                recvh = [dp.tile([2 * C, TTW], F32, tag=f"recv{h}", name=f"recv{h}")
                         for h in range(NTT)]
                for tt in range(NTT):
                    sl = slice(tt * TTW, (tt + 1) * TTW)
                    # prefetch cm_Wr weights under the ln/shift compute
                    wtr = load_wblk(cmrg[l][:, 0:512], [128, CT, 512])
                    xbs = [xb[i][:, sl] for i in range(CT)]
                    bc0, bc1 = ln_stats(xbs)
                    xc = [kp.tile([128, TTW], F32, tag=f"xt{i}", name=f"xt{i}") for i in range(CT)]
                    ln_apply(xbs, bc0, bc1, vecs, V_LN2W, V_LN2B, xc)
                    xx2 = [kp.tile([128, TTW], F32, tag=f"xx{i}", name=f"xx{i}") for i in range(CT)]
                    for i in range(CT):
                        nc.vector.tensor_sub(out=xx2[i][:, 1:TTW],
                                             in0=xc[i][:, 0:TTW - 1],
                                             in1=xc[i][:, 1:TTW])
                        nc.vector.scalar_tensor_tensor(
                            out=xx2[i][:, 0:1], in0=carry2[i], scalar=1.0,
                            in1=xc[i][:, 0:1], op0=OP.mult, op1=OP.subtract)
                        nc.gpsimd.tensor_copy(out=carry2[i], in_=xc[i][:, TTW - 1:TTW])
                    xk2 = [kp.tile([128, TTW], BF16, tag=f"xk2{i}", name=f"xk2{i}") for i in range(CT)]
                    xr2 = [kp.tile([128, TTW], BF16, tag=f"xr2{i}", name=f"xr2{i}") for i in range(CT)]
                    for i in range(CT):
                        nc.vector.scalar_tensor_tensor(
                            out=xk2[i], in0=xx2[i], scalar=vcol(vecs, i, V_CMK),
                            in1=xc[i], op0=OP.mult, op1=OP.add)
                        nc.vector.scalar_tensor_tensor(
                            out=xr2[i], in0=xx2[i], scalar=vcol(vecs, i, V_CMR),
                            in1=xc[i], op0=OP.mult, op1=OP.add)
                    # cm_Wr -> sigmoid
                    accs = [psA.tile([128, TTW], F32, tag=f"acc{m}", name=f"acc{m}") for m in range(4)]
                    for i in range(CT):
                        for m in range(4):
                            nc.tensor.matmul(out=accs[m],
                                             lhsT=wtr[:, i, m * 128:(m + 1) * 128],
                                             rhs=xr2[i], start=(i == 0),
                                             stop=(i == CT - 1))
                    sig = [kp.tile([128, TTW], F32, tag=f"gtm{m}", name=f"sig{m}") for m in range(4)]
                    for m in range(4):
                        nc.scalar.activation(out=sig[m], in_=accs[m], func=AF.Sigmoid)
                    # kk loop with cm_Wv accumulation
                    accs = [psA.tile([128, TTW], F32, tag=f"acc{m}", name=f"acc{m}") for m in range(4)]
                    wfq = wvq = None
                    for f in range(NFF):
                        fq, fr2 = f // 4, f % 4
                        nq = min(4, NFF - 4 * fq)
                        if fr2 == 0:
                            wfq = wp.tile([128, nq, CT, 128], BF16, tag="wblk",
                                          name="wfq", bufs=2)
                            nc.sync.dma_start(
                                out=wfq,
                                in_=cmkp[l][4 * fq:4 * fq + nq]
                                .rearrange("f (k p) n -> p f k n", p=128))
                            wvq = wp.tile([128, nq, C], BF16, tag="wblk",
                                          name="wvq", bufs=2)
                            nc.sync.dma_start(
                                out=wvq,
                                in_=cmvp[l][4 * fq * 128:(4 * fq + nq) * 128, :]
                                .rearrange("(f p) n -> p f n", p=128))
                        kkp = psB.tile([128, TTW], F32, tag="pw", name="pw")
                        for i in range(CT):
                            nc.tensor.matmul(out=kkp, lhsT=wfq[:, fr2, i, :],
                                             rhs=xk2[i],
                                             start=(i == 0), stop=(i == CT - 1))
                        rl = k2.tile([128, TTW], F32, tag="lnt1", name="lnt1")
                        nc.scalar.activation(out=rl, in_=kkp, func=AF.Relu)
                        kkf = k2.tile([128, TTW], BF16, tag="lnt2k", name="lnt2k")
                        nc.scalar.activation(out=kkf, in_=rl, func=AF.Square)
                        for m in range(4):
                            nc.tensor.matmul(out=accs[m],
                                             lhsT=wvq[:, fr2, m * 128:(m + 1) * 128],
                                             rhs=kkf, start=(f == 0),
                                             stop=(f == NFF - 1))
                    for m in range(4):
                        nc.vector.tensor_mul(out=sig[m], in0=sig[m], in1=accs[m])
                        nc.vector.tensor_add(out=xb[m][:, sl], in0=xb[m][:, sl],
                                             in1=sig[m])
                    # fire this half's exchange as soon as it is final: own
                    # slice tt lands (time-reversed) in the partner's half
                    # NTT-1-tt, so its AllGather overlaps the next tt's
                    # compute instead of serializing at the layer boundary.
                    oh = NTT - 1 - tt
                    for i in range(CT):
                        rv = k2.tile([128, TTW], F32, tag="revst", name="revst")
                        nc.vector.tensor_copy(
                            out=rv, in_=_revap(xb[i][:, sl].bitcast(F32)))
                        nc.sync.dma_start(out=sendh[oh][i * 128:(i + 1) * 128, :],
                                          in_=rv)
                    if solo:
                        for i in range(CT):
                            nc.sync.dma_start(
                                out=recvh[tt][i * 128:(i + 1) * 128, :],
                                in_=xb[i][:, sl].bitcast(F32))
                            nc.sync.dma_start(
                                out=recvh[tt][C + i * 128:C + (i + 1) * 128, :],
                                in_=xb[i][:, sl].bitcast(F32))
                    else:
                        nc.gpsimd.collective_compute(
                            "AllGather", OP.bypass, replica_groups=groups,
                            ins=[sendh[oh].opt()], outs=[recvh[oh].opt()])
                if dbg and l == 0:
                    for i in range(CT):
                        nc.sync.dma_start(out=dbg_o["p0"][i * 128:(i + 1) * 128, :],
                                          in_=xb[i].bitcast(F32))

                # ================= join =================
                # tt=NTT-1 first: its recv half was AllGathered right after
                # the first chanmix slice, so it never waits on the wire.
                for tt in reversed(range(NTT)):
                    sl = slice(tt * TTW, (tt + 1) * TTW)
                    # conv residue, all ct (fp32 accumulation, bf16 final for matmul)
                    cv32 = [kp.tile([128, TTW], F32, tag=f"ksb{i}", name=f"ksb{i}") for i in range(CT)]
                    cv = [k2.tile([128, TTW], BF16, tag=f"cvb{i}", name=f"cvb{i}") for i in range(CT)]
                    a = tt * TTW
                    for i in range(CT):
                        nc.vector.tensor_scalar_mul(out=cv32[i], in0=xres[i][:, sl],
                                                    scalar1=vcol(vecs, i, V_CW1))
                        lo = 1 if tt == 0 else 0
                        nc.vector.scalar_tensor_tensor(
                            out=cv32[i][:, lo:TTW],
                            in0=xres[i][:, a + lo - 1:a + TTW - 1],
                            scalar=vcol(vecs, i, V_CW0),
                            in1=cv32[i][:, lo:TTW], op0=OP.mult, op1=OP.add)
                        hi = TTW - 1 if tt == NTT - 1 else TTW
                        nc.vector.scalar_tensor_tensor(
                            out=cv[i][:, 0:hi],
                            in0=xres[i][:, a + 1:a + hi + 1],
                            scalar=vcol(vecs, i, V_CW2),
                            in1=cv32[i][:, 0:hi], op0=OP.mult, op1=OP.add)
                        if hi < TTW:
                            nc.scalar.activation(out=cv[i][:, hi:TTW],
                                                 in_=cv32[i][:, hi:TTW], func=AF.Copy)
                    accs = [psA.tile([128, TTW], F32, tag=f"acc{m}", name=f"acc{m}") for m in range(4)]
                    wt = load_wblk(cmrg[l][:, 512:1024], [128, CT, 512])
                    for i in range(CT):
                        for m in range(4):
                            nc.tensor.matmul(out=accs[m],
                                             lhsT=wt[:, i, m * 128:(m + 1) * 128],
                                             rhs=cv[i], start=(i == 0),
                                             stop=(i == CT - 1))
                    for m in range(4):
                        jr0 = kp.tile([128, TTW], F32, tag="vtm0", name="vtm0")
                        jr1 = kp.tile([128, TTW], F32, tag="vtm1", name="vtm1")
                        nc.sync.dma_start(out=jr0,
                                          in_=recvh[tt][m * 128:(m + 1) * 128, :])
                        nc.sync.dma_start(out=jr1,
                                          in_=recvh[tt][C + m * 128:C + (m + 1) * 128, :])
                        nc.vector.tensor_scalar_mul(
                            out=jr0, in0=jr0, scalar1=selt[:, S_ALPHA:S_ALPHA + 1])
                        nc.vector.scalar_tensor_tensor(
                            out=jr1, in0=jr1, scalar=selt[:, S_BETA:S_BETA + 1],
                            in1=jr0, op0=OP.mult, op1=OP.add)
                        sg = k2.tile([128, TTW], F32, tag="er0", name="er0", bufs=2)
                        nc.scalar.activation(out=sg, in_=accs[m], func=AF.Sigmoid,
                                             scale=selt[:, S_NEGS:S_NEGS + 1],
                                             bias=vcol(vecs, m, V_GBM))  # w_recv
                        wown = kp.tile([128, TTW], F32, tag="vtm3", name="vtm3")
                        nc.vector.tensor_scalar(out=wown, in0=sg, scalar1=-1.0,
                                                scalar2=1.0, op0=OP.mult, op1=OP.add)
                        jsum = kp.tile([128, TTW], F32, tag="vtm2", name="vtm2")
                        nc.vector.tensor_mul(out=jsum, in0=wown, in1=xb[m][:, sl])
                        nc.vector.tensor_mul(out=jr1, in0=sg, in1=jr1)
                        nc.vector.tensor_add(out=jsum, in0=jsum, in1=jr1)
                        # note: host mask05 already includes the 2x factor fold:
                        # mask05 = mask (not 0.5*mask) since sigmoid form used.
                        nc.vector.tensor_mul(out=xres[m][:, sl], in0=jsum,
                                             in1=maskt[:, sl])
            # ---- output ----
            for i in range(CT):
                nc.sync.dma_start(out=xout[i * 128:(i + 1) * 128, :],
                                  in_=xres[i].bitcast(F32))
    nc.compile()
    return nc


def _host_inputs(inputs):
    x = np.asarray(inputs["x"], np.float32)
    lengths = np.asarray(inputs["lengths"]).astype(np.int64)
    pos = np.arange(T, dtype=np.float32)[:, None]
    div = np.exp(np.arange(0, C, 2, dtype=np.float32) * (-np.log(10000.0) / C))
    pe = np.zeros((T, C), np.float32)
    pe[:, 0::2] = np.sin(pos * div)
    pe[:, 1::2] = np.cos(pos * div)
    mask = (np.arange(T)[None, :] < lengths[:, None]).astype(np.float32)

    consts = np.zeros((128, 384), np.float32)
    consts[:, 0:128] = np.eye(128, dtype=np.float32)
    consts[:, 128:256] = np.triu(np.ones((128, 128), np.float32), 1)
    consts[:, 256:384] = 1.0

    gw = np.asarray(inputs["gate_w"], np.float32)
    gb = np.asarray(inputs["gate_b"], np.float32)
    cw = np.asarray(inputs["conv_w"], np.float32)
    cb = np.asarray(inputs["conv_b"], np.float32)

    in_maps = []
    for c in range(8):
        b, d = c % 4, c // 4
        rev = d == 1
        s = -1.0 if rev else 1.0
        xin = (x[b] + pe)
        mrow = mask[b]
        if rev:
            xin = xin[::-1]
            mrow = mrow[::-1]
        m = {
            "x0": np.ascontiguousarray(xin.T),
            "mask05": np.ascontiguousarray(np.broadcast_to(mrow, (128, T))),
            "consts": consts,
        }
        sel = np.zeros((128, 8), np.float32)
        sel[:, S_LN0] = 0.0 if rev else 1.0
        sel[:, S_NEGS] = -s
        sel[:, S_ALPHA] = 1.0 if rev else 0.0
        sel[:, S_BETA] = 0.0 if rev else 1.0
        m["sel"] = sel
        for l in range(NL):
            W = {k: np.asarray(inputs[k], np.float32)[d, l]
                 for k in ["ln1_w", "ln1_b", "ln2_w", "ln2_b", "maa_x", "maa_w",
                           "maa_k", "maa_v", "maa_r", "maa_g", "tm_w1", "tm_w2",
                           "td_w1", "td_w2", "time_decay", "Wr", "Wk", "Wv",
                           "Wg", "Wo", "lnx_w", "lnx_b", "cm_maa_k", "cm_maa_r",
                           "cm_Wk", "cm_Wv", "cm_Wr", "time_faaaa"]}
            import ml_dtypes
            bf16 = ml_dtypes.bfloat16
            m[f"wbig{l}"] = np.ascontiguousarray(np.concatenate(
                [W["Wr"], W["Wk"], W["Wv"], W["Wg"], W["tm_w1"], W["td_w1"],
                 W["Wo"]], axis=1).astype(bf16))
            m[f"tmw2{l}"] = np.ascontiguousarray(
                W["tm_w2"].reshape(5 * TM, C).astype(bf16))
            m[f"tdw2{l}"] = np.ascontiguousarray(W["td_w2"].astype(bf16))
            m[f"cmk{l}"] = np.ascontiguousarray(
                W["cm_Wk"].reshape(C, NFF, 128).transpose(1, 0, 2).astype(bf16))
            m[f"cmv{l}"] = np.ascontiguousarray(W["cm_Wv"].astype(bf16))
            m[f"cmrg{l}"] = np.ascontiguousarray(
                np.concatenate([W["cm_Wr"], gw[l]], axis=1).astype(bf16))
            cwe = cw[l] if not rev else cw[l][:, ::-1]
            gbe = cb[l] @ gw[l] + gb[l]
            vec = np.zeros((C, NV), np.float32)
            vec[:, V_LN1W] = W["ln1_w"]; vec[:, V_LN1B] = W["ln1_b"]
            vec[:, V_LN2W] = W["ln2_w"]; vec[:, V_LN2B] = W["ln2_b"]
            vec[:, V_MAAX] = W["maa_x"]; vec[:, V_MAAW] = W["maa_w"]
            vec[:, V_MAAK] = W["maa_k"]; vec[:, V_MAAV] = W["maa_v"]
            vec[:, V_MAAR] = W["maa_r"]; vec[:, V_MAAG] = W["maa_g"]
            vec[:, V_TDCY] = W["time_decay"]
            vec[:, V_CMK] = W["cm_maa_k"]; vec[:, V_CMR] = W["cm_maa_r"]
            vec[:, V_GBM] = -s * gbe
            vec[:, V_CW0] = cwe[:, 0]
            vec[:, V_CW1] = cwe[:, 1] - 1.0
            vec[:, V_CW2] = cwe[:, 2]
            vec[:, V_LN0W] = np.asarray(inputs["ln0_w"], np.float32)
            vec[:, V_LN0B] = np.asarray(inputs["ln0_b"], np.float32)
            m[f"vecs{l}"] = vec
            lnx = np.zeros((128, 1024), np.float32)
            lnx[:, 0:512] = W["lnx_w"][None, :]
            lnx[:, 512:1024] = W["lnx_b"][None, :]
            m[f"lnx{l}"] = lnx
            u = W["time_faaaa"].reshape(C)
            hmu = np.zeros((C, 8), np.float32)
            for h in range(H):
                hmu[h * HN:(h + 1) * HN, h] = u[h * HN:(h + 1) * HN]
            m[f"hmu{l}"] = hmu.astype(bf16)
        in_maps.append(m)
    return in_maps


def kernel(**inputs):
    if "nc" not in _CACHE:
        _CACHE["nc"] = _build(dbg=False)
    nc = _CACHE["nc"]
    in_maps = _host_inputs(inputs)
    res = run_bass_kernel_spmd(nc, in_maps, list(range(8)))
    out = np.empty((B, T, C), np.float32)
    for b in range(B):
        out[b] = res.results[b]["xout"].T
    return out


if __name__ == "__main__":
    rng = np.random.default_rng(0)
    demo = None



# revision 54
# speedup vs baseline: 11.9913x; 1.0192x over previous
"""BiWKV6 encoder kernel for 8 Trainium2 NeuronCores.

Sharding: (batch, direction) -> 8 units, one per core; core c handles
batch c % 4, direction c // 4. Backward cores run the identical SPMD
program on time-reversed inputs; the only cross-core communication is a
pairwise AllGather of each layer's block output, written time-reversed
into the partner's domain. Within a core activations are channel-major
[C, T]; the WKV scan uses the chunked linear-attention formulation
(chunk 128) with log-space cumulative decay from the DVE prefix scan.
"""
import numpy as np

import concourse.bass as bass
import concourse.tile as tile
from concourse import bacc, mybir
from concourse.bass_utils import run_bass_kernel_spmd

B, T, C = 4, 1024, 512
H, HN = 8, 64
L = 128
TTW = 512
NTT = T // TTW
NCPT = TTW // L
CT = C // 128
TM, TD, FFN, NL = 32, 64, 1792, 2
NFF = FFN // 128
EPS_LN, EPS_GN = 1e-5, 64e-5
NV = 19

F32 = mybir.dt.float32
F32R = mybir.dt.float32r
BF16 = mybir.dt.bfloat16
AF = mybir.ActivationFunctionType
OP = mybir.AluOpType

WB_R, WB_K, WB_V, WB_G, WB_TM1, WB_TD1, WB_O = 0, 512, 1024, 1536, 2048, 2208, 2272
WB_COLS = 2784
(V_LN1W, V_LN1B, V_LN2W, V_LN2B, V_MAAX, V_MAAW, V_MAAK, V_MAAV, V_MAAR,
 V_MAAG, V_TDCY, V_CMK, V_CMR, V_GBM, V_CW0, V_CW1, V_CW2, V_LN0W,
 V_LN0B) = range(NV)
S_LN0, S_NEGS, S_ALPHA, S_BETA = 0, 1, 2, 3

_CACHE = {}


def _revap(ap):
    n = ap.ap[-1][1]
    return bass.AP(tensor=ap.tensor, offset=ap.offset + (n - 1) * ap.ap[-1][0],
                   ap=[ap.ap[0], [-ap.ap[-1][0], n]])


def _build(dbg=False, solo=False):
    nc = bacc.Bacc("TRN2", target_bir_lowering=False, debug=False, num_devices=8)

    x0 = nc.declare_dram_parameter("x0", [C, T], F32, isOutput=False)
    mask05 = nc.declare_dram_parameter("mask05", [128, T], F32, isOutput=False)
    sel_in = nc.declare_dram_parameter("sel", [128, 8], F32, isOutput=False)
    consts = nc.declare_dram_parameter("consts", [128, 384], F32, isOutput=False)
    wbig, tmw2, tdw2, cmkp, cmvp, cmrg, vecs_in, lnx_in, hmu_in = \
        [], [], [], [], [], [], [], [], []
    for l in range(NL):
        wbig.append(nc.declare_dram_parameter(f"wbig{l}", [C, WB_COLS], BF16, isOutput=False))
        tmw2.append(nc.declare_dram_parameter(f"tmw2{l}", [5 * TM, C], BF16, isOutput=False))
        tdw2.append(nc.declare_dram_parameter(f"tdw2{l}", [TD, C], BF16, isOutput=False))
        cmkp.append(nc.declare_dram_parameter(f"cmk{l}", [NFF, C, 128], BF16, isOutput=False))
        cmvp.append(nc.declare_dram_parameter(f"cmv{l}", [FFN, C], BF16, isOutput=False))
        cmrg.append(nc.declare_dram_parameter(f"cmrg{l}", [C, 1024], BF16, isOutput=False))
        vecs_in.append(nc.declare_dram_parameter(f"vecs{l}", [C, NV], F32, isOutput=False))
        lnx_in.append(nc.declare_dram_parameter(f"lnx{l}", [128, 1024], F32, isOutput=False))
        hmu_in.append(nc.declare_dram_parameter(f"hmu{l}", [C, 8], BF16, isOutput=False))
    xout = nc.declare_dram_parameter("xout", [C, T], F32, isOutput=True)
    dbg_o = {}
    if dbg:
        for nm in ["xt0", "r0", "lai0", "xbtm0", "p0", "w0"]:
            dbg_o[nm] = nc.declare_dram_parameter(nm, [C, T], F32, isOutput=True)
        for nm in ["y0tm", "g0tm"]:
            dbg_o[nm] = nc.declare_dram_parameter(nm, [T, C], F32, isOutput=True)

    groups = [[0, 4], [1, 5], [2, 6], [3, 7]]

    with tile.TileContext(nc) as tc:
        with (
            tc.tile_pool(name="pp", bufs=1) as pp,
            tc.tile_pool(name="wp", bufs=2) as wp,
            tc.tile_pool(name="kp", bufs=1) as kp,
            tc.tile_pool(name="k2", bufs=2) as k2,
            tc.tile_pool(name="psA", bufs=1, space="PSUM") as psA,
            tc.tile_pool(name="psB", bufs=2, space="PSUM") as psB,
            tc.tile_pool(name="psC", bufs=1, space="PSUM") as psC,
            tc.tile_pool(name="dp", bufs=2, space="DRAM") as dp,
        ):
            # ------------- persistent loads -------------
            xres = [pp.tile([128, T], F32R, tag=f"xres{i}", name=f"xres{i}") for i in range(CT)]
            xb = [pp.tile([128, T], F32R, tag=f"xb{i}", name=f"xb{i}") for i in range(CT)]
            for i in range(CT):
                nc.sync.dma_start(out=xres[i],
                                  in_=x0[i * 128:(i + 1) * 128, :].bitcast(F32R))
            maskt = pp.tile([128, T], F32, tag="mask", name="mask")
            nc.sync.dma_start(out=maskt, in_=mask05[:, :])
            selt = pp.tile([128, 8], F32, tag="sel", name="sel")
            nc.sync.dma_start(out=selt, in_=sel_in[:, :])
            cst = pp.tile([128, 384], F32, tag="consts", name="consts")
            nc.sync.dma_start(out=cst, in_=consts[:, :])
            eps_ln_t = pp.tile([128, 1], F32, tag="epsln", name="epsln")
            nc.vector.memset(eps_ln_t, EPS_LN)
            eps_gn_t = pp.tile([128, 1], F32, tag="epsgn", name="epsgn")
            nc.vector.memset(eps_gn_t, EPS_GN)
            ident = cst[:, 0:128]
            triu = cst[:, 128:256]
            identb = pp.tile([128, 128], BF16, tag="identb", name="identb")
            nc.scalar.activation(out=identb, in_=ident, func=AF.Copy)
            onesr_t = pp.tile([128, 128], F32R, tag="onesr", name="onesr")
            nc.sync.dma_start(out=onesr_t, in_=consts[:, 256:384].bitcast(F32R))
            ones_r = onesr_t

            def vcol(vt, i, j):
                return vt[i][:, j:j + 1]

            def load_w(dram_ap, shape, tag, bufs=2):
                t = wp.tile(shape, BF16, tag=tag, name="wld", bufs=bufs)
                nc.sync.dma_start(out=t, in_=dram_ap)
                return t

            def load_wblk(dram_2d, shape):
                # one DMA for a [C, w] weight block -> SBUF [128, CT, w]
                t = wp.tile(shape, BF16, tag="wblk", name="wblk", bufs=2)
                nc.sync.dma_start(
                    out=t, in_=dram_2d.rearrange("(k p) n -> p k n", p=128))
                return t

            def ln_stats(src_sl):
                ssum = psC.tile([1, TTW], F32, tag="stA", name="stA")
                ssq = psC.tile([1, TTW], F32, tag="stB", name="stB")
                for i in range(CT):
                    sq = k2.tile([128, TTW], F32R, tag="lnt1", name="lnt1")
                    nc.vector.tensor_mul(out=sq, in0=src_sl[i], in1=src_sl[i])
                    nc.tensor.matmul(out=ssum, lhsT=ones_r[:, 0:1], rhs=src_sl[i],
                                     start=(i == 0), stop=(i == CT - 1))
                    nc.tensor.matmul(out=ssq, lhsT=ones_r[:, 0:1], rhs=sq,
                                     start=(i == 0), stop=(i == CT - 1))
                rows = k2.tile([128, TTW], F32, tag="lnrows", name="lnrows", bufs=1)
                srow, s2, varu, lnv = (rows[j:j + 1, :] for j in (0, 32, 64, 96))
                nc.scalar.activation(out=srow, in_=ssum, func=AF.Copy)
                nc.vector.tensor_mul(out=s2, in0=srow, in1=srow)
                nc.vector.scalar_tensor_tensor(out=varu, in0=s2, scalar=-1.0 / C,
                                               in1=ssq, op0=OP.mult, op1=OP.add)
                nc.scalar.activation(out=lnv, in_=varu, func=AF.Ln, scale=1.0 / C,
                                     bias=eps_ln_t[0:1, :])
                rs = k2.tile([1, TTW], F32R, tag="lnrs", name="lnrs", bufs=1)
                nc.scalar.activation(out=rs, in_=lnv, func=AF.Exp, scale=-0.5)
                murs = k2.tile([1, TTW], F32R, tag="lnmu", name="lnmu", bufs=1)
                nc.vector.scalar_tensor_tensor(out=murs, in0=srow, scalar=1.0 / C,
                                               in1=rs, op0=OP.mult, op1=OP.mult)
                bc0 = psB.tile([128, TTW], F32, tag="pw", name="pw")
                nc.tensor.matmul(out=bc0, lhsT=ones_r[0:1, 0:128], rhs=rs,
                                 start=True, stop=True)
                bc1 = psB.tile([128, TTW], F32, tag="pw", name="pw")
                nc.tensor.matmul(out=bc1, lhsT=ones_r[0:1, 0:128], rhs=murs,
                                 start=True, stop=True)
                return bc0, bc1

            def ln_apply(src_sl, bc0, bc1, vt, wi, bi, out_tiles):
                for i in range(CT):
                    t1 = k2.tile([128, TTW], F32, tag="lnt1", name="lnt1")
                    nc.vector.tensor_mul(out=t1, in0=src_sl[i], in1=bc0)
                    t2 = k2.tile([128, TTW], F32, tag="lnt2", name="lnt2")
                    nc.vector.tensor_sub(out=t2, in0=t1, in1=bc1)
                    nc.vector.tensor_scalar(out=out_tiles[i], in0=t2,
                                            scalar1=vcol(vt, i, wi),
                                            scalar2=vcol(vt, i, bi),
                                            op0=OP.mult, op1=OP.add)

            def tanh_route(psum_ap, out_tile):
                nc.scalar.activation(out=out_tile, in_=psum_ap, func=AF.Tanh)

            # ================= layers =================
            for l in range(NL):
                vecs = []
                for i in range(CT):
                    vt = pp.tile([128, NV], F32, tag=f"vecs{i}", name=f"vecs{i}")
                    nc.sync.dma_start(out=vt, in_=vecs_in[l][i * 128:(i + 1) * 128, :])
                    vecs.append(vt)
                lnxt = pp.tile([128, 1024], F32, tag="lnx", name="lnx")
                nc.sync.dma_start(out=lnxt, in_=lnx_in[l][:, :])
                hmu = []
                for i in range(CT):
                    ht = pp.tile([128, 8], BF16, tag=f"hmu{i}", name=f"hmu{i}")
                    nc.sync.dma_start(out=ht, in_=hmu_in[l][i * 128:(i + 1) * 128, :])
                    hmu.append(ht)

                # ---- xb init ----
                if l == 0:
                    for tt in range(NTT):
                        sl = slice(tt * TTW, (tt + 1) * TTW)
                        src = [xres[i][:, sl] for i in range(CT)]
                        bc0, bc1 = ln_stats(src)
                        xbs = [xb[i][:, sl] for i in range(CT)]
                        ln_apply(src, bc0, bc1, vecs, V_LN0W, V_LN0B, xbs)
                        for i in range(CT):
                            d = k2.tile([128, TTW], F32, tag="lnt2", name="lnt2")
                            nc.vector.tensor_sub(out=d, in0=xb[i][:, sl],
                                                 in1=xres[i][:, sl])
                            nc.vector.scalar_tensor_tensor(
                                out=xb[i][:, sl], in0=d,
                                scalar=selt[:, S_LN0:S_LN0 + 1],
                                in1=xres[i][:, sl], op0=OP.mult, op1=OP.add)
                else:
                    for i in range(CT):
                        nc.gpsimd.tensor_copy(out=xb[i], in_=xres[i])

                S_cur = [pp.tile([128, HN], BF16, tag=f"S{i}", name=f"S{i}") for i in range(CT)]
                for i in range(CT):
                    nc.vector.memset(S_cur[i], 0.0)
                carry = [pp.tile([128, 1], F32, tag=f"ca{i}", name=f"ca{i}") for i in range(CT)]
                carry2 = [pp.tile([128, 1], F32, tag=f"cb{i}", name=f"cb{i}") for i in range(CT)]
                for i in range(CT):
                    nc.gpsimd.memset(carry[i], 0.0)
                    nc.gpsimd.memset(carry2[i], 0.0)

                # ================= time mix =================
                for tt in range(NTT):
                    sl = slice(tt * TTW, (tt + 1) * TTW)
                    # prefetch ALL projection weights (R,K,V,G are contiguous
                    # columns) in one DMA: it lands while ln/t5 compute runs,
                    # so no wmm ever stalls on a weight load.
                    wtA = wp.tile([128, CT, 2048], BF16, tag="wblkA", name="wblkA",
                                  bufs=1)
                    nc.sync.dma_start(
                        out=wtA, in_=wbig[l][:, 0:2048]
                        .rearrange("(k p) n -> p k n", p=128))
                    xbs = [xb[i][:, sl] for i in range(CT)]
                    bc0, bc1 = ln_stats(xbs)
                    xt = [kp.tile([128, TTW], F32, tag=f"xt{i}", name=f"xt{i}") for i in range(CT)]
                    ln_apply(xbs, bc0, bc1, vecs, V_LN1W, V_LN1B, xt)
                    xx = [kp.tile([128, TTW], F32, tag=f"xx{i}", name=f"xx{i}") for i in range(CT)]
                    for i in range(CT):
                        nc.vector.tensor_sub(out=xx[i][:, 1:TTW],
                                             in0=xt[i][:, 0:TTW - 1],
                                             in1=xt[i][:, 1:TTW])
                        nc.vector.scalar_tensor_tensor(
                            out=xx[i][:, 0:1], in0=carry[i], scalar=1.0,
                            in1=xt[i][:, 0:1], op0=OP.mult, op1=OP.subtract)
                        nc.gpsimd.tensor_copy(out=carry[i], in_=xt[i][:, TTW - 1:TTW])
                    if dbg and l == 0:
                        for i in range(CT):
                            nc.sync.dma_start(out=dbg_o["xt0"][i * 128:(i + 1) * 128, sl],
                                              in_=xt[i])

                    # ---- t5 ----
                    mx = [k2.tile([128, TTW], BF16, tag=f"xf{i}", name=f"xf{i}") for i in range(CT)]
                    for i in range(CT):
                        nc.vector.scalar_tensor_tensor(
                            out=mx[i], in0=xx[i], scalar=vcol(vecs, i, V_MAAX),
                            in1=xt[i], op0=OP.mult, op1=OP.add)
                    p160a = psC.tile([128, TTW], F32, tag="stA", name="stA")
                    p160b = psC.tile([32, TTW], F32, tag="stB", name="stB")
                    wtmtd = wp.tile([128, CT, 224], BF16, tag="wtmtd", name="wtmtd",
                                    bufs=1)
                    nc.sync.dma_start(
                        out=wtmtd,
                        in_=wbig[l][:, WB_TM1:WB_TM1 + 224]
                        .rearrange("(k p) n -> p k n", p=128))
                    for i in range(CT):
                        nc.tensor.matmul(out=p160a, lhsT=wtmtd[:, i, 0:128], rhs=mx[i],
                                         start=(i == 0), stop=(i == CT - 1))
                        nc.tensor.matmul(out=p160b, lhsT=wtmtd[:, i, 128:160], rhs=mx[i],
                                         start=(i == 0), stop=(i == CT - 1))
                    t5 = [k2.tile([32, TTW], BF16, tag=f"t5{f}", name=f"t5{f}", bufs=1) for f in range(5)]
                    for f in range(4):
                        tanh_route(p160a[f * 32:(f + 1) * 32, :], t5[f])
                    tanh_route(p160b, t5[4])

                    def build_xf(fidx, maa_i):
                        w2 = load_w(tmw2[l][fidx * TM:(fidx + 1) * TM, :],
                                    [TM, C], "wtm2")
                        xft = []
                        for i in range(CT):
                            dlp = psB.tile([128, TTW], F32, tag="pw", name="pw")
                            nc.tensor.matmul(out=dlp,
                                             lhsT=w2[:, i * 128:(i + 1) * 128],
                                             rhs=t5[fidx], start=True, stop=True)
                            a = k2.tile([128, TTW], F32, tag="lnt1", name="lnt1")
                            nc.vector.scalar_tensor_tensor(
                                out=a, in0=dlp, scalar=vcol(vecs, i, maa_i),
                                in1=xx[i], op0=OP.add, op1=OP.mult)
                            xf = k2.tile([128, TTW], BF16, tag=f"xf{i}", name=f"xf{i}")
                            nc.vector.tensor_add(out=xf, in0=a, in1=xt[i])
                            xft.append(xf)
                        return xft

                    def wmm(col_off, xft):
                        accs = [psA.tile([128, TTW], F32, tag=f"acc{m}",
                                         name=f"acc{m}") for m in range(4)]
                        for i in range(CT):
                            for m in range(4):
                                nc.tensor.matmul(
                                    out=accs[m],
                                    lhsT=wtA[:, i, col_off + m * 128:
                                             col_off + (m + 1) * 128],
                                    rhs=xft[i], start=(i == 0),
                                    stop=(i == CT - 1))
                        return accs

                    def wmm_tm(col_off, xft):
                        accs = [psA.tile([128, TTW], F32, tag=f"acc{m}",
                                         name=f"acc{m}") for m in range(4)]
                        for i in range(CT):
                            for ci in range(NCPT):
                                nc.tensor.matmul(
                                    out=accs[ci],
                                    lhsT=xft[i][:, ci * L:(ci + 1) * L],
                                    rhs=wtA[:, i, col_off:col_off + 512],
                                    start=(i == 0), stop=(i == CT - 1))
                        return accs

                    # w -> wacc -> lai FIRST: the four serial prefix scans run
                    # on the vector engine while the tensor engine chews
                    # through the k/v/r/g projections below.
                    xf = build_xf(0, V_MAAW)
                    tdp = psC.tile([TD, TTW], F32, tag="stA", name="stA")
                    for i in range(CT):
                        nc.tensor.matmul(out=tdp, lhsT=wtmtd[:, i, 160:224], rhs=xf[i],
                                         start=(i == 0), stop=(i == CT - 1))
                    tdt = k2.tile([TD, TTW], BF16, tag="tdt", name="tdt", bufs=1)
                    tanh_route(tdp, tdt)
                    w2t = load_w(tdw2[l][:, :], [TD, C], "wtd2", bufs=1)
                    lai = [kp.tile([128, 1 + TTW], F32, tag=f"lai{i}", name=f"lai{i}") for i in range(CT)]
                    for i in range(CT):
                        wwp = psB.tile([128, TTW], F32, tag="pw", name="pw")
                        nc.tensor.matmul(out=wwp, lhsT=w2t[:, i * 128:(i + 1) * 128],
                                         rhs=tdt, start=True, stop=True)
                        wacc = k2.tile([128, TTW], F32, tag="lnt1", name="lnt1")
                        nc.scalar.activation(out=wacc, in_=wwp, func=AF.Exp,
                                             bias=vcol(vecs, i, V_TDCY))
                        nc.gpsimd.memset(lai[i][:, 0:1], 0.0)
                        nc.vector.tensor_tensor_scan(
                            out=lai[i][:, 1:1 + TTW], data0=wacc, data1=wacc,
                            initial=0.0, op0=OP.add, op1=OP.bypass)
                        if dbg and l == 0:
                            nc.sync.dma_start(
                                out=dbg_o["w0"][i * 128:(i + 1) * 128, sl], in_=wacc)
                            nc.sync.dma_start(
                                out=dbg_o["lai0"][i * 128:(i + 1) * 128, sl],
                                in_=lai[i][:, 1:1 + TTW])
                    # k
                    xf = build_xf(1, V_MAAK)
                    accs = wmm(WB_K, xf)
                    k_sb = [kp.tile([128, TTW], F32, tag=f"ksb{i}", name=f"ksb{i}") for i in range(CT)]
                    for m in range(4):
                        nc.scalar.activation(out=k_sb[m], in_=accs[m], func=AF.Copy)
                    # v token-major
                    xf = build_xf(2, V_MAAV)
                    accs = wmm_tm(WB_V, xf)
                    v_tm = [kp.tile([128, C], F32, tag=f"vtm{ci}", name=f"vtm{ci}") for ci in range(NCPT)]
                    v_tmb = [kp.tile([128, C], BF16, tag=f"vtb{ci}", name=f"vtb{ci}") for ci in range(NCPT)]
                    for ci in range(NCPT):
                        nc.scalar.activation(out=v_tm[ci], in_=accs[ci], func=AF.Copy)
                        nc.scalar.activation(out=v_tmb[ci], in_=accs[ci], func=AF.Copy)
                    # r
                    xf = build_xf(3, V_MAAR)
                    accs = wmm(WB_R, xf)
                    r_sb = [kp.tile([128, TTW], F32, tag=f"rsb{i}", name=f"rsb{i}") for i in range(CT)]
                    for m in range(4):
                        nc.scalar.activation(out=r_sb[m], in_=accs[m], func=AF.Copy)
                    if dbg and l == 0:
                        for i in range(CT):
                            nc.sync.dma_start(out=dbg_o["r0"][i * 128:(i + 1) * 128, sl],
                                              in_=r_sb[i])
                    # g token-major, silu
                    xf = build_xf(4, V_MAAG)
                    accs = wmm_tm(WB_G, xf)
                    g_tm = [kp.tile([128, C], F32, tag=f"gtm{ci}", name=f"gtm{ci}") for ci in range(NCPT)]
                    for ci in range(NCPT):
                        nc.scalar.activation(out=g_tm[ci], in_=accs[ci], func=AF.Silu)

                    # ---- wkv chunks ----
                    # prefetch Wo weights so the post-chunk matmuls never wait
                    wto = load_wblk(wbig[l][:, WB_O:WB_O + 512], [128, CT, 512])
                    ztc = [kp.tile([128, TTW], BF16, tag=f"ztc{i}", name=f"ztc{i}") for i in range(CT)]
                    for ci in range(NCPT):
                        gc = tt * NCPT + ci
                        c0 = ci * L
                        fpc, fnc, rt_t, kt_t, kh_tm, m_t = [], [], [], [], [], []
                        for i in range(CT):
                            ngc = k2.tile([128, 1], F32, tag="ngc", name="ngc", bufs=4)
                            nc.vector.tensor_scalar_mul(out=ngc,
                                                        in0=lai[i][:, c0:c0 + 1],
                                                        scalar1=-1.0)
                            fp = k2.tile([128, 1 + L], F32, tag="fp", name="fp", bufs=4)
                            nc.scalar.activation(out=fp, in_=lai[i][:, c0:c0 + 1 + L],
                                                 func=AF.Exp, bias=ngc)
                            fn = k2.tile([128, 1 + L], F32, tag="fn", name="fn", bufs=4)
                            nc.scalar.activation(out=fn, in_=lai[i][:, c0:c0 + 1 + L],
                                                 func=AF.Exp, scale=-1.0,
                                                 bias=lai[i][:, c0:c0 + 1])
                            fpc.append(fp)
                            fnc.append(fn)
                            rt = k2.tile([128, L], BF16, tag="rt", name="rt", bufs=4)
                            nc.vector.tensor_mul(out=rt, in0=r_sb[i][:, c0:c0 + L],
                                                 in1=fn[:, 0:L])
                            kt = k2.tile([128, L], BF16, tag="kt", name="kt", bufs=4)
                            nc.vector.tensor_mul(out=kt, in0=k_sb[i][:, c0:c0 + L],
                                                 in1=fp[:, 1:1 + L])
                            kh = k2.tile([128, L], F32, tag="kh", name="kh", bufs=4)
                            nc.vector.tensor_scalar_mul(out=kh, in0=kt,
                                                        scalar1=fn[:, L:L + 1])
                            mt = k2.tile([128, L], BF16, tag="mt", name="mt", bufs=4)
                            nc.vector.tensor_mul(out=mt, in0=r_sb[i][:, c0:c0 + L],
                                                 in1=k_sb[i][:, c0:c0 + L])
                            rt_t.append(rt)
                            kt_t.append(kt)
                            m_t.append(mt)
                            trp = psB.tile([128, L], F32, tag="pw", name="pw")
                            nc.tensor.transpose(out=trp, in_=kh, identity=ident)
                            kht = k2.tile([128, L], BF16, tag="khtm", name="khtm", bufs=4)
                            nc.scalar.activation(out=kht, in_=trp, func=AF.Copy)
                            kh_tm.append(kht)
                        dall = psC.tile([128, 8], F32, tag="stB", name="stB")
                        for i in range(CT):
                            nc.tensor.matmul(out=dall, lhsT=m_t[i], rhs=hmu[i],
                                             start=(i == 0), stop=(i == CT - 1))
                        yps = psA.tile([128, C], F32, tag="acc0", name="acc0")
                        S_new = [k2.tile([128, HN], BF16, tag=f"Sn{i}", name=f"Sn{i}") for i in range(CT)]
                        for i in range(CT):
                            sup = psC.tile([128, HN], F32, tag="stA", name="stA")
                            for hh in range(2):
                                h = 2 * i + hh
                                hb = hh * HN
                                pt = psB.tile([L, L], F32, tag="pw", name="pw")
                                nc.tensor.matmul(out=pt, lhsT=kt_t[i][hb:hb + HN, :],
                                                 rhs=rt_t[i][hb:hb + HN, :],
                                                 start=True, stop=True)
                                pts = k2.tile([L, L], BF16, tag="pts", name="pts")
                                nc.vector.tensor_mul(out=pts, in0=pt, in1=triu)
                                nc.tensor.matmul(
                                    out=yps[:, h * HN:(h + 1) * HN], lhsT=pts,
                                    rhs=v_tmb[ci][:, h * HN:(h + 1) * HN],
                                    start=True, stop=(gc == 0), skip_group_check=True)
                                if gc > 0:
                                    nc.tensor.matmul(
                                        out=yps[:, h * HN:(h + 1) * HN],
                                        lhsT=rt_t[i][hb:hb + HN, :],
                                        rhs=S_cur[i][hb:hb + HN, :],
                                        start=False, stop=True, skip_group_check=True)
                                nc.tensor.matmul(
                                    out=sup[hb:hb + HN, :],
                                    lhsT=kh_tm[i][:, hb:hb + HN],
                                    rhs=v_tmb[ci][:, h * HN:(h + 1) * HN],
                                    start=True, stop=True, skip_group_check=True)
                            t0 = k2.tile([128, HN], F32, tag="ssc", name="ssc", bufs=4)
                            nc.vector.tensor_scalar_mul(out=t0, in0=S_cur[i],
                                                        scalar1=fnc[i][:, L:L + 1])
                            nc.vector.tensor_add(out=S_new[i], in0=t0, in1=sup)
                        S_cur = S_new
                        ysb = k2.tile([128, C], F32, tag="ysb", name="ysb")
                        for h in range(H):
                            nc.vector.scalar_tensor_tensor(
                                out=ysb[:, h * HN:(h + 1) * HN],
                                in0=v_tm[ci][:, h * HN:(h + 1) * HN],
                                scalar=dall[:, h:h + 1],
                                in1=yps[:, h * HN:(h + 1) * HN],
                                op0=OP.mult, op1=OP.add)
                        if dbg and l == 0:
                            nc.sync.dma_start(
                                out=dbg_o["y0tm"][tt * TTW + c0:tt * TTW + c0 + L, :],
                                in_=ysb)
                            nc.sync.dma_start(
                                out=dbg_o["g0tm"][tt * TTW + c0:tt * TTW + c0 + L, :],
                                in_=g_tm[ci])
                        # groupnorm + affine + *g  (token-major)
                        mv = k2.tile([128, 16], F32, tag="gnmv", name="gnmv")
                        for h in range(H):
                            st = k2.tile([128, 6], F32, tag="gnst", name="gnst")
                            nc.vector.bn_stats(out=st, in_=ysb[:, h * HN:(h + 1) * HN])
                            nc.vector.bn_aggr(out=mv[:, 2 * h:2 * h + 2], in_=st)
                        lnv = k2.tile([128, 8], F32, tag="gnln", name="gnln")
                        var_view = bass.AP(tensor=mv.tensor, offset=mv.offset + 1,
                                           ap=[mv.ap[0], [2, 8]])
                        nc.scalar.activation(out=lnv, in_=var_view, func=AF.Ln,
                                             bias=eps_gn_t)
                        rsg = k2.tile([128, 8], F32, tag="gnrs", name="gnrs")
                        nc.scalar.activation(out=rsg, in_=lnv, func=AF.Exp, scale=-0.5)
                        for h in range(H):
                            nc.vector.tensor_scalar(
                                out=ysb[:, h * HN:(h + 1) * HN],
                                in0=ysb[:, h * HN:(h + 1) * HN],
                                scalar1=mv[:, 2 * h:2 * h + 1],
                                scalar2=rsg[:, h:h + 1],
                                op0=OP.subtract, op1=OP.mult)
                        nc.vector.tensor_mul(out=ysb, in0=ysb, in1=lnxt[:, 0:512])
                        nc.vector.tensor_add(out=ysb, in0=ysb, in1=lnxt[:, 512:1024])
                        nc.vector.tensor_mul(out=ysb, in0=ysb, in1=g_tm[ci])
                        for i in range(CT):
                            trp = psB.tile([128, L], F32, tag="pw", name="pw")
                            nc.tensor.transpose(out=trp,
                                                in_=ysb[:, i * 128:(i + 1) * 128],
                                                identity=ident)
                            nc.scalar.activation(out=ztc[i][:, c0:c0 + L], in_=trp,
                                                 func=AF.Copy)
                    # ---- Wo ----
                    accs = [psA.tile([128, TTW], F32, tag=f"acc{m}",
                                     name=f"acc{m}") for m in range(4)]
                    for i in range(CT):
                        for m in range(4):
                            nc.tensor.matmul(out=accs[m],
                                             lhsT=wto[:, i, m * 128:(m + 1) * 128],
                                             rhs=ztc[i], start=(i == 0),
                                             stop=(i == CT - 1))
                    for m in range(4):
                        nc.vector.tensor_add(out=xb[m][:, sl], in0=xb[m][:, sl],
                                             in1=accs[m])
                    if dbg and l == 0:
                        for i in range(CT):
                            nc.sync.dma_start(
                                out=dbg_o["xbtm0"][i * 128:(i + 1) * 128, sl],
                                in_=xb[i][:, sl].bitcast(F32))

                # ================= channel mix =================
                sendh = [dp.tile([C, TTW], BF16, tag=f"send{h}", name=f"send{h}")
                         for h in range(NTT)]
                recvh = [dp.tile([2 * C, TTW], BF16, tag=f"recv{h}", name=f"recv{h}")
                         for h in range(NTT)]
                for tt in range(NTT):
                    sl = slice(tt * TTW, (tt + 1) * TTW)
                    # prefetch cm_Wr weights under the ln/shift compute
                    wtr = load_wblk(cmrg[l][:, 0:512], [128, CT, 512])
                    xbs = [xb[i][:, sl] for i in range(CT)]
                    bc0, bc1 = ln_stats(xbs)
                    xc = [kp.tile([128, TTW], F32, tag=f"xt{i}", name=f"xt{i}") for i in range(CT)]
                    ln_apply(xbs, bc0, bc1, vecs, V_LN2W, V_LN2B, xc)
                    xx2 = [kp.tile([128, TTW], F32, tag=f"xx{i}", name=f"xx{i}") for i in range(CT)]
                    for i in range(CT):
                        nc.vector.tensor_sub(out=xx2[i][:, 1:TTW],
                                             in0=xc[i][:, 0:TTW - 1],
                                             in1=xc[i][:, 1:TTW])
                        nc.vector.scalar_tensor_tensor(
                            out=xx2[i][:, 0:1], in0=carry2[i], scalar=1.0,
                            in1=xc[i][:, 0:1], op0=OP.mult, op1=OP.subtract)
                        nc.gpsimd.tensor_copy(out=carry2[i], in_=xc[i][:, TTW - 1:TTW])
                    xk2 = [kp.tile([128, TTW], BF16, tag=f"xk2{i}", name=f"xk2{i}") for i in range(CT)]
                    xr2 = [kp.tile([128, TTW], BF16, tag=f"xr2{i}", name=f"xr2{i}") for i in range(CT)]
                    for i in range(CT):
                        nc.vector.scalar_tensor_tensor(
                            out=xk2[i], in0=xx2[i], scalar=vcol(vecs, i, V_CMK),
                            in1=xc[i], op0=OP.mult, op1=OP.add)
                        nc.vector.scalar_tensor_tensor(
                            out=xr2[i], in0=xx2[i], scalar=vcol(vecs, i, V_CMR),
                            in1=xc[i], op0=OP.mult, op1=OP.add)
                    # cm_Wr -> sigmoid
                    accs = [psA.tile([128, TTW], F32, tag=f"acc{m}", name=f"acc{m}") for m in range(4)]
                    for i in range(CT):
                        for m in range(4):
                            nc.tensor.matmul(out=accs[m],
                                             lhsT=wtr[:, i, m * 128:(m + 1) * 128],
                                             rhs=xr2[i], start=(i == 0),
                                             stop=(i == CT - 1))
                    sig = [kp.tile([128, TTW], F32, tag=f"gtm{m}", name=f"sig{m}") for m in range(4)]
                    for m in range(4):
                        nc.scalar.activation(out=sig[m], in_=accs[m], func=AF.Sigmoid)
                    # kk loop with cm_Wv accumulation
                    accs = [psA.tile([128, TTW], F32, tag=f"acc{m}", name=f"acc{m}") for m in range(4)]
                    wfq = wvq = None
                    for f in range(NFF):
                        fq, fr2 = f // 4, f % 4
                        nq = min(4, NFF - 4 * fq)
                        if fr2 == 0:
                            wfq = wp.tile([128, nq, CT, 128], BF16, tag="wblk",
                                          name="wfq", bufs=2)
                            nc.sync.dma_start(
                                out=wfq,
                                in_=cmkp[l][4 * fq:4 * fq + nq]
                                .rearrange("f (k p) n -> p f k n", p=128))
                            wvq = wp.tile([128, nq, C], BF16, tag="wblk",
                                          name="wvq", bufs=2)
                            nc.sync.dma_start(
                                out=wvq,
                                in_=cmvp[l][4 * fq * 128:(4 * fq + nq) * 128, :]
                                .rearrange("(f p) n -> p f n", p=128))
                        kkp = psB.tile([128, TTW], F32, tag="pw", name="pw")
                        for i in range(CT):
                            nc.tensor.matmul(out=kkp, lhsT=wfq[:, fr2, i, :],
                                             rhs=xk2[i],
                                             start=(i == 0), stop=(i == CT - 1))
                        rl = k2.tile([128, TTW], F32, tag="lnt1", name="lnt1")
                        nc.scalar.activation(out=rl, in_=kkp, func=AF.Relu)
                        kkf = k2.tile([128, TTW], BF16, tag="lnt2k", name="lnt2k")
                        nc.scalar.activation(out=kkf, in_=rl, func=AF.Square)
                        for m in range(4):
                            nc.tensor.matmul(out=accs[m],
                                             lhsT=wvq[:, fr2, m * 128:(m + 1) * 128],
                                             rhs=kkf, start=(f == 0),
                                             stop=(f == NFF - 1))
                    for m in range(4):
                        nc.vector.tensor_mul(out=sig[m], in0=sig[m], in1=accs[m])
                        nc.vector.tensor_add(out=xb[m][:, sl], in0=xb[m][:, sl],
                                             in1=sig[m])
                    # fire this half's exchange as soon as it is final: own
                    # slice tt lands (time-reversed) in the partner's half
                    # NTT-1-tt, so its AllGather overlaps the next tt's
                    # compute instead of serializing at the layer boundary.
                    oh = NTT - 1 - tt
                    for i in range(CT):
                        rv = k2.tile([128, TTW], BF16, tag="revst", name="revst")
                        nc.vector.tensor_scalar_mul(
                            out=rv, in0=_revap(xb[i][:, sl].bitcast(F32)),
                            scalar1=1.0)
                        nc.sync.dma_start(out=sendh[oh][i * 128:(i + 1) * 128, :],
                                          in_=rv)
                    if solo:
                        for i in range(CT):
                            nc.sync.dma_start(
                                out=recvh[tt][i * 128:(i + 1) * 128, :],
                                in_=xb[i][:, sl].bitcast(F32))
                            nc.sync.dma_start(
                                out=recvh[tt][C + i * 128:C + (i + 1) * 128, :],
                                in_=xb[i][:, sl].bitcast(F32))
                    else:
                        nc.gpsimd.collective_compute(
                            "AllGather", OP.bypass, replica_groups=groups,
                            ins=[sendh[oh].opt()], outs=[recvh[oh].opt()])
                if dbg and l == 0:
                    for i in range(CT):
                        nc.sync.dma_start(out=dbg_o["p0"][i * 128:(i + 1) * 128, :],
                                          in_=xb[i].bitcast(F32))

                # ================= join =================
                # tt=NTT-1 first: its recv half was AllGathered right after
                # the first chanmix slice, so it never waits on the wire.
                for tt in reversed(range(NTT)):
                    sl = slice(tt * TTW, (tt + 1) * TTW)
                    # conv residue, all ct (fp32 accumulation, bf16 final for matmul)
                    cv32 = [kp.tile([128, TTW], F32, tag=f"ksb{i}", name=f"ksb{i}") for i in range(CT)]
                    cv = [k2.tile([128, TTW], BF16, tag=f"cvb{i}", name=f"cvb{i}") for i in range(CT)]
                    a = tt * TTW
                    for i in range(CT):
                        nc.vector.tensor_scalar_mul(out=cv32[i], in0=xres[i][:, sl],
                                                    scalar1=vcol(vecs, i, V_CW1))
                        lo = 1 if tt == 0 else 0
                        nc.vector.scalar_tensor_tensor(
                            out=cv32[i][:, lo:TTW],
                            in0=xres[i][:, a + lo - 1:a + TTW - 1],
                            scalar=vcol(vecs, i, V_CW0),
                            in1=cv32[i][:, lo:TTW], op0=OP.mult, op1=OP.add)
                        hi = TTW - 1 if tt == NTT - 1 else TTW
                        nc.vector.scalar_tensor_tensor(
                            out=cv[i][:, 0:hi],
                            in0=xres[i][:, a + 1:a + hi + 1],
                            scalar=vcol(vecs, i, V_CW2),
                            in1=cv32[i][:, 0:hi], op0=OP.mult, op1=OP.add)
                        if hi < TTW:
                            nc.scalar.activation(out=cv[i][:, hi:TTW],
                                                 in_=cv32[i][:, hi:TTW], func=AF.Copy)
                    accs = [psA.tile([128, TTW], F32, tag=f"acc{m}", name=f"acc{m}") for m in range(4)]
                    wt = load_wblk(cmrg[l][:, 512:1024], [128, CT, 512])
                    for i in range(CT):
                        for m in range(4):
                            nc.tensor.matmul(out=accs[m],
                                             lhsT=wt[:, i, m * 128:(m + 1) * 128],
                                             rhs=cv[i], start=(i == 0),
                                             stop=(i == CT - 1))
                    for m in range(4):
                        jr0 = kp.tile([128, TTW], BF16, tag="vtm0", name="vtm0")
                        jr1 = kp.tile([128, TTW], BF16, tag="vtm1", name="vtm1")
                        nc.sync.dma_start(out=jr0,
                                          in_=recvh[tt][m * 128:(m + 1) * 128, :])
                        nc.sync.dma_start(out=jr1,
                                          in_=recvh[tt][C + m * 128:C + (m + 1) * 128, :])
                        nc.vector.tensor_scalar_mul(
                            out=jr0, in0=jr0, scalar1=selt[:, S_ALPHA:S_ALPHA + 1])
                        nc.vector.scalar_tensor_tensor(
                            out=jr1, in0=jr1, scalar=selt[:, S_BETA:S_BETA + 1],
                            in1=jr0, op0=OP.mult, op1=OP.add)
                        sg = k2.tile([128, TTW], F32, tag="er0", name="er0", bufs=2)
                        nc.scalar.activation(out=sg, in_=accs[m], func=AF.Sigmoid,
                                             scale=selt[:, S_NEGS:S_NEGS + 1],
                                             bias=vcol(vecs, m, V_GBM))  # w_recv
                        wown = kp.tile([128, TTW], F32, tag="vtm3", name="vtm3")
                        nc.vector.tensor_scalar(out=wown, in0=sg, scalar1=-1.0,
                                                scalar2=1.0, op0=OP.mult, op1=OP.add)
                        jsum = kp.tile([128, TTW], F32, tag="vtm2", name="vtm2")
                        nc.vector.tensor_mul(out=jsum, in0=wown, in1=xb[m][:, sl])
                        nc.vector.tensor_mul(out=jr1, in0=sg, in1=jr1)
                        nc.vector.tensor_add(out=jsum, in0=jsum, in1=jr1)
                        # note: host mask05 already includes the 2x factor fold:
                        # mask05 = mask (not 0.5*mask) since sigmoid form used.
                        nc.vector.tensor_mul(out=xres[m][:, sl], in0=jsum,
                                             in1=maskt[:, sl])
                        if l == NL - 1:
                            # stream the final output per joined slice instead
                            # of waiting for the whole tensor at the end
                            nc.sync.dma_start(
                                out=xout[m * 128:(m + 1) * 128, sl],
                                in_=xres[m][:, sl].bitcast(F32))
    nc.compile()
    return nc


def _host_inputs(inputs):
    x = np.asarray(inputs["x"], np.float32)
    lengths = np.asarray(inputs["lengths"]).astype(np.int64)
    pos = np.arange(T, dtype=np.float32)[:, None]
    div = np.exp(np.arange(0, C, 2, dtype=np.float32) * (-np.log(10000.0) / C))
    pe = np.zeros((T, C), np.float32)
    pe[:, 0::2] = np.sin(pos * div)
    pe[:, 1::2] = np.cos(pos * div)
    mask = (np.arange(T)[None, :] < lengths[:, None]).astype(np.float32)

    consts = np.zeros((128, 384), np.float32)
    consts[:, 0:128] = np.eye(128, dtype=np.float32)
    consts[:, 128:256] = np.triu(np.ones((128, 128), np.float32), 1)
    consts[:, 256:384] = 1.0

    gw = np.asarray(inputs["gate_w"], np.float32)
    gb = np.asarray(inputs["gate_b"], np.float32)
    cw = np.asarray(inputs["conv_w"], np.float32)
    cb = np.asarray(inputs["conv_b"], np.float32)

    in_maps = []
    for c in range(8):
        b, d = c % 4, c // 4
        rev = d == 1
        s = -1.0 if rev else 1.0
        xin = (x[b] + pe)
        mrow = mask[b]
        if rev:
            xin = xin[::-1]
            mrow = mrow[::-1]
        m = {
            "x0": np.ascontiguousarray(xin.T),
            "mask05": np.ascontiguousarray(np.broadcast_to(mrow, (128, T))),
            "consts": consts,
        }
        sel = np.zeros((128, 8), np.float32)
        sel[:, S_LN0] = 0.0 if rev else 1.0
        sel[:, S_NEGS] = -s
        sel[:, S_ALPHA] = 1.0 if rev else 0.0
        sel[:, S_BETA] = 0.0 if rev else 1.0
        m["sel"] = sel
        for l in range(NL):
            W = {k: np.asarray(inputs[k], np.float32)[d, l]
                 for k in ["ln1_w", "ln1_b", "ln2_w", "ln2_b", "maa_x", "maa_w",
                           "maa_k", "maa_v", "maa_r", "maa_g", "tm_w1", "tm_w2",
                           "td_w1", "td_w2", "time_decay", "Wr", "Wk", "Wv",
                           "Wg", "Wo", "lnx_w", "lnx_b", "cm_maa_k", "cm_maa_r",
                           "cm_Wk", "cm_Wv", "cm_Wr", "time_faaaa"]}
            import ml_dtypes
            bf16 = ml_dtypes.bfloat16
            m[f"wbig{l}"] = np.ascontiguousarray(np.concatenate(
                [W["Wr"], W["Wk"], W["Wv"], W["Wg"], W["tm_w1"], W["td_w1"],
                 W["Wo"]], axis=1).astype(bf16))
            m[f"tmw2{l}"] = np.ascontiguousarray(
                W["tm_w2"].reshape(5 * TM, C).astype(bf16))
            m[f"tdw2{l}"] = np.ascontiguousarray(W["td_w2"].astype(bf16))
            m[f"cmk{l}"] = np.ascontiguousarray(
                W["cm_Wk"].reshape(C, NFF, 128).transpose(1, 0, 2).astype(bf16))
            m[f"cmv{l}"] = np.ascontiguousarray(W["cm_Wv"].astype(bf16))
            m[f"cmrg{l}"] = np.ascontiguousarray(
                np.concatenate([W["cm_Wr"], gw[l]], axis=1).astype(bf16))
            cwe = cw[l] if not rev else cw[l][:, ::-1]
            gbe = cb[l] @ gw[l] + gb[l]
            vec = np.zeros((C, NV), np.float32)
            vec[:, V_LN1W] = W["ln1_w"]; vec[:, V_LN1B] = W["ln1_b"]
            vec[:, V_LN2W] = W["ln2_w"]; vec[:, V_LN2B] = W["ln2_b"]
            vec[:, V_MAAX] = W["maa_x"]; vec[:, V_MAAW] = W["maa_w"]
            vec[:, V_MAAK] = W["maa_k"]; vec[:, V_MAAV] = W["maa_v"]
            vec[:, V_MAAR] = W["maa_r"]; vec[:, V_MAAG] = W["maa_g"]
            vec[:, V_TDCY] = W["time_decay"]
            vec[:, V_CMK] = W["cm_maa_k"]; vec[:, V_CMR] = W["cm_maa_r"]
            vec[:, V_GBM] = -s * gbe
            vec[:, V_CW0] = cwe[:, 0]
            vec[:, V_CW1] = cwe[:, 1] - 1.0
            vec[:, V_CW2] = cwe[:, 2]
            vec[:, V_LN0W] = np.asarray(inputs["ln0_w"], np.float32)
            vec[:, V_LN0B] = np.asarray(inputs["ln0_b"], np.float32)
            m[f"vecs{l}"] = vec
            lnx = np.zeros((128, 1024), np.float32)
            lnx[:, 0:512] = W["lnx_w"][None, :]
            lnx[:, 512:1024] = W["lnx_b"][None, :]
            m[f"lnx{l}"] = lnx
            u = W["time_faaaa"].reshape(C)
            hmu = np.zeros((C, 8), np.float32)
            for h in range(H):
                hmu[h * HN:(h + 1) * HN, h] = u[h * HN:(h + 1) * HN]
            m[f"hmu{l}"] = hmu.astype(bf16)
        in_maps.append(m)
    return in_maps


def kernel(**inputs):
    if "nc" not in _CACHE:
        _CACHE["nc"] = _build(dbg=False)
    nc = _CACHE["nc"]
    in_maps = _host_inputs(inputs)
    res = run_bass_kernel_spmd(nc, in_maps, list(range(8)))
    out = np.empty((B, T, C), np.float32)
    for b in range(B):
        out[b] = res.results[b]["xout"].T
    return out


if __name__ == "__main__":
    rng = np.random.default_rng(0)
    demo = None

